# revision 1
# baseline (speedup 1.0000x reference)
"""Causal self-attention Trainium2 kernel.

Sharding: 8 cores = 4 batches x 2 head-groups (8 heads each).
Per-core dataflow (everything "transposed" so the contraction dim sits on
SBUF partitions):
  qT/kT  [64*2, T]  = Wq/Wk-slice.T-as-lhsT  @ xT          (fp32r matmuls)
  V      [T, 64*8]  = xT-as-lhsT @ Wv-slice, stored per (head, ktile) as
                      [128, 65] with a ones column (row-sum trick)
  ST     [k, q]     = kT-as-lhsT @ qT   (two heads row-packed in PE array)
  PT     = exp(ST/8) with causal lower-left structure (upper blocks skipped,
           diagonal blocks masked on GPSIMD)
  OT[65, q] += Vaug-as-lhsT @ PT        (row 64 accumulates softmax sums)
  outT   = OT[0:64] * bcast(1/OT[64])
  y      = outT-as-lhsT @ Wo-row-slice  (partial; host sums the 2 head-groups)
"""
import numpy as np

B, T, D, H = 4, 2048, 1024, 16
HD = D // H            # 64
NCORES = 8
HPC = 8                # heads per core
FPC = HPC * HD         # 512 feature cols per core
NPAIR = HPC // 2       # 4 head pairs
ND = D // 128          # 8 contraction tiles
KT = T // 128          # 16 k-tiles
NCH = T // 512         # 4 q-chunks

_CACHE = {}


def _build(phases="BCD"):
    import concourse.mybir as mybir
    import concourse.tile as tile
    from concourse import bacc
    from contextlib import ExitStack

    f32 = mybir.dt.float32
    f32r = mybir.dt.float32r
    Exp = mybir.ActivationFunctionType.Exp

    nc = bacc.Bacc("TRN2", target_bir_lowering=False, debug=False,
                   num_devices=NCORES)
    # xT repacked host-side as [chunk, dtile, 128, 512]; Wq/Wk as
    # [pair, 128, 1024] so every load is a contiguous-row DMA
    xT = nc.dram_tensor("xT", [NCH, ND, 128, 512], f32r, kind="ExternalInput")
    Wq = nc.dram_tensor("Wq", [NPAIR, 128, D], f32r, kind="ExternalInput")
    Wk = nc.dram_tensor("Wk", [NPAIR, 128, D], f32r, kind="ExternalInput")
    Wv = nc.dram_tensor("Wv", [D, FPC], f32r, kind="ExternalInput")
    bq = nc.dram_tensor("bq", [FPC], f32, kind="ExternalInput")
    bk = nc.dram_tensor("bk", [FPC], f32, kind="ExternalInput")
    bv = nc.dram_tensor("bv", [FPC], f32, kind="ExternalInput")
    Wo = nc.dram_tensor("Wo", [FPC, D], f32r, kind="ExternalInput")
    y = nc.dram_tensor("y", [T, D], f32, kind="ExternalOutput")

    VSTR = KT * 65     # per-head stride in vaug free dim

    with tile.TileContext(nc) as tc, ExitStack() as es:
        pers = es.enter_context(tc.tile_pool(name="pers", bufs=1))
        qkp = es.enter_context(tc.tile_pool(name="qkp", bufs=2))
        oTp = es.enter_context(tc.tile_pool(name="oTp", bufs=1))
        wqkp = es.enter_context(tc.tile_pool(name="wqkp", bufs=2))
        xsp = es.enter_context(tc.tile_pool(name="xsp", bufs=2))
        ptp = es.enter_context(tc.tile_pool(name="ptp", bufs=4))
        nrm = es.enter_context(tc.tile_pool(name="nrm", bufs=2))
        obp = es.enter_context(tc.tile_pool(name="obp", bufs=6))
        psctx = ExitStack()
        psA = psctx.enter_context(tc.tile_pool(name="psA", bufs=2, space="PSUM"))
        psS = psctx.enter_context(tc.tile_pool(name="psS", bufs=2, space="PSUM"))
        psO = psctx.enter_context(tc.tile_pool(name="psO", bufs=1, space="PSUM"))

        vaug = pers.tile([128, HPC * VSTR], f32r, tag="vaug")
        bq_sb = pers.tile([128, NPAIR], f32, tag="bq")
        bk_sb = pers.tile([128, NPAIR], f32, tag="bk")
        bv_row = pers.tile([1, FPC], f32, tag="bvr")
        bv_bc = pers.tile([128, FPC], f32, tag="bvb")
        outT = [oTp.tile([128, T], f32r, tag=f"outT{p}", name=f"outT{p}")
                for p in range(NPAIR)]

        nc.sync.dma_start(bq_sb[:], bq[:].rearrange("(t p) -> p t", p=128))
        nc.sync.dma_start(bk_sb[:], bk[:].rearrange("(t p) -> p t", p=128))
        nc.sync.dma_start(bv_row[:], bv[:])
        nc.gpsimd.partition_broadcast(bv_bc[:], bv_row[:])
        # ones columns of vaug (col 64 of each [128,65] block); memset can't
        # encode f32r so memset f32 then cast-copy on DVE
        ones_f32 = pers.tile([128, KT], f32, tag="ones1", name="ones_f32")
        nc.vector.memset(ones_f32[:], 1.0)
        for h in range(HPC):
            ones_ap = vaug[:, VSTR * h:VSTR * (h + 1)].rearrange(
                "p (k x) -> p k x", x=65)[:, :, 64:65]
            nc.vector.tensor_copy(ones_ap, ones_f32[:].rearrange(
                "p (k x) -> p k x", x=1))

        with tc.tile_pool(name="wvp", bufs=1) as wvp:
            wv_sb = [wvp.tile([128, FPC], f32r, tag=f"wv{d}", name=f"wv{d}")
                     for d in range(ND)]
            wv_loaded = False

            for p in range(NPAIR):
                # ---- per-pair qkv projections, x streamed in quarters ----
                wq_p = wqkp.tile([128, D], f32r, tag="wq", name="wq_p")
                wk_p = wqkp.tile([128, D], f32r, tag="wk", name="wk_p")
                nc.sync.dma_start(wq_p[:], Wq[p])
                nc.sync.dma_start(wk_p[:], Wk[p])
                qTp = qkp.tile([128, T], f32r, tag="qT", name="qTp")
                kTp = qkp.tile([128, T], f32r, tag="kT", name="kTp")
                for ch in range(NCH):
                    xs = [xsp.tile([128, 512], f32r, tag=f"xs{d}",
                                   name=f"xs{d}") for d in range(ND)]
                    for d in range(ND):
                        nc.sync.dma_start(xs[d][:], xT[ch, d])
                    if not wv_loaded:
                        wv_loaded = True
                        for d in range(ND):
                            nc.sync.dma_start(wv_sb[d][:],
                                              Wv[128 * d:128 * (d + 1), :])
                    for (wt, bsb, dst) in ((wq_p, bq_sb, qTp),
                                           (wk_p, bk_sb, kTp)):
                        ps = psA.tile([128, 512], f32, tag="psA", name="psqk")
                        for d in range(ND):
                            nc.tensor.matmul(
                                ps[:], wt[:, 128 * d:128 * (d + 1)],
                                xs[d][:], start=(d == 0), stop=(d == ND - 1))
                        nc.vector.tensor_scalar_add(
                            dst[:, 512 * ch:512 * (ch + 1)],
                            ps[:], bsb[:, p:p + 1])
                    if p == 0:
                        # V for all 8 heads, using this quarter's x
                        for tj in range(4 * ch, 4 * (ch + 1)):
                            ps = psA.tile([128, FPC], f32, tag="psA",
                                          name="psv")
                            lo = (tj - 4 * ch) * 128
                            for d in range(ND):
                                nc.tensor.matmul(
                                    ps[:], xs[d][:, lo:lo + 128], wv_sb[d][:],
                                    start=(d == 0), stop=(d == ND - 1))
                            dst = vaug[:].rearrange("p (h z) -> p h z",
                                                    h=HPC)[
                                :, :, 65 * tj:65 * tj + HD]
                            srcv = ps[:].rearrange("p (h x) -> p h x", h=HPC)
                            bsrc = bv_bc[:].rearrange("p (h x) -> p h x",
                                                      h=HPC)
                            nc.vector.tensor_add(dst, srcv, bsrc)

                # ---- attention for this pair ----
                if "C" not in phases:
                    continue
                for c in range(NCH):
                    qs = 512 * c
                    nk = 4 * c + 4
                    po = [psO.tile([65, 512], f32, tag=f"psO{h}",
                                   name=f"po{h}") for h in range(2)]
                    for i in range(nk):
                        dg = i - 4 * c
                        qo = max(dg, 0) * 128
                        # both heads' scores into one 2-bank psum tile
                        ss = psS.tile([128, 1024], f32, tag="psS", name="ss")
                        pt = ptp.tile([128, 1024], f32r, tag="pt", name="pt")
                        # fp32r runs 4 cyc/row when N<256: widen narrow
                        # diagonal score matmuls to 256 cols (the extra
                        # columns land in psum but exp never reads them)
                        qow = min(qo, 256)
                        for h in range(2):
                            r0, r1 = 64 * h, 64 * (h + 1)
                            nc.tensor.matmul(
                                ss[:, 512 * h + qow:512 * (h + 1)],
                                kTp[r0:r1, 128 * i:128 * (i + 1)],
                                qTp[r0:r1, qs + qow:qs + 512],
                                start=True, stop=True)
                        sv = ss[:].rearrange("p (s x) -> p s x", s=2)[
                            :, :, qo:]
                        pv = pt[:].rearrange("p (s x) -> p s x", s=2)[
                            :, :, qo:]
                        nc.scalar.activation(pv, sv, Exp, scale=0.125)
                        if dg >= 0:
                            # one affine covers [qow, qo+128): cond
                            # f' - (qo-qow) - p >= 0 zero-fills the widened
                            # [qow,qo) zone and masks the diagonal triangle,
                            # so the PV rhs below can start at qow (N>=256,
                            # full fp32r rate); the widened output columns
                            # accumulate exact zeros.
                            mw = (qo - qow) + 128
                            blk = pt[:].rearrange(
                                "p (s x) -> p s x", s=2)[:, :, qow:qow + mw]
                            nc.gpsimd.affine_select(
                                out=blk, in_=blk,
                                compare_op=mybir.AluOpType.is_ge,
                                fill=0.0, base=-(qo - qow),
                                pattern=[[0, 2], [1, mw]],
                                channel_multiplier=-1)
                        for h in range(2):
                            hh = 2 * p + h
                            nc.tensor.matmul(
                                po[h][:, qow:],
                                vaug[:, VSTR * hh + 65 * i:
                                     VSTR * hh + 65 * i + 65],
                                pt[:, 512 * h + qow:512 * (h + 1)],
                                start=(i == 0), stop=(i == nk - 1))
                    for h in range(2):
                        r0, r1 = 64 * h, 64 * (h + 1)
                        # single cheap copy frees the psO bank; the 3-op
                        # normalize chain then runs off-bank
                        og = nrm.tile([65, 512], f32, tag="og", name="og")
                        nc.vector.tensor_copy(og[:], po[h][:])
                        rec = nrm.tile([1, 512], f32, tag="rec", name="rec")
                        nc.vector.reciprocal(rec[:], og[64:65, :])
                        bc = nrm.tile([64, 512], f32, tag="bc", name="bc")
                        nc.gpsimd.partition_broadcast(bc[:], rec[:])
                        nc.vector.tensor_mul(
                            outT[p][r0:r1, qs:qs + 512],
                            og[0:64, :], bc[:])

        psctx.close()
        # ================= output projection =================
        if "D" not in phases:
            return nc
        with tc.tile_pool(name="wo", bufs=1) as wop, \
             tc.tile_pool(name="psD", bufs=6, space="PSUM") as psD:
            wo_sb = [wop.tile([128, D], f32r, tag=f"wo{p}", name=f"wo{p}")
                     for p in range(NPAIR)]
            for p in range(NPAIR):
                nc.sync.dma_start(wo_sb[p][:], Wo[128 * p:128 * (p + 1), :])
            for tj in range(KT):
                for n in range(2):
                    ps = psD.tile([128, 512], f32, tag="psD", name="pspr")
                    for p in range(NPAIR):
                        nc.tensor.matmul(
                            ps[:],
                            outT[p][:, 128 * tj:128 * (tj + 1)],
                            wo_sb[p][:, 512 * n:512 * (n + 1)],
                            start=(p == 0), stop=(p == NPAIR - 1))
                    ob = obp.tile([128, 512], f32, tag="ob", name="ob")
                    nc.vector.tensor_copy(ob[:], ps[:])
                    nc.sync.dma_start(
                        y[128 * tj:128 * (tj + 1),
                          512 * n:512 * (n + 1)], ob[:])
    nc.compile()
    return nc


class _Runner:
    def __init__(self, nc):
        import jax
        from jax.sharding import Mesh, PartitionSpec, NamedSharding
        from jax.experimental.shard_map import shard_map
        import concourse.mybir as mybir
        from concourse.bass2jax import (_bass_exec_p, partition_id_tensor,
                                        install_neuronx_cc_hook)
        install_neuronx_cc_hook()
        self.jax = jax
        part = nc.partition_id_tensor.name if nc.partition_id_tensor else None
        in_names, out_names, out_avals = [], [], []
        for alloc in nc.m.functions[0].allocations:
            if not isinstance(alloc, mybir.MemoryLocationSet):
                continue
            name = alloc.memorylocations[0].name
            if alloc.kind == "ExternalInput":
                if name != part:
                    in_names.append(name)
            elif alloc.kind == "ExternalOutput":
                out_names.append(name)
                out_avals.append(jax.core.ShapedArray(
                    tuple(alloc.tensor_shape), mybir.dt.np(alloc.dtype)))
        self.in_names, self.out_names, self.out_avals = in_names, out_names, out_avals
        all_in = list(in_names) + list(out_names) + ([part] if part else [])

        def _body(*args):
            ops = list(args)
            if part:
                ops.append(partition_id_tensor())
            return tuple(_bass_exec_p.bind(
                *ops, out_avals=tuple(out_avals), in_names=tuple(all_in),
                out_names=tuple(out_names), lowering_input_output_aliases=(),
                sim_require_finite=True, sim_require_nnan=True, nc=nc))

        devices = jax.devices()[:NCORES]
        mesh = Mesh(np.asarray(devices), ("core",))
        nin = len(in_names) + len(out_names)
        self.fn = jax.jit(
            shard_map(_body, mesh=mesh,
                      in_specs=(PartitionSpec("core"),) * nin,
                      out_specs=(PartitionSpec("core"),) * len(out_names),
                      check_rep=False),
            keep_unused=True)
        self.sharding = NamedSharding(mesh, PartitionSpec("core"))

    def put_inputs(self, in_maps):
        args = []
        for name in self.in_names:
            cat = np.concatenate([np.asarray(m[name]) for m in in_maps], axis=0)
            args.append(self.jax.device_put(cat, self.sharding))
        for av in self.out_avals:
            z = np.zeros((NCORES * av.shape[0], *av.shape[1:]), av.dtype)
            args.append(self.jax.device_put(z, self.sharding))
        return args

    def run_np(self, args):
        outs = self.fn(*args)
        return [
            {n: np.asarray(outs[i]).reshape(NCORES, *self.out_avals[i].shape)[c]
             for i, n in enumerate(self.out_names)}
            for c in range(NCORES)
        ]


def _get_runner():
    if "r" not in _CACHE:
        nc = _build()
        _CACHE["nc"] = nc
        _CACHE["r"] = _Runner(nc)
    return _CACHE["r"]


def _rne11(a):
    """Round fp32 to 11 mantissa bits, round-to-nearest-even (= hw fp32r)."""
    ai = np.ascontiguousarray(a, dtype=np.float32).view(np.uint32).astype(np.uint64)
    lsb = (ai >> 12) & 1
    out = (((ai + 2047 + lsb) >> 12) << 12).astype(np.uint32)
    return out.view(np.float32)


def make_in_maps(x, Wqkv, bqkv, Wo, bo=None, mask=None):
    in_maps = []
    for c in range(NCORES):
        b, g = c // 2, c % 2
        sl = slice(g * FPC, (g + 1) * FPC)
        wqs = Wqkv[:, 0 * D:1 * D][:, sl].reshape(ND, 128, NPAIR, 128)
        wks = Wqkv[:, 1 * D:2 * D][:, sl].reshape(ND, 128, NPAIR, 128)
        in_maps.append({
            "xT": _rne11(x[b].reshape(NCH, 512, ND, 128).transpose(0, 2, 3, 1)),
            "Wq": _rne11(wqs.transpose(2, 1, 0, 3).reshape(NPAIR, 128, D)),
            "Wk": _rne11(wks.transpose(2, 1, 0, 3).reshape(NPAIR, 128, D)),
            "Wv": _rne11(Wqkv[:, 2 * D:3 * D][:, sl]),
            "bq": np.ascontiguousarray(bqkv[0 * D:1 * D][sl]),
            "bk": np.ascontiguousarray(bqkv[1 * D:2 * D][sl]),
            "bv": np.ascontiguousarray(bqkv[2 * D:3 * D][sl]),
            "Wo": _rne11(Wo[sl, :]),
        })
    return in_maps


def kernel(x, Wqkv, bqkv, Wo, bo, mask=None, **_unused):
    x = np.asarray(x, dtype=np.float32)
    Wqkv = np.asarray(Wqkv, dtype=np.float32)
    bqkv = np.asarray(bqkv, dtype=np.float32)
    Wo = np.asarray(Wo, dtype=np.float32)
    bo = np.asarray(bo, dtype=np.float32)
    in_maps = make_in_maps(x, Wqkv, bqkv, Wo)
    last_err = None
    for _attempt in range(3):
        try:
            r = _get_runner()
            args = r.put_inputs(in_maps)
            res = r.run_np(args)
            break
        except Exception as e:  # transient device wedge: retry fresh
            last_err = e
            _CACHE.clear()
            import time
            time.sleep(5)
    else:
        raise last_err
    out = np.empty((B, T, D), dtype=np.float32)
    for b in range(B):
        out[b] = res[2 * b]["y"] + res[2 * b + 1]["y"] + bo
    return out



# revision 2
# speedup vs baseline: 1.1071x; 1.1071x over previous
"""Causal self-attention Trainium2 kernel, v2.

Sharding: 8 cores = 4 batches x 2 head-groups (8 heads each).

Per-core dataflow:
  - QKV projections as fp8e4 DoubleRow matmuls (256-feature contraction
    per instruction) with a hi/lo split of both x and W (host-prepared):
    q = xh@Wh + xl@Wh + xh@Wl  -- 3 DoubleRow passes = 6N cycles vs
    fp32r's 8N, with ~0.1% error.
  - q,k stored bf16 (rate-1 matmuls at any N, so causal diagonal blocks
    need no 256-col widening); scores per k-tile into PSUM.
  - exp on ACT writes P^T directly as bf16; causal triangle zeroed on
    GPSIMD (affine_select); PV matmuls in bf16 with a ones-column in the
    V tile accumulating softmax denominators.
  - out = PV / rowsum via DVE reciprocal + GPSIMD partition broadcast.
  - y = outT.T @ Wo in fp32r (partial; host sums the 2 head-groups).

Scheduling: projection chains for chunk ch+1 and output-projection tiles
for chunk ch-1 are interleaved between attention k-tiles of chunk ch so
the PE never idles while ACT paces the softmax.
"""
import numpy as np

B, T, D, H = 4, 2048, 1024, 16
HD = D // H            # 64
NCORES = 8
HPC = 8                # heads per core
FPC = HPC * HD         # 512 feature cols per core
NPAIR = HPC // 2       # 4 head pairs
NG = 4                 # fp8 DoubleRow contraction groups (256 feats each)
KT = T // 128          # 16 k-tiles
NCH = T // 512         # 4 q-chunks
WSC = 50.0             # host weight scaling before fp8 quantization
VSTR = 65              # per-k-tile stride in vaug free dim
HSTR = KT * VSTR       # per-head stride in vaug free dim

_CACHE = {}


def _build():
    import concourse.mybir as mybir
    import concourse.tile as tile
    from concourse import bacc
    from contextlib import ExitStack

    f32 = mybir.dt.float32
    f32r = mybir.dt.float32r
    bf16 = mybir.dt.bfloat16
    f8 = mybir.dt.float8e4
    DR = mybir.MatmulPerfMode.DoubleRow
    Exp = mybir.ActivationFunctionType.Exp
    Alu = mybir.AluOpType

    nc = bacc.Bacc("TRN2", target_bir_lowering=False, debug=False,
                   num_devices=NCORES)
    # x hi/lo fp8, repacked host-side as [ch, 128, g, i, tok]:
    # feature = 256*g + 128*i + partition
    xh = nc.dram_tensor("xh", [NCH, 128, NG * 1024], f8, kind="ExternalInput")
    xl = nc.dram_tensor("xl", [NCH, 128, NG * 1024], f8, kind="ExternalInput")
    # fp8 weights, 6 tensors: q/k: [p][g][i][f128]; v: [g][i][f512]
    Wsec = [nc.dram_tensor(nm, [128, 4096], f8, kind="ExternalInput")
            for nm in ("Wqh", "Wql", "Wkh", "Wkl", "Wvh", "Wvl")]
    BQ = nc.dram_tensor("BQ", [128, NPAIR], f32, kind="ExternalInput")
    BK = nc.dram_tensor("BK", [128, NPAIR], f32, kind="ExternalInput")
    BV = nc.dram_tensor("BV", [FPC], f32, kind="ExternalInput")
    Wo = nc.dram_tensor("Wo", [FPC, D], f32r, kind="ExternalInput")
    y = nc.dram_tensor("y", [T, D], f32, kind="ExternalOutput")

    with tile.TileContext(nc) as tc, ExitStack() as es:
        pers = es.enter_context(tc.tile_pool(name="pers", bufs=1))
        xsp = es.enter_context(tc.tile_pool(name="xsp", bufs=2))
        ptp = es.enter_context(tc.tile_pool(name="ptp", bufs=6))
        nrm = es.enter_context(tc.tile_pool(name="nrm", bufs=2))
        obp = es.enter_context(tc.tile_pool(name="obp", bufs=3))
        psA = es.enter_context(tc.tile_pool(name="psA", bufs=2, space="PSUM"))
        psS = es.enter_context(tc.tile_pool(name="psS", bufs=2, space="PSUM"))
        psO = es.enter_context(tc.tile_pool(name="psO", bufs=1, space="PSUM"))

        ws_sb = [pers.tile([128, 4096], f8, tag=f"ws{i}", name=f"ws{i}")
                 for i in range(6)]
        bq_sb = pers.tile([128, NPAIR], f32, tag="bq")
        bk_sb = pers.tile([128, NPAIR], f32, tag="bk")
        bv_row = pers.tile([1, FPC], f32, tag="bvr")
        bv_bc = pers.tile([128, FPC], f32, tag="bvb")
        wo_sb = pers.tile([128, NPAIR * D], f32r, tag="wo")
        vaug = pers.tile([128, HPC * HSTR], bf16, tag="vaug")
        qT = [pers.tile([128, T], bf16, tag=f"qT{p}", name=f"qT{p}")
              for p in range(NPAIR)]
        kT = [pers.tile([128, T], bf16, tag=f"kT{p}", name=f"kT{p}")
              for p in range(NPAIR)]
        outT = [pers.tile([128, T], f32r, tag=f"oT{p}", name=f"oT{p}")
                for p in range(NPAIR)]

        vaug4 = vaug[:].rearrange("p (h k x) -> p h k x", h=HPC, k=KT)

        xtiles = {}

        def emit_xdma(ch):
            xh_sb = xsp.tile([128, NG * 1024], f8, tag="xh", name="xh_sb")
            xl_sb = xsp.tile([128, NG * 1024], f8, tag="xl", name="xl_sb")
            if ch == 0:
                half = NG * 512
                nc.sync.dma_start(xh_sb[:, 0:half], xh[ch][:, 0:half])
                nc.sync.dma_start(xl_sb[:, 0:half], xl[ch][:, 0:half])
                nc.sync.dma_start(xh_sb[:, half:], xh[ch][:, half:])
                nc.sync.dma_start(xl_sb[:, half:], xl[ch][:, half:])
            else:
                nc.sync.dma_start(xh_sb[:], xh[ch])
                nc.sync.dma_start(xl_sb[:], xl[ch])
            xtiles[ch] = [xx[:, g * 1024:(g + 1) * 1024].rearrange(
                "p (i t) -> p i t", i=2)
                for xx in (xh_sb, xl_sb) for g in range(NG)]

        def w_ap(sec, p, g):
            base = p * 1024 + g * 256
            return ws_sb[sec][:, base:base + 256].rearrange(
                "p (i f) -> p i f", i=2)

        def wv_ap(sec, g):
            base = g * 1024
            return ws_sb[sec][:, base:base + 1024].rearrange(
                "p (i f) -> p i f", i=2)

        # (x-part, w-section) term order: both Wh terms first so the Wl
        # DMA can trail the Wh one at startup
        def proj_qk(ch, p, sec, bsb, dst):
            xg = xtiles[ch]
            ps = psA.tile([128, 512], f32, tag="psA", name="psqk")
            n = 0
            for (xt, ws) in ((0, sec), (1, sec), (0, sec + 1)):
                for g in range(NG):
                    nc.tensor.matmul(
                        ps[:], w_ap(ws, p, g), xg[xt * NG + g],
                        start=(n == 0), stop=(n == 3 * NG - 1),
                        perf_mode=DR)
                    n += 1
            nc.vector.tensor_scalar(
                dst[p][:, 512 * ch:512 * (ch + 1)], ps[:],
                1.0 / WSC, bsb[:, p:p + 1], Alu.mult, Alu.add)

        def proj_v(ch, tj):
            xg = xtiles[ch]
            lo = (tj - 4 * ch) * 128
            ps = psA.tile([128, FPC], f32, tag="psA", name="psv")
            n = 0
            for (xt, ws) in ((0, 4), (1, 4), (0, 5)):
                for g in range(NG):
                    nc.tensor.matmul(
                        ps[:], xg[xt * NG + g][:, :, lo:lo + 128],
                        wv_ap(ws, g),
                        start=(n == 0), stop=(n == 3 * NG - 1),
                        perf_mode=DR)
                    n += 1
            nc.vector.scalar_tensor_tensor(
                vaug4[:, :, tj, 0:64],
                ps[:].rearrange("p (h x) -> p h x", h=HPC),
                1.0 / WSC,
                bv_bc[:].rearrange("p (h x) -> p h x", h=HPC),
                Alu.mult, Alu.add)

        def proj_chunk_fill(ch):
            out = []
            for p in range(NPAIR):
                out.append(lambda p=p: proj_qk(ch, p, 0, bq_sb, qT))
            for p in range(NPAIR):
                out.append(lambda p=p: proj_qk(ch, p, 2, bk_sb, kT))
            for tj in range(4 * ch, 4 * (ch + 1)):
                out.append(lambda tj=tj: proj_v(ch, tj))
            return out

        def outproj_tile(tj):
            ob = obp.tile([128, D], f32, tag="ob", name="ob")
            for n in range(2):
                ps = psA.tile([128, 512], f32, tag="psA", name="pspr")
                for p in range(NPAIR):
                    nc.tensor.matmul(
                        ps[:],
                        outT[p][:, 128 * tj:128 * (tj + 1)],
                        wo_sb[:, p * D + 512 * n:p * D + 512 * (n + 1)],
                        start=(p == 0), stop=(p == NPAIR - 1))
                nc.vector.tensor_copy(ob[:, 512 * n:512 * (n + 1)], ps[:])
            nc.sync.dma_start(y[128 * tj:128 * (tj + 1), :], ob[:])

        def emit_norm(p, ch, po, c0, c1):
            w = c1 - c0
            for h in range(2):
                og = nrm.tile([65, 512], f32, tag="og", name="og")
                nc.vector.tensor_copy(og[:, 0:w], po[h][:, c0:c1])
                rec = nrm.tile([1, 512], f32, tag="rec", name="rec")
                nc.vector.reciprocal(rec[:, 0:w], og[64:65, 0:w])
                bc = nrm.tile([64, 512], f32, tag="bc", name="bc")
                nc.gpsimd.partition_broadcast(bc[:, 0:w], rec[:, 0:w])
                nc.vector.tensor_mul(
                    outT[p][64 * h:64 * (h + 1),
                            512 * ch + c0:512 * ch + c1],
                    og[0:64, 0:w], bc[:, 0:w])

        def attn_column(p, ch, fill, tailjobs=None):
            """fill: iterator of emitters to interleave between k-tiles."""
            po = [psO.tile([65, 512], f32, tag=f"po{h}", name=f"po{h}")
                  for h in range(2)]
            nk = 4 * ch + 4

            def emit_pv(kt, pt):
                qo = max(kt - 4 * ch, 0) * 128
                for h in range(2):
                    nc.tensor.matmul(
                        po[h][:, qo:],
                        vaug4[:, 2 * p + h, kt, :],
                        pt[:, h, qo:],
                        start=(kt == 0), stop=(kt == nk - 1),
                        skip_group_check=True)

            pend = []
            for kt in range(nk):
                dg = kt - 4 * ch
                qo = max(dg, 0) * 128
                pt = ptp.tile([128, 2, 512], bf16, tag="pt", name="pt")
                ss = psS.tile([128, 2, 512], f32, tag="psS", name="ss")
                for h in range(2):
                    r0, r1 = 64 * h, 64 * (h + 1)
                    nc.tensor.matmul(
                        ss[:, h, qo:],
                        kT[p][r0:r1, 128 * kt:128 * (kt + 1)],
                        qT[p][r0:r1, 512 * ch + qo:512 * (ch + 1)],
                        start=True, stop=True)
                nc.scalar.activation(pt[:, :, qo:], ss[:, :, qo:],
                                     Exp, scale=0.125)
                if dg >= 0:
                    # zero the upper (q < k) triangle of the diagonal block
                    blk = pt[:, :, qo:qo + 128]
                    nc.gpsimd.affine_select(
                        out=blk, in_=blk,
                        compare_op=mybir.AluOpType.is_ge,
                        fill=0.0, base=0,
                        pattern=[[0, 2], [1, 128]],
                        channel_multiplier=-1)
                # fill work, then a 2-tile-deep software-pipelined PV so
                # the PE never waits inline on an exp
                f = next(fill, None)
                if f:
                    f()
                pend.append((kt, pt))
                if len(pend) > 2:
                    emit_pv(*pend.pop(0))
            if tailjobs is None:
                for pv in pend:
                    f = next(fill, None)
                    if f:
                        f()
                    emit_pv(*pv)
                emit_norm(p, ch, po, 0, 512)
            else:
                # last column: po[:, 0:256] is complete before the last
                # two (diagonal) PVs -- normalize and project it early
                emit_pv(*pend.pop(0))
                emit_norm(p, ch, po, 0, 256)
                tailjobs[0]()
                emit_pv(*pend.pop(0))
                tailjobs[1]()
                emit_norm(p, ch, po, 256, 512)
                for j in tailjobs[2:]:
                    j()

        # ---------------- schedule ----------------
        emit_xdma(0)
        for w, dram in zip(ws_sb, Wsec):
            nc.sync.dma_start(w[:], dram[:])
        nc.sync.dma_start(bq_sb[:], BQ[:])
        nc.sync.dma_start(bk_sb[:], BK[:])
        nc.sync.dma_start(bv_row[:], BV[:])
        nc.gpsimd.partition_broadcast(bv_bc[:], bv_row[:])
        nc.vector.memset(vaug4[:, :, :, 64:65], 1.0)
        nc.sync.dma_start(wo_sb[:].rearrange("p (t c) -> p t c", t=NPAIR),
                          Wo[:].rearrange("(t p) c -> p t c", p=128))

        for f in proj_chunk_fill(0):
            f()
        for ch in range(NCH):
            if ch + 1 < NCH:
                emit_xdma(ch + 1)
            fill = []
            if ch + 1 < NCH:
                fill += proj_chunk_fill(ch + 1)
            if ch > 0:
                fill += [lambda tj=tj: outproj_tile(tj)
                         for tj in range(4 * (ch - 1), 4 * ch)]
            # spread fill over the chunk's 4*(4ch+4) k-tiles (one slot
            # every 2 tiles -> 2*(4ch+4) slots per chunk)
            it = iter(fill)
            last = NCH - 1
            for p in range(NPAIR):
                tailjobs = None
                if ch == last and p == NPAIR - 1:
                    tailjobs = [lambda: outproj_tile(4 * last),
                                lambda: outproj_tile(4 * last + 1),
                                lambda: outproj_tile(4 * last + 2),
                                lambda: outproj_tile(4 * last + 3)]
                attn_column(p, ch, it, tailjobs)
            for f in it:
                f()
    nc.compile()
    return nc


class _Runner:
    def __init__(self, nc):
        import jax
        from jax.sharding import Mesh, PartitionSpec, NamedSharding
        from jax.experimental.shard_map import shard_map
        import concourse.mybir as mybir
        from concourse.bass2jax import (_bass_exec_p, partition_id_tensor,
                                        install_neuronx_cc_hook)
        install_neuronx_cc_hook()
        self.jax = jax
        part = nc.partition_id_tensor.name if nc.partition_id_tensor else None
        in_names, out_names, out_avals = [], [], []
        for alloc in nc.m.functions[0].allocations:
            if not isinstance(alloc, mybir.MemoryLocationSet):
                continue
            name = alloc.memorylocations[0].name
            if alloc.kind == "ExternalInput":
                if name != part:
                    in_names.append(name)
            elif alloc.kind == "ExternalOutput":
                out_names.append(name)
                out_avals.append(jax.core.ShapedArray(
                    tuple(alloc.tensor_shape), mybir.dt.np(alloc.dtype)))
        self.in_names, self.out_names, self.out_avals = in_names, out_names, out_avals
        all_in = list(in_names) + list(out_names) + ([part] if part else [])

        def _body(*args):
            ops = list(args)
            if part:
                ops.append(partition_id_tensor())
            return tuple(_bass_exec_p.bind(
                *ops, out_avals=tuple(out_avals), in_names=tuple(all_in),
                out_names=tuple(out_names), lowering_input_output_aliases=(),
                sim_require_finite=True, sim_require_nnan=True, nc=nc))

        devices = jax.devices()[:NCORES]
        mesh = Mesh(np.asarray(devices), ("core",))
        nin = len(in_names) + len(out_names)
        self.fn = jax.jit(
            shard_map(_body, mesh=mesh,
                      in_specs=(PartitionSpec("core"),) * nin,
                      out_specs=(PartitionSpec("core"),) * len(out_names),
                      check_rep=False),
            keep_unused=True)
        self.sharding = NamedSharding(mesh, PartitionSpec("core"))

    def put_inputs(self, in_maps):
        args = []
        for name in self.in_names:
            cat = np.concatenate([np.asarray(m[name]) for m in in_maps], axis=0)
            args.append(self.jax.device_put(cat, self.sharding))
        for av in self.out_avals:
            z = np.zeros((NCORES * av.shape[0], *av.shape[1:]), av.dtype)
            args.append(self.jax.device_put(z, self.sharding))
        return args

    def run_np(self, args):
        outs = self.fn(*args)
        return [
            {n: np.asarray(outs[i]).reshape(NCORES, *self.out_avals[i].shape)[c]
             for i, n in enumerate(self.out_names)}
            for c in range(NCORES)
        ]


def _get_runner():
    if "r" not in _CACHE:
        nc = _build()
        _CACHE["nc"] = nc
        _CACHE["r"] = _Runner(nc)
    return _CACHE["r"]


def _rne11(a):
    """Round fp32 to 11 mantissa bits, round-to-nearest-even (= hw fp32r)."""
    ai = np.ascontiguousarray(a, dtype=np.float32).view(np.uint32).astype(np.uint64)
    lsb = (ai >> 12) & 1
    out = (((ai + 2047 + lsb) >> 12) << 12).astype(np.uint32)
    return out.view(np.float32)


def _q8(a):
    import ml_dtypes
    a = np.clip(np.asarray(a, np.float32), -240.0, 240.0)
    return a.astype(ml_dtypes.float8_e4m3)


def make_in_maps(x, Wqkv, bqkv, Wo, bo=None, mask=None):
    # x repack: [ch, part, g, i, tok], feature = 256g + 128i + part
    xhs, xls = [], []
    for b in range(B):
        xr = np.ascontiguousarray(
            x[b].reshape(NCH, 512, NG, 2, 128).transpose(0, 4, 2, 3, 1))
        h8 = _q8(xr)
        l8 = _q8(xr - h8.astype(np.float32))
        xhs.append(h8.reshape(NCH, 128, NG * 1024))
        xls.append(l8.reshape(NCH, 128, NG * 1024))

    in_maps = []
    for c in range(NCORES):
        b, g = c // 2, c % 2
        sl = slice(g * FPC, (g + 1) * FPC)

        def pack_qk(w):
            # [d, col] -> [part, p, g, i, f]: d = 256g+128i+part, col = 128p+f
            wr = (w * WSC).reshape(NG, 2, 128, NPAIR, 128).transpose(
                2, 3, 0, 1, 4)
            hi = _q8(wr)
            lo = _q8(wr - hi.astype(np.float32))
            return (hi.reshape(128, 4096), lo.reshape(128, 4096))

        def pack_v(w):
            # [d, col] -> [part, g, i, f]: d = 256g+128i+part, col = f(512)
            wr = (w * WSC).reshape(NG, 2, 128, FPC).transpose(2, 0, 1, 3)
            hi = _q8(wr)
            lo = _q8(wr - hi.astype(np.float32))
            return (hi.reshape(128, 4096), lo.reshape(128, 4096))

        qh, ql = pack_qk(Wqkv[:, 0 * D:1 * D][:, sl])
        kh, kl = pack_qk(Wqkv[:, 1 * D:2 * D][:, sl])
        vh, vl = pack_v(Wqkv[:, 2 * D:3 * D][:, sl])

        in_maps.append({
            "xh": xhs[b], "xl": xls[b],
            "Wqh": qh, "Wql": ql, "Wkh": kh, "Wkl": kl, "Wvh": vh, "Wvl": vl,
            "BQ": np.ascontiguousarray(
                bqkv[0 * D:1 * D][sl].reshape(NPAIR, 128).T),
            "BK": np.ascontiguousarray(
                bqkv[1 * D:2 * D][sl].reshape(NPAIR, 128).T),
            "BV": np.ascontiguousarray(bqkv[2 * D:3 * D][sl]),
            "Wo": _rne11(Wo[sl, :]),
        })
    return in_maps


def kernel(x, Wqkv, bqkv, Wo, bo, mask=None, **_unused):
    x = np.asarray(x, dtype=np.float32)
    Wqkv = np.asarray(Wqkv, dtype=np.float32)
    bqkv = np.asarray(bqkv, dtype=np.float32)
    Wo = np.asarray(Wo, dtype=np.float32)
    bo = np.asarray(bo, dtype=np.float32)
    in_maps = make_in_maps(x, Wqkv, bqkv, Wo)
    last_err = None
    for _attempt in range(3):
        try:
            r = _get_runner()
            args = r.put_inputs(in_maps)
            res = r.run_np(args)
            break
        except Exception as e:  # transient device wedge: retry fresh
            last_err = e
            _CACHE.clear()
            import time
            time.sleep(5)
    else:
        raise last_err
    out = np.empty((B, T, D), dtype=np.float32)
    for b in range(B):
        out[b] = res[2 * b]["y"] + res[2 * b + 1]["y"] + bo
    return out


# revision 3
# speedup vs baseline: 1.1987x; 1.0827x over previous
"""Causal self-attention Trainium2 kernel, v2.

Sharding: 8 cores = 4 batches x 2 head-groups (8 heads each).

Per-core dataflow:
  - QKV projections as fp8e4 DoubleRow matmuls (256-feature contraction
    per instruction) with a hi/lo split of both x and W (host-prepared):
    q = xh@Wh + xl@Wh + xh@Wl  -- 3 DoubleRow passes = 6N cycles vs
    fp32r's 8N, with ~0.1% error.
  - q,k stored bf16 (rate-1 matmuls at any N, so causal diagonal blocks
    need no 256-col widening); scores per k-tile into PSUM.
  - exp on ACT writes P^T directly as bf16; causal triangle zeroed on
    GPSIMD (affine_select); PV matmuls in bf16 with a ones-column in the
    V tile accumulating softmax denominators.
  - out = PV / rowsum via DVE reciprocal + GPSIMD partition broadcast.
  - y = outT.T @ Wo in fp32r (partial; host sums the 2 head-groups).

Scheduling: projection chains for chunk ch+1 and output-projection tiles
for chunk ch-1 are interleaved between attention k-tiles of chunk ch so
the PE never idles while ACT paces the softmax.
"""
import numpy as np

B, T, D, H = 4, 2048, 1024, 16
HD = D // H            # 64
NCORES = 8
HPC = 8                # heads per core
FPC = HPC * HD         # 512 feature cols per core
NPAIR = HPC // 2       # 4 head pairs
NG = 4                 # fp8 DoubleRow contraction groups (256 feats each)
KT = T // 128          # 16 k-tiles
NCH = T // 512         # 4 q-chunks
WSC = 50.0             # host weight scaling before fp8 quantization
VSTR = 65              # per-k-tile stride in vaug free dim
HSTR = KT * VSTR       # per-head stride in vaug free dim

_CACHE = {}


def _build():
    import concourse.mybir as mybir
    import concourse.tile as tile
    from concourse import bacc
    from contextlib import ExitStack

    f32 = mybir.dt.float32
    f32r = mybir.dt.float32r
    bf16 = mybir.dt.bfloat16
    f8 = mybir.dt.float8e4
    DR = mybir.MatmulPerfMode.DoubleRow
    Exp = mybir.ActivationFunctionType.Exp
    Alu = mybir.AluOpType

    nc = bacc.Bacc("TRN2", target_bir_lowering=False, debug=False,
                   num_devices=NCORES)
    # x hi/lo fp8, repacked host-side as [ch, 128, g, i, tok]:
    # feature = 256*g + 128*i + partition
    xh = nc.dram_tensor("xh", [NCH, 128, NG * 1024], f8, kind="ExternalInput")
    xl = nc.dram_tensor("xl", [NCH, 128, NG * 1024], f8, kind="ExternalInput")
    # fp8 weights, 6 tensors: q/k: [p][g][i][f128]; v: [g][i][f512]
    Wsec = [nc.dram_tensor(nm, [128, 4096], f8, kind="ExternalInput")
            for nm in ("Wqh", "Wql", "Wkh", "Wkl", "Wvh", "Wvl")]
    BQ = nc.dram_tensor("BQ", [128, NPAIR], f32, kind="ExternalInput")
    BK = nc.dram_tensor("BK", [128, NPAIR], f32, kind="ExternalInput")
    BV = nc.dram_tensor("BV", [FPC], f32, kind="ExternalInput")
    Wo = nc.dram_tensor("Wo", [FPC, D], f32r, kind="ExternalInput")
    y = nc.dram_tensor("y", [T, D], f32, kind="ExternalOutput")

    with tile.TileContext(nc) as tc, ExitStack() as es:
        pers = es.enter_context(tc.tile_pool(name="pers", bufs=1))
        xsp = es.enter_context(tc.tile_pool(name="xsp", bufs=2))
        ptp = es.enter_context(tc.tile_pool(name="ptp", bufs=6))
        nrm = es.enter_context(tc.tile_pool(name="nrm", bufs=2))
        obp = es.enter_context(tc.tile_pool(name="obp", bufs=3))
        psA = es.enter_context(tc.tile_pool(name="psA", bufs=2, space="PSUM"))
        psS = es.enter_context(tc.tile_pool(name="psS", bufs=2, space="PSUM"))
        psO = es.enter_context(tc.tile_pool(name="psO", bufs=1, space="PSUM"))

        ws_sb = [pers.tile([128, 4096], f8, tag=f"ws{i}", name=f"ws{i}")
                 for i in range(6)]
        bq_sb = pers.tile([128, NPAIR], f32, tag="bq")
        bk_sb = pers.tile([128, NPAIR], f32, tag="bk")
        bv_row = pers.tile([1, FPC], f32, tag="bvr")
        bv_bc = pers.tile([128, FPC], f32, tag="bvb")
        wo_sb = pers.tile([128, NPAIR * D], f32r, tag="wo")
        vaug = pers.tile([128, HPC * HSTR], bf16, tag="vaug")
        qT = [pers.tile([128, T], bf16, tag=f"qT{p}", name=f"qT{p}")
              for p in range(NPAIR)]
        kT = [pers.tile([128, T], bf16, tag=f"kT{p}", name=f"kT{p}")
              for p in range(NPAIR)]
        outT = [pers.tile([128, T], f32r, tag=f"oT{p}", name=f"oT{p}")
                for p in range(NPAIR)]

        vaug4 = vaug[:].rearrange("p (h k x) -> p h k x", h=HPC, k=KT)

        xtiles = {}

        def emit_xdma(ch):
            xh_sb = xsp.tile([128, NG * 1024], f8, tag="xh", name="xh_sb")
            xl_sb = xsp.tile([128, NG * 1024], f8, tag="xl", name="xl_sb")
            if ch == 0:
                half = NG * 512
                nc.sync.dma_start(xh_sb[:, 0:half], xh[ch][:, 0:half])
                nc.sync.dma_start(xl_sb[:, 0:half], xl[ch][:, 0:half])
                nc.sync.dma_start(xh_sb[:, half:], xh[ch][:, half:])
                nc.sync.dma_start(xl_sb[:, half:], xl[ch][:, half:])
            else:
                nc.sync.dma_start(xh_sb[:], xh[ch])
                nc.sync.dma_start(xl_sb[:], xl[ch])
            xtiles[ch] = [xx[:, g * 1024:(g + 1) * 1024].rearrange(
                "p (i t) -> p i t", i=2)
                for xx in (xh_sb, xl_sb) for g in range(NG)]

        def w_ap(sec, p, g):
            base = p * 1024 + g * 256
            return ws_sb[sec][:, base:base + 256].rearrange(
                "p (i f) -> p i f", i=2)

        def wv_ap(sec, g):
            base = g * 1024
            return ws_sb[sec][:, base:base + 1024].rearrange(
                "p (i f) -> p i f", i=2)

        # (x-part, w-section) term order: xl last so its DMA can trail
        # at startup
        def proj_qk(ch, p, sec, bsb, dst):
            xg = xtiles[ch]
            ps = psA.tile([128, 512], f32, tag="psA", name="psqk")
            n = 0
            for (xt, ws) in ((0, sec), (0, sec + 1), (1, sec)):
                for g in range(NG):
                    nc.tensor.matmul(
                        ps[:], w_ap(ws, p, g), xg[xt * NG + g],
                        start=(n == 0), stop=(n == 3 * NG - 1),
                        perf_mode=DR)
                    n += 1
            nc.vector.tensor_scalar(
                dst[p][:, 512 * ch:512 * (ch + 1)], ps[:],
                1.0 / WSC, bsb[:, p:p + 1], Alu.mult, Alu.add)

        def proj_v(ch, tj):
            xg = xtiles[ch]
            lo = (tj - 4 * ch) * 128
            ps = psA.tile([128, FPC], f32, tag="psA", name="psv")
            n = 0
            for (xt, ws) in ((0, 4), (0, 5), (1, 4)):
                for g in range(NG):
                    nc.tensor.matmul(
                        ps[:], xg[xt * NG + g][:, :, lo:lo + 128],
                        wv_ap(ws, g),
                        start=(n == 0), stop=(n == 3 * NG - 1),
                        perf_mode=DR)
                    n += 1
            nc.vector.scalar_tensor_tensor(
                vaug4[:, :, tj, 0:64],
                ps[:].rearrange("p (h x) -> p h x", h=HPC),
                1.0 / WSC,
                bv_bc[:].rearrange("p (h x) -> p h x", h=HPC),
                Alu.mult, Alu.add)

        def proj_chunk_fill(ch):
            out = []
            for p in range(NPAIR):
                out.append(lambda p=p: proj_qk(ch, p, 0, bq_sb, qT))
            for p in range(NPAIR):
                out.append(lambda p=p: proj_qk(ch, p, 2, bk_sb, kT))
            for tj in range(4 * ch, 4 * (ch + 1)):
                out.append(lambda tj=tj: proj_v(ch, tj))
            return out

        Copy = mybir.ActivationFunctionType.Copy

        def outproj_tile(tj):
            ob = obp.tile([128, D], f32, tag="ob", name="ob")
            for n in range(2):
                ps = psA.tile([128, 512], f32, tag="psA", name="pspr")
                for p in range(NPAIR):
                    nc.tensor.matmul(
                        ps[:],
                        outT[p][:, 128 * tj:128 * (tj + 1)],
                        wo_sb[:, p * D + 512 * n:p * D + 512 * (n + 1)],
                        start=(p == 0), stop=(p == NPAIR - 1))
                # evacuate on ACT: keeps the tail normalize chain off the
                # DVE queue
                nc.scalar.activation(ob[:, 512 * n:512 * (n + 1)], ps[:],
                                     Copy)
            nc.sync.dma_start(y[128 * tj:128 * (tj + 1), :], ob[:])

        def emit_norm(p, ch, po, c0, c1):
            w = c1 - c0
            for h in range(2):
                og = nrm.tile([65, 512], f32, tag="og", name="og")
                nc.vector.tensor_copy(og[:, 0:w], po[h][:, c0:c1])
                rec = nrm.tile([1, 512], f32, tag="rec", name="rec")
                nc.vector.reciprocal(rec[:, 0:w], og[64:65, 0:w])
                bc = nrm.tile([64, 512], f32, tag="bc", name="bc")
                nc.gpsimd.partition_broadcast(bc[:, 0:w], rec[:, 0:w])
                nc.vector.tensor_mul(
                    outT[p][64 * h:64 * (h + 1),
                            512 * ch + c0:512 * ch + c1],
                    og[0:64, 0:w], bc[:, 0:w])

        def attn_column(p, ch, fill, tailjobs=None):
            """fill: iterator of emitters to interleave between k-tiles."""
            po = [psO.tile([65, 512], f32, tag=f"po{h}", name=f"po{h}")
                  for h in range(2)]
            nk = 4 * ch + 4

            def emit_pv(kt, pt):
                qo = max(kt - 4 * ch, 0) * 128
                for h in range(2):
                    nc.tensor.matmul(
                        po[h][:, qo:],
                        vaug4[:, 2 * p + h, kt, :],
                        pt[:, h, qo:],
                        start=(kt == 0), stop=(kt == nk - 1),
                        skip_group_check=True)

            pend = []
            for kt in range(nk):
                dg = kt - 4 * ch
                qo = max(dg, 0) * 128
                pt = ptp.tile([128, 2, 512], bf16, tag="pt", name="pt")
                ss = psS.tile([128, 2, 512], f32, tag="psS", name="ss")
                for h in range(2):
                    r0, r1 = 64 * h, 64 * (h + 1)
                    nc.tensor.matmul(
                        ss[:, h, qo:],
                        kT[p][r0:r1, 128 * kt:128 * (kt + 1)],
                        qT[p][r0:r1, 512 * ch + qo:512 * (ch + 1)],
                        start=True, stop=True)
                nc.scalar.activation(pt[:, :, qo:], ss[:, :, qo:],
                                     Exp, scale=0.125)
                if dg >= 0:
                    # zero the upper (q < k) triangle of the diagonal block
                    blk = pt[:, :, qo:qo + 128]
                    nc.gpsimd.affine_select(
                        out=blk, in_=blk,
                        compare_op=mybir.AluOpType.is_ge,
                        fill=0.0, base=0,
                        pattern=[[0, 2], [1, 128]],
                        channel_multiplier=-1)
                # fill, then 2-tile-deep software-pipelined PV so the
                # PE never waits inline on an exp
                f = next(fill, None)
                if f:
                    f()
                pend.append((kt, pt))
                if len(pend) > 2:
                    emit_pv(*pend.pop(0))
            if tailjobs is None:
                for pv in pend:
                    f = next(fill, None)
                    if f:
                        f()
                    emit_pv(*pv)
                emit_norm(p, ch, po, 0, 512)
            else:
                # last column: po[:, 0:128*q] is complete as soon as the
                # q-th diagonal PV lands -- normalize and project each
                # quarter as it completes, with reserved chunk-2 output
                # tiles filling each normalize chain's latency
                emit_norm(p, ch, po, 0, 128)
                tailjobs[0]()
                tailjobs[4]()
                emit_norm(p, ch, po, 128, 256)
                tailjobs[1]()
                tailjobs[5]()
                emit_pv(*pend.pop(0))
                emit_norm(p, ch, po, 256, 384)
                tailjobs[2]()
                tailjobs[6]()
                emit_pv(*pend.pop(0))
                emit_norm(p, ch, po, 384, 512)
                tailjobs[3]()
                tailjobs[7]()

        # ---------------- schedule ----------------
        # startup DMA order: q weights and x first (in first-use order,
        # split so the first projection chain can start ~4us in)
        H2 = 2048
        nc.sync.dma_start(ws_sb[0][:, 0:H2], Wsec[0][:, 0:H2])
        emit_xdma(0)
        nc.sync.dma_start(ws_sb[1][:, 0:H2], Wsec[1][:, 0:H2])
        nc.sync.dma_start(ws_sb[0][:, H2:], Wsec[0][:, H2:])
        nc.sync.dma_start(ws_sb[1][:, H2:], Wsec[1][:, H2:])
        nc.sync.dma_start(bq_sb[:], BQ[:])
        for w, dram in list(zip(ws_sb, Wsec))[2:]:
            nc.sync.dma_start(w[:], dram[:])
        nc.sync.dma_start(bk_sb[:], BK[:])
        nc.sync.dma_start(bv_row[:], BV[:])
        nc.gpsimd.partition_broadcast(bv_bc[:], bv_row[:])
        nc.vector.memset(vaug4[:, :, :, 64:65], 1.0)
        nc.sync.dma_start(wo_sb[:].rearrange("p (t c) -> p t c", t=NPAIR),
                          Wo[:].rearrange("(t p) c -> p t c", p=128))

        for f in proj_chunk_fill(0):
            f()

        # ---- deadline-packed fill assignment over the k-tile slots ----
        slots = []           # (ch, p, kt) in emission order
        slot_of = {}
        for ch in range(NCH):
            for p in range(NPAIR):
                for kt in range(4 * ch + 4):
                    slot_of[(ch, p, kt)] = len(slots)
                    slots.append((ch, p, kt))
        nslots = len(slots)
        chunk_start = {ch: slot_of[(ch, 0, 0)] for ch in range(NCH)}
        chunk_end = {ch: slot_of[(ch, NPAIR - 1, 4 * ch + 3)]
                     for ch in range(NCH)}

        items = []  # (deadline, avail, fn)
        for ch in range(1, NCH):
            av = chunk_start[ch - 1]
            for p in range(NPAIR):
                dl = slot_of[(ch, p, 0)] - 2
                items.append((dl, av, lambda ch=ch, p=p:
                              proj_qk(ch, p, 0, bq_sb, qT)))
                items.append((dl, av, lambda ch=ch, p=p:
                              proj_qk(ch, p, 2, bk_sb, kT)))
            for tj in range(4 * ch, 4 * (ch + 1)):
                dl = slot_of[(ch, 0, tj)] - 2
                items.append((dl, av, lambda ch=ch, tj=tj: proj_v(ch, tj)))
        for tj in range(4 * (NCH - 2)):
            items.append((nslots - 1, chunk_end[tj // 4] + 1,
                          lambda tj=tj: outproj_tile(tj)))

        assigned = {}
        for dl, av, fn in sorted(items, key=lambda it: -it[0]):
            s = min(dl, nslots - 1)
            while s >= av and s in assigned:
                s -= 1
            if s < av:
                s = av
                while s in assigned:
                    s += 1
            assigned[s] = fn

        fills = {}
        for s, fn in assigned.items():
            fills.setdefault(s, []).append(fn)

        def _slotfill(seq):
            for fl in seq:
                yield fl[0] if fl else None
            while True:
                yield None

        last = NCH - 1
        for ch in range(NCH):
            if ch + 1 < NCH:
                emit_xdma(ch + 1)
            for p in range(NPAIR):
                nk = 4 * ch + 4
                seq = []
                for kt in range(nk):
                    fl = fills.get(slot_of[(ch, p, kt)], [])
                    seq.append(fl)
                it = _slotfill(seq)
                tailjobs = None
                if ch == last and p == NPAIR - 1:
                    tailjobs = [lambda: outproj_tile(4 * (last - 1)),
                                lambda: outproj_tile(4 * (last - 1) + 1),
                                lambda: outproj_tile(4 * (last - 1) + 2),
                                lambda: outproj_tile(4 * (last - 1) + 3),
                                lambda: outproj_tile(4 * last),
                                lambda: outproj_tile(4 * last + 1),
                                lambda: outproj_tile(4 * last + 2),
                                lambda: outproj_tile(4 * last + 3)]
                attn_column(p, ch, it, tailjobs)
    nc.compile()
    return nc


class _Runner:
    def __init__(self, nc):
        import jax
        from jax.sharding import Mesh, PartitionSpec, NamedSharding
        from jax.experimental.shard_map import shard_map
        import concourse.mybir as mybir
        from concourse.bass2jax import (_bass_exec_p, partition_id_tensor,
                                        install_neuronx_cc_hook)
        install_neuronx_cc_hook()
        self.jax = jax
        part = nc.partition_id_tensor.name if nc.partition_id_tensor else None
        in_names, out_names, out_avals = [], [], []
        for alloc in nc.m.functions[0].allocations:
            if not isinstance(alloc, mybir.MemoryLocationSet):
                continue
            name = alloc.memorylocations[0].name
            if alloc.kind == "ExternalInput":
                if name != part:
                    in_names.append(name)
            elif alloc.kind == "ExternalOutput":
                out_names.append(name)
                out_avals.append(jax.core.ShapedArray(
                    tuple(alloc.tensor_shape), mybir.dt.np(alloc.dtype)))
        self.in_names, self.out_names, self.out_avals = in_names, out_names, out_avals
        all_in = list(in_names) + list(out_names) + ([part] if part else [])

        def _body(*args):
            ops = list(args)
            if part:
                ops.append(partition_id_tensor())
            return tuple(_bass_exec_p.bind(
                *ops, out_avals=tuple(out_avals), in_names=tuple(all_in),
                out_names=tuple(out_names), lowering_input_output_aliases=(),
                sim_require_finite=True, sim_require_nnan=True, nc=nc))

        devices = jax.devices()[:NCORES]
        mesh = Mesh(np.asarray(devices), ("core",))
        nin = len(in_names) + len(out_names)
        self.fn = jax.jit(
            shard_map(_body, mesh=mesh,
                      in_specs=(PartitionSpec("core"),) * nin,
                      out_specs=(PartitionSpec("core"),) * len(out_names),
                      check_rep=False),
            keep_unused=True)
        self.sharding = NamedSharding(mesh, PartitionSpec("core"))

    def put_inputs(self, in_maps):
        args = []
        for name in self.in_names:
            cat = np.concatenate([np.asarray(m[name]) for m in in_maps], axis=0)
            args.append(self.jax.device_put(cat, self.sharding))
        for av in self.out_avals:
            z = np.zeros((NCORES * av.shape[0], *av.shape[1:]), av.dtype)
            args.append(self.jax.device_put(z, self.sharding))
        return args

    def run_np(self, args):
        outs = self.fn(*args)
        return [
            {n: np.asarray(outs[i]).reshape(NCORES, *self.out_avals[i].shape)[c]
             for i, n in enumerate(self.out_names)}
            for c in range(NCORES)
        ]


def _get_runner():
    if "r" not in _CACHE:
        nc = _build()
        _CACHE["nc"] = nc
        _CACHE["r"] = _Runner(nc)
    return _CACHE["r"]


def _rne11(a):
    """Round fp32 to 11 mantissa bits, round-to-nearest-even (= hw fp32r)."""
    ai = np.ascontiguousarray(a, dtype=np.float32).view(np.uint32).astype(np.uint64)
    lsb = (ai >> 12) & 1
    out = (((ai + 2047 + lsb) >> 12) << 12).astype(np.uint32)
    return out.view(np.float32)


def _q8(a):
    import ml_dtypes
    a = np.clip(np.asarray(a, np.float32), -240.0, 240.0)
    return a.astype(ml_dtypes.float8_e4m3)


def make_in_maps(x, Wqkv, bqkv, Wo, bo=None, mask=None):
    # x repack: [ch, part, g, i, tok], feature = 256g + 128i + part
    xhs, xls = [], []
    for b in range(B):
        xr = np.ascontiguousarray(
            x[b].reshape(NCH, 512, NG, 2, 128).transpose(0, 4, 2, 3, 1))
        h8 = _q8(xr)
        l8 = _q8(xr - h8.astype(np.float32))
        xhs.append(h8.reshape(NCH, 128, NG * 1024))
        xls.append(l8.reshape(NCH, 128, NG * 1024))

    in_maps = []
    for c in range(NCORES):
        b, g = c // 2, c % 2
        sl = slice(g * FPC, (g + 1) * FPC)

        def pack_qk(w):
            # [d, col] -> [part, p, g, i, f]: d = 256g+128i+part, col = 128p+f
            wr = (w * WSC).reshape(NG, 2, 128, NPAIR, 128).transpose(
                2, 3, 0, 1, 4)
            hi = _q8(wr)
            lo = _q8(wr - hi.astype(np.float32))
            return (hi.reshape(128, 4096), lo.reshape(128, 4096))

        def pack_v(w):
            # [d, col] -> [part, g, i, f]: d = 256g+128i+part, col = f(512)
            wr = (w * WSC).reshape(NG, 2, 128, FPC).transpose(2, 0, 1, 3)
            hi = _q8(wr)
            lo = _q8(wr - hi.astype(np.float32))
            return (hi.reshape(128, 4096), lo.reshape(128, 4096))

        qh, ql = pack_qk(Wqkv[:, 0 * D:1 * D][:, sl])
        kh, kl = pack_qk(Wqkv[:, 1 * D:2 * D][:, sl])
        vh, vl = pack_v(Wqkv[:, 2 * D:3 * D][:, sl])

        in_maps.append({
            "xh": xhs[b], "xl": xls[b],
            "Wqh": qh, "Wql": ql, "Wkh": kh, "Wkl": kl, "Wvh": vh, "Wvl": vl,
            "BQ": np.ascontiguousarray(
                bqkv[0 * D:1 * D][sl].reshape(NPAIR, 128).T),
            "BK": np.ascontiguousarray(
                bqkv[1 * D:2 * D][sl].reshape(NPAIR, 128).T),
            "BV": np.ascontiguousarray(bqkv[2 * D:3 * D][sl]),
            "Wo": _rne11(Wo[sl, :]),
        })
    return in_maps


def kernel(x, Wqkv, bqkv, Wo, bo, mask=None, **_unused):
    x = np.asarray(x, dtype=np.float32)
    Wqkv = np.asarray(Wqkv, dtype=np.float32)
    bqkv = np.asarray(bqkv, dtype=np.float32)
    Wo = np.asarray(Wo, dtype=np.float32)
    bo = np.asarray(bo, dtype=np.float32)
    in_maps = make_in_maps(x, Wqkv, bqkv, Wo)
    last_err = None
    for _attempt in range(3):
        try:
            r = _get_runner()
            args = r.put_inputs(in_maps)
            res = r.run_np(args)
            break
        except Exception as e:  # transient device wedge: retry fresh
            last_err = e
            _CACHE.clear()
            import time
            time.sleep(5)
    else:
        raise last_err
    out = np.empty((B, T, D), dtype=np.float32)
    for b in range(B):
        out[b] = res[2 * b]["y"] + res[2 * b + 1]["y"] + bo
    return out


# revision 5
# speedup vs baseline: 1.2058x; 1.0059x over previous
"""Causal self-attention Trainium2 kernel, v2.

Sharding: 8 cores = 4 batches x 2 head-groups (8 heads each).

Per-core dataflow:
  - QKV projections as fp8e4 DoubleRow matmuls (256-feature contraction
    per instruction) with a hi/lo split of both x and W (host-prepared):
    q = xh@Wh + xl@Wh + xh@Wl  -- 3 DoubleRow passes = 6N cycles vs
    fp32r's 8N, with ~0.1% error.
  - q,k stored bf16 (rate-1 matmuls at any N, so causal diagonal blocks
    need no 256-col widening); scores per k-tile into PSUM.
  - exp on ACT writes P^T directly as bf16; causal triangle zeroed on
    GPSIMD (affine_select); PV matmuls in bf16 with a ones-column in the
    V tile accumulating softmax denominators.
  - out = PV / rowsum via DVE reciprocal + GPSIMD partition broadcast.
  - y = outT.T @ Wo in fp32r (partial; host sums the 2 head-groups).

Scheduling: projection chains for chunk ch+1 and output-projection tiles
for chunk ch-1 are interleaved between attention k-tiles of chunk ch so
the PE never idles while ACT paces the softmax.
"""
import numpy as np

B, T, D, H = 4, 2048, 1024, 16
HD = D // H            # 64
NCORES = 8
HPC = 8                # heads per core
FPC = HPC * HD         # 512 feature cols per core
NPAIR = HPC // 2       # 4 head pairs
NG = 4                 # fp8 DoubleRow contraction groups (256 feats each)
KT = T // 128          # 16 k-tiles
NCH = T // 512         # 4 q-chunks
WSC = 50.0             # host weight scaling before fp8 quantization
QKS = 16.0             # q/k fp8 storage scale
VSTR = 65              # per-k-tile stride in vaug free dim
HSTR = KT * VSTR       # per-head stride in vaug free dim

_CACHE = {}


def _build():
    import concourse.mybir as mybir
    import concourse.tile as tile
    from concourse import bacc
    from contextlib import ExitStack

    f32 = mybir.dt.float32
    f32r = mybir.dt.float32r
    bf16 = mybir.dt.bfloat16
    f8 = mybir.dt.float8e4
    DR = mybir.MatmulPerfMode.DoubleRow
    Exp = mybir.ActivationFunctionType.Exp
    Alu = mybir.AluOpType

    nc = bacc.Bacc("TRN2", target_bir_lowering=False, debug=False,
                   num_devices=NCORES)
    # x hi/lo fp8, repacked host-side as [ch, 128, g, i, tok]:
    # feature = 256*g + 128*i + partition
    xh = nc.dram_tensor("xh", [NCH, 128, NG * 1024], f8, kind="ExternalInput")
    xl = nc.dram_tensor("xl", [NCH, 128, NG * 1024], f8, kind="ExternalInput")
    # fp8 weights, 6 tensors: q/k: [p][g][i][f128]; v: [g][i][f512]
    Wsec = [nc.dram_tensor(nm, [128, 4096], f8, kind="ExternalInput")
            for nm in ("Wqh", "Wql", "Wkh", "Wkl", "Wvh", "Wvl")]
    BQ = nc.dram_tensor("BQ", [128, NPAIR], f32, kind="ExternalInput")
    BK = nc.dram_tensor("BK", [128, NPAIR], f32, kind="ExternalInput")
    BV = nc.dram_tensor("BV", [FPC], f32, kind="ExternalInput")
    Wo = nc.dram_tensor("Wo", [FPC, D], f32r, kind="ExternalInput")
    y = nc.dram_tensor("y", [T, D], f32, kind="ExternalOutput")

    with tile.TileContext(nc) as tc, ExitStack() as es:
        pers = es.enter_context(tc.tile_pool(name="pers", bufs=1))
        xsp = es.enter_context(tc.tile_pool(name="xsp", bufs=2))
        ptp = es.enter_context(tc.tile_pool(name="ptp", bufs=6))
        nrm = es.enter_context(tc.tile_pool(name="nrm", bufs=2))
        obp = es.enter_context(tc.tile_pool(name="obp", bufs=3))
        stgp = es.enter_context(tc.tile_pool(name="stgp", bufs=4))
        psA = es.enter_context(tc.tile_pool(name="psA", bufs=2, space="PSUM"))
        psS = es.enter_context(tc.tile_pool(name="psS", bufs=2, space="PSUM"))
        psO = es.enter_context(tc.tile_pool(name="psO", bufs=1, space="PSUM"))

        ws_sb = [pers.tile([128, 4096], f8, tag=f"ws{i}", name=f"ws{i}")
                 for i in range(6)]
        bq_sb = pers.tile([128, NPAIR], f32, tag="bq")
        bk_sb = pers.tile([128, NPAIR], f32, tag="bk")
        bv_row = pers.tile([1, FPC], f32, tag="bvr")
        bv_bc = pers.tile([128, FPC], f32, tag="bvb")
        wo_sb = pers.tile([128, NPAIR * D], f32r, tag="wo")
        vaug = pers.tile([128, HPC * HSTR], bf16, tag="vaug")
        # q/k in fp8 for DoubleRow scores: tile u holds pairs (2u, 2u+1);
        # partition = 64*(pr%2) + 32*head + hd%32, free = (hd//32, token)
        qT8 = [pers.tile([128, 2, T], f8, tag=f"qT8{u}", name=f"qT8{u}")
               for u in range(2)]
        kT8 = [pers.tile([128, 2, T], f8, tag=f"kT8{u}", name=f"kT8{u}")
               for u in range(2)]
        outT = [pers.tile([128, T], f32r, tag=f"oT{p}", name=f"oT{p}")
                for p in range(NPAIR)]

        vaug4 = vaug[:].rearrange("p (h k x) -> p h k x", h=HPC, k=KT)

        xtiles = {}

        def emit_xdma(ch):
            xh_sb = xsp.tile([128, NG * 1024], f8, tag="xh", name="xh_sb")
            xl_sb = xsp.tile([128, NG * 1024], f8, tag="xl", name="xl_sb")
            if ch == 0:
                half = NG * 512
                nc.sync.dma_start(xh_sb[:, 0:half], xh[ch][:, 0:half])
                nc.sync.dma_start(xl_sb[:, 0:half], xl[ch][:, 0:half])
                nc.sync.dma_start(xh_sb[:, half:], xh[ch][:, half:])
                nc.sync.dma_start(xl_sb[:, half:], xl[ch][:, half:])
            else:
                nc.sync.dma_start(xh_sb[:], xh[ch])
                nc.sync.dma_start(xl_sb[:], xl[ch])
            xtiles[ch] = [xx[:, g * 1024:(g + 1) * 1024].rearrange(
                "p (i t) -> p i t", i=2)
                for xx in (xh_sb, xl_sb) for g in range(NG)]

        def w_ap(sec, p, g):
            base = p * 1024 + g * 256
            return ws_sb[sec][:, base:base + 256].rearrange(
                "p (i f) -> p i f", i=2)

        def wv_ap(sec, g):
            base = g * 1024
            return ws_sb[sec][:, base:base + 1024].rearrange(
                "p (i f) -> p i f", i=2)

        # (x-part, w-section) term order: xl last so its DMA can trail
        # at startup. psum partitions are ordered (hd-half j, head, hd%32)
        # by the host weight packing; the evac writes scaled fp8 into a
        # staging tile whose two j-halves are then DMA'd into the
        # partition-sliced qT8/kT8 layout.
        def proj_qk(ch, p, sec, bsb, dst):
            xg = xtiles[ch]
            ps = psA.tile([128, 512], f32, tag="psA", name="psqk")
            n = 0
            for (xt, ws) in ((0, sec), (0, sec + 1), (1, sec)):
                for g in range(NG):
                    nc.tensor.matmul(
                        ps[:], w_ap(ws, p, g), xg[xt * NG + g],
                        start=(n == 0), stop=(n == 3 * NG - 1),
                        perf_mode=DR)
                    n += 1
            stg = stgp.tile([128, 512], f8, tag="stg", name="stg")
            nc.vector.tensor_scalar(
                stg[:], ps[:], QKS / WSC, bsb[:, p:p + 1],
                Alu.mult, Alu.add)
            u, e = p // 2, p % 2
            for j in range(2):
                nc.sync.dma_start(
                    dst[u][64 * e:64 * e + 64, j,
                           512 * ch:512 * (ch + 1)],
                    stg[64 * j:64 * j + 64, :])

        def proj_v(ch, tj):
            xg = xtiles[ch]
            lo = (tj - 4 * ch) * 128
            ps = psA.tile([128, FPC], f32, tag="psA", name="psv")
            n = 0
            for (xt, ws) in ((0, 4), (0, 5), (1, 4)):
                for g in range(NG):
                    nc.tensor.matmul(
                        ps[:], xg[xt * NG + g][:, :, lo:lo + 128],
                        wv_ap(ws, g),
                        start=(n == 0), stop=(n == 3 * NG - 1),
                        perf_mode=DR)
                    n += 1
            nc.vector.scalar_tensor_tensor(
                vaug4[:, :, tj, 0:64],
                ps[:].rearrange("p (h x) -> p h x", h=HPC),
                1.0 / WSC,
                bv_bc[:].rearrange("p (h x) -> p h x", h=HPC),
                Alu.mult, Alu.add)

        def proj_chunk_fill(ch):
            out = []
            for p in range(NPAIR):
                out.append(lambda p=p: proj_qk(ch, p, 0, bq_sb, qT8))
            for p in range(NPAIR):
                out.append(lambda p=p: proj_qk(ch, p, 2, bk_sb, kT8))
            for tj in range(4 * ch, 4 * (ch + 1)):
                out.append(lambda tj=tj: proj_v(ch, tj))
            return out

        Copy = mybir.ActivationFunctionType.Copy

        def outproj_tile(tj, act_evac=False):
            ob = obp.tile([128, D], f32, tag="ob", name="ob")
            for n in range(2):
                ps = psA.tile([128, 512], f32, tag="psA", name="pspr")
                for p in range(NPAIR):
                    nc.tensor.matmul(
                        ps[:],
                        outT[p][:, 128 * tj:128 * (tj + 1)],
                        wo_sb[:, p * D + 512 * n:p * D + 512 * (n + 1)],
                        start=(p == 0), stop=(p == NPAIR - 1))
                if act_evac:
                    # ACT evac keeps the tail normalize off the DVE queue
                    nc.scalar.activation(ob[:, 512 * n:512 * (n + 1)],
                                         ps[:], Copy)
                else:
                    nc.vector.tensor_copy(ob[:, 512 * n:512 * (n + 1)],
                                          ps[:])
            nc.sync.dma_start(y[128 * tj:128 * (tj + 1), :], ob[:])

        def emit_norm(p, ch, po, c0, c1):
            w = c1 - c0
            for h in range(2):
                og = nrm.tile([65, 512], f32, tag="og", name="og")
                nc.vector.tensor_copy(og[:, 0:w], po[h][:, c0:c1])
                rec = nrm.tile([1, 512], f32, tag="rec", name="rec")
                nc.vector.reciprocal(rec[:, 0:w], og[64:65, 0:w])
                bc = nrm.tile([64, 512], f32, tag="bc", name="bc")
                nc.gpsimd.partition_broadcast(bc[:, 0:w], rec[:, 0:w])
                nc.vector.tensor_mul(
                    outT[p][64 * h:64 * (h + 1),
                            512 * ch + c0:512 * ch + c1],
                    og[0:64, 0:w], bc[:, 0:w])

        def attn_column(p, ch, fill, tailjobs=None):
            """fill: iterator of emitters to interleave between k-tiles."""
            po = [psO.tile([65, 512], f32, tag=f"po{h}", name=f"po{h}")
                  for h in range(2)]
            nk = 4 * ch + 4

            def emit_pv(kt, pt):
                qo = max(kt - 4 * ch, 0) * 128
                for h in range(2):
                    nc.tensor.matmul(
                        po[h][:, qo:],
                        vaug4[:, 2 * p + h, kt, :],
                        pt[:, h, qo:],
                        start=(kt == 0), stop=(kt == nk - 1),
                        skip_group_check=True)

            pend = []
            for kt in range(nk):
                dg = kt - 4 * ch
                qo = max(dg, 0) * 128
                pt = ptp.tile([128, 2, 512], bf16, tag="pt", name="pt")
                ss = psS.tile([128, 2, 512], f32, tag="psS", name="ss")
                u, e = p // 2, p % 2
                for h in range(2):
                    b0 = 64 * e + 32 * h
                    nc.tensor.matmul(
                        ss[:, h, qo:],
                        kT8[u][b0:b0 + 32, :, 128 * kt:128 * (kt + 1)],
                        qT8[u][b0:b0 + 32, :,
                               512 * ch + qo:512 * (ch + 1)],
                        start=True, stop=True, perf_mode=DR,
                        tile_position=(b0, 0))
                nc.scalar.activation(pt[:, :, qo:], ss[:, :, qo:],
                                     Exp, scale=0.125 / (QKS * QKS))
                if dg >= 0:
                    # zero the upper (q < k) triangle of the diagonal block
                    blk = pt[:, :, qo:qo + 128]
                    nc.gpsimd.affine_select(
                        out=blk, in_=blk,
                        compare_op=mybir.AluOpType.is_ge,
                        fill=0.0, base=0,
                        pattern=[[0, 2], [1, 128]],
                        channel_multiplier=-1)
                # fill, then 2-tile-deep software-pipelined PV so the
                # PE never waits inline on an exp
                f = next(fill, None)
                if f:
                    f()
                pend.append((kt, pt))
                if len(pend) > 2:
                    emit_pv(*pend.pop(0))
            if tailjobs is None:
                for pv in pend:
                    f = next(fill, None)
                    if f:
                        f()
                    emit_pv(*pv)
                emit_norm(p, ch, po, 0, 512)
            else:
                # last column: po[:, 0:128*q] is complete as soon as the
                # q-th diagonal PV lands -- normalize and project each
                # quarter as it completes, with reserved chunk-2 output
                # tiles filling each normalize chain's latency
                emit_norm(p, ch, po, 0, 128)
                tailjobs[0]()
                tailjobs[4]()
                emit_norm(p, ch, po, 128, 256)
                tailjobs[1]()
                emit_pv(*pend.pop(0))
                tailjobs[5]()
                emit_pv(*pend.pop(0))
                emit_norm(p, ch, po, 256, 384)
                emit_norm(p, ch, po, 384, 512)
                tailjobs[2]()
                tailjobs[6]()
                tailjobs[3]()
                tailjobs[7]()

        # ---------------- schedule ----------------
        # startup DMA order: q weights and x first (in first-use order,
        # split so the first projection chain can start ~4us in)
        H2 = 2048
        nc.sync.dma_start(ws_sb[0][:, 0:H2], Wsec[0][:, 0:H2])
        emit_xdma(0)
        nc.sync.dma_start(ws_sb[1][:, 0:H2], Wsec[1][:, 0:H2])
        nc.sync.dma_start(bq_sb[:], BQ[:])
        nc.sync.dma_start(bv_row[:], BV[:])
        nc.sync.dma_start(ws_sb[0][:, H2:], Wsec[0][:, H2:])
        nc.sync.dma_start(ws_sb[1][:, H2:], Wsec[1][:, H2:])
        nc.sync.dma_start(ws_sb[2][:], Wsec[2][:])
        nc.sync.dma_start(ws_sb[3][:], Wsec[3][:])
        nc.sync.dma_start(bk_sb[:], BK[:])
        nc.gpsimd.partition_broadcast(bv_bc[:], bv_row[:])
        nc.vector.memset(vaug4[:, :, :, 64:65], 1.0)

        fill0 = proj_chunk_fill(0)
        # q p0-3, k p0-3, then V weights DMA just before the V chains
        for f in fill0[:8]:
            f()
        nc.sync.dma_start(ws_sb[4][:], Wsec[4][:])
        nc.sync.dma_start(ws_sb[5][:], Wsec[5][:])
        for f in fill0[8:]:
            f()

        # ---- deadline-packed fill assignment over the k-tile slots ----
        slots = []           # (ch, p, kt) in emission order
        slot_of = {}
        for ch in range(NCH):
            for p in range(NPAIR):
                for kt in range(4 * ch + 4):
                    slot_of[(ch, p, kt)] = len(slots)
                    slots.append((ch, p, kt))
        nslots = len(slots)
        chunk_start = {ch: slot_of[(ch, 0, 0)] for ch in range(NCH)}
        chunk_end = {ch: slot_of[(ch, NPAIR - 1, 4 * ch + 3)]
                     for ch in range(NCH)}

        items = []  # (deadline, avail, fn)
        for ch in range(1, NCH):
            av = chunk_start[ch - 1]
            for p in range(NPAIR):
                dl = slot_of[(ch, p, 0)] - 6
                items.append((dl, av, lambda ch=ch, p=p:
                              proj_qk(ch, p, 0, bq_sb, qT8)))
                items.append((dl, av, lambda ch=ch, p=p:
                              proj_qk(ch, p, 2, bk_sb, kT8)))
            for tj in range(4 * ch, 4 * (ch + 1)):
                dl = slot_of[(ch, 0, tj)] - 2
                items.append((dl, av, lambda ch=ch, tj=tj: proj_v(ch, tj)))
        for tj in range(4 * (NCH - 2)):
            items.append((nslots - 1, chunk_end[tj // 4] + 1,
                          lambda tj=tj: outproj_tile(tj)))
        items.append((chunk_end[0] + 8, 0, lambda: nc.sync.dma_start(
            wo_sb[:].rearrange("p (t c) -> p t c", t=NPAIR),
            Wo[:].rearrange("(t p) c -> p t c", p=128))))

        assigned = {}
        for dl, av, fn in sorted(items, key=lambda it: -it[0]):
            s = min(dl, nslots - 1)
            while s >= av and s in assigned:
                s -= 1
            if s < av:
                s = av
                while s in assigned:
                    s += 1
            assigned[s] = fn

        fills = {}
        for s, fn in assigned.items():
            fills.setdefault(s, []).append(fn)

        def _slotfill(seq):
            for fl in seq:
                yield fl[0] if fl else None
            while True:
                yield None

        last = NCH - 1
        for ch in range(NCH):
            if ch + 1 < NCH:
                emit_xdma(ch + 1)
            for p in range(NPAIR):
                nk = 4 * ch + 4
                seq = []
                for kt in range(nk):
                    fl = fills.get(slot_of[(ch, p, kt)], [])
                    seq.append(fl)
                it = _slotfill(seq)
                tailjobs = None
                if ch == last and p == NPAIR - 1:
                    tailjobs = [
                        lambda tj=tj: outproj_tile(tj, act_evac=True)
                        for tj in list(range(4 * (last - 1), 4 * last)) +
                        list(range(4 * last, 4 * last + 4))]
                attn_column(p, ch, it, tailjobs)
    nc.compile()
    return nc


class _Runner:
    def __init__(self, nc):
        import jax
        from jax.sharding import Mesh, PartitionSpec, NamedSharding
        from jax.experimental.shard_map import shard_map
        import concourse.mybir as mybir
        from concourse.bass2jax import (_bass_exec_p, partition_id_tensor,
                                        install_neuronx_cc_hook)
        install_neuronx_cc_hook()
        self.jax = jax
        part = nc.partition_id_tensor.name if nc.partition_id_tensor else None
        in_names, out_names, out_avals = [], [], []
        for alloc in nc.m.functions[0].allocations:
            if not isinstance(alloc, mybir.MemoryLocationSet):
                continue
            name = alloc.memorylocations[0].name
            if alloc.kind == "ExternalInput":
                if name != part:
                    in_names.append(name)
            elif alloc.kind == "ExternalOutput":
                out_names.append(name)
                out_avals.append(jax.core.ShapedArray(
                    tuple(alloc.tensor_shape), mybir.dt.np(alloc.dtype)))
        self.in_names, self.out_names, self.out_avals = in_names, out_names, out_avals
        all_in = list(in_names) + list(out_names) + ([part] if part else [])

        def _body(*args):
            ops = list(args)
            if part:
                ops.append(partition_id_tensor())
            return tuple(_bass_exec_p.bind(
                *ops, out_avals=tuple(out_avals), in_names=tuple(all_in),
                out_names=tuple(out_names), lowering_input_output_aliases=(),
                sim_require_finite=True, sim_require_nnan=True, nc=nc))

        devices = jax.devices()[:NCORES]
        mesh = Mesh(np.asarray(devices), ("core",))
        nin = len(in_names) + len(out_names)
        self.fn = jax.jit(
            shard_map(_body, mesh=mesh,
                      in_specs=(PartitionSpec("core"),) * nin,
                      out_specs=(PartitionSpec("core"),) * len(out_names),
                      check_rep=False),
            keep_unused=True)
        self.sharding = NamedSharding(mesh, PartitionSpec("core"))

    def put_inputs(self, in_maps):
        args = []
        for name in self.in_names:
            cat = np.concatenate([np.asarray(m[name]) for m in in_maps], axis=0)
            args.append(self.jax.device_put(cat, self.sharding))
        for av in self.out_avals:
            z = np.zeros((NCORES * av.shape[0], *av.shape[1:]), av.dtype)
            args.append(self.jax.device_put(z, self.sharding))
        return args

    def run_np(self, args):
        outs = self.fn(*args)
        return [
            {n: np.asarray(outs[i]).reshape(NCORES, *self.out_avals[i].shape)[c]
             for i, n in enumerate(self.out_names)}
            for c in range(NCORES)
        ]


def _get_runner():
    if "r" not in _CACHE:
        nc = _build()
        _CACHE["nc"] = nc
        _CACHE["r"] = _Runner(nc)
    return _CACHE["r"]


def _rne11(a):
    """Round fp32 to 11 mantissa bits, round-to-nearest-even (= hw fp32r)."""
    ai = np.ascontiguousarray(a, dtype=np.float32).view(np.uint32).astype(np.uint64)
    lsb = (ai >> 12) & 1
    out = (((ai + 2047 + lsb) >> 12) << 12).astype(np.uint32)
    return out.view(np.float32)


def _q8(a):
    import ml_dtypes
    a = np.clip(np.asarray(a, np.float32), -240.0, 240.0)
    return a.astype(ml_dtypes.float8_e4m3)


def _pack_bias(b):
    # [512] -> [128, NPAIR] with the (j, h, s) psum-partition order,
    # pre-scaled by the q/k fp8 storage scale
    br = (b * QKS).reshape(NPAIR, 2, 2, 32).transpose(2, 1, 3, 0)
    return np.ascontiguousarray(br.reshape(128, NPAIR).astype(np.float32))


def make_in_maps(x, Wqkv, bqkv, Wo, bo=None, mask=None):
    # x repack: [ch, part, g, i, tok], feature = 256g + 128i + part
    xhs, xls = [], []
    for b in range(B):
        xr = np.ascontiguousarray(
            x[b].reshape(NCH, 512, NG, 2, 128).transpose(0, 4, 2, 3, 1))
        h8 = _q8(xr)
        l8 = _q8(xr - h8.astype(np.float32))
        xhs.append(h8.reshape(NCH, 128, NG * 1024))
        xls.append(l8.reshape(NCH, 128, NG * 1024))

    in_maps = []
    for c in range(NCORES):
        b, g = c // 2, c % 2
        sl = slice(g * FPC, (g + 1) * FPC)

        def pack_qk(w):
            # [d, col] -> [part, p, g, i, f]: d = 256g+128i+part, col = 128p+f
            # and f (psum partition) reordered to (j, h, s):
            # feature-within-pair = 64h + 32j + s  ->  f = 64j + 32h + s
            wr = (w * WSC).reshape(NG, 2, 128, NPAIR, 2, 2, 32).transpose(
                2, 3, 0, 1, 5, 4, 6)
            hi = _q8(wr)
            lo = _q8(wr - hi.astype(np.float32))
            return (hi.reshape(128, 4096), lo.reshape(128, 4096))

        def pack_v(w):
            # [d, col] -> [part, g, i, f]: d = 256g+128i+part, col = f(512)
            wr = (w * WSC).reshape(NG, 2, 128, FPC).transpose(2, 0, 1, 3)
            hi = _q8(wr)
            lo = _q8(wr - hi.astype(np.float32))
            return (hi.reshape(128, 4096), lo.reshape(128, 4096))

        qh, ql = pack_qk(Wqkv[:, 0 * D:1 * D][:, sl])
        kh, kl = pack_qk(Wqkv[:, 1 * D:2 * D][:, sl])
        vh, vl = pack_v(Wqkv[:, 2 * D:3 * D][:, sl])

        in_maps.append({
            "xh": xhs[b], "xl": xls[b],
            "Wqh": qh, "Wql": ql, "Wkh": kh, "Wkl": kl, "Wvh": vh, "Wvl": vl,
            "BQ": _pack_bias(bqkv[0 * D:1 * D][sl]),
            "BK": _pack_bias(bqkv[1 * D:2 * D][sl]),
            "BV": np.ascontiguousarray(bqkv[2 * D:3 * D][sl]),
            "Wo": _rne11(Wo[sl, :]),
        })
    return in_maps


def kernel(x, Wqkv, bqkv, Wo, bo, mask=None, **_unused):
    x = np.asarray(x, dtype=np.float32)
    Wqkv = np.asarray(Wqkv, dtype=np.float32)
    bqkv = np.asarray(bqkv, dtype=np.float32)
    Wo = np.asarray(Wo, dtype=np.float32)
    bo = np.asarray(bo, dtype=np.float32)
    in_maps = make_in_maps(x, Wqkv, bqkv, Wo)
    last_err = None
    for _attempt in range(3):
        try:
            r = _get_runner()
            args = r.put_inputs(in_maps)
            res = r.run_np(args)
            break
        except Exception as e:  # transient device wedge: retry fresh
            last_err = e
            _CACHE.clear()
            import time
            time.sleep(5)
    else:
        raise last_err
    out = np.empty((B, T, D), dtype=np.float32)
    for b in range(B):
        out[b] = res[2 * b]["y"] + res[2 * b + 1]["y"] + bo
    return out


# revision 6
# speedup vs baseline: 1.2233x; 1.0145x over previous
"""Causal self-attention Trainium2 kernel, v2.

Sharding: 8 cores = 4 batches x 2 head-groups (8 heads each).

Per-core dataflow:
  - QKV projections as fp8e4 DoubleRow matmuls (256-feature contraction
    per instruction) with a hi/lo split of both x and W (host-prepared):
    q = xh@Wh + xl@Wh + xh@Wl  -- 3 DoubleRow passes = 6N cycles vs
    fp32r's 8N, with ~0.1% error.
  - q,k stored bf16 (rate-1 matmuls at any N, so causal diagonal blocks
    need no 256-col widening); scores per k-tile into PSUM.
  - exp on ACT writes P^T directly as bf16; causal triangle zeroed on
    GPSIMD (affine_select); PV matmuls in bf16 with a ones-column in the
    V tile accumulating softmax denominators.
  - out = PV / rowsum via DVE reciprocal + GPSIMD partition broadcast.
  - y = outT.T @ Wo in fp32r (partial; host sums the 2 head-groups).

Scheduling: projection chains for chunk ch+1 and output-projection tiles
for chunk ch-1 are interleaved between attention k-tiles of chunk ch so
the PE never idles while ACT paces the softmax.
"""
import numpy as np

B, T, D, H = 4, 2048, 1024, 16
HD = D // H            # 64
NCORES = 8
HPC = 8                # heads per core
FPC = HPC * HD         # 512 feature cols per core
NPAIR = HPC // 2       # 4 head pairs
NG = 4                 # fp8 DoubleRow contraction groups (256 feats each)
KT = T // 128          # 16 k-tiles
NCH = T // 512         # 4 q-chunks
WSC = 50.0             # host weight scaling before fp8 quantization
QKS = 16.0             # q/k fp8 storage scale
VSTR = 65              # per-k-tile stride in vaug free dim
HSTR = KT * VSTR       # per-head stride in vaug free dim

_CACHE = {}


def _build():
    import concourse.mybir as mybir
    import concourse.tile as tile
    from concourse import bacc
    from contextlib import ExitStack

    f32 = mybir.dt.float32
    f32r = mybir.dt.float32r
    bf16 = mybir.dt.bfloat16
    f8 = mybir.dt.float8e4
    DR = mybir.MatmulPerfMode.DoubleRow
    Exp = mybir.ActivationFunctionType.Exp
    Alu = mybir.AluOpType

    nc = bacc.Bacc("TRN2", target_bir_lowering=False, debug=False,
                   num_devices=NCORES)
    # x hi/lo fp8, repacked host-side as [ch, 128, g, i, tok]:
    # feature = 256*g + 128*i + partition
    xh = nc.dram_tensor("xh", [NCH, 128, NG * 1024], f8, kind="ExternalInput")
    xl = nc.dram_tensor("xl", [NCH, 128, NG * 1024], f8, kind="ExternalInput")
    # fp8 weights, 6 tensors: q/k: [p][g][i][f128]; v: [g][i][f512]
    Wsec = [nc.dram_tensor(nm, [128, 4096], f8, kind="ExternalInput")
            for nm in ("Wqh", "Wql", "Wkh", "Wkl", "Wvh", "Wvl")]
    BQ = nc.dram_tensor("BQ", [128, NPAIR], f32, kind="ExternalInput")
    BK = nc.dram_tensor("BK", [128, NPAIR], f32, kind="ExternalInput")
    BV = nc.dram_tensor("BV", [FPC], f32, kind="ExternalInput")
    Wo = nc.dram_tensor("Wo", [FPC, D], f32r, kind="ExternalInput")
    y = nc.dram_tensor("y", [T, D], f32, kind="ExternalOutput")

    with tile.TileContext(nc) as tc, ExitStack() as es:
        pers = es.enter_context(tc.tile_pool(name="pers", bufs=1))
        xsp = es.enter_context(tc.tile_pool(name="xsp", bufs=2))
        ptp = es.enter_context(tc.tile_pool(name="ptp", bufs=6))
        nrm = es.enter_context(tc.tile_pool(name="nrm", bufs=2))
        obp = es.enter_context(tc.tile_pool(name="obp", bufs=3))
        stgp = es.enter_context(tc.tile_pool(name="stgp", bufs=4))
        psA = es.enter_context(tc.tile_pool(name="psA", bufs=2, space="PSUM"))
        psS = es.enter_context(tc.tile_pool(name="psS", bufs=2, space="PSUM"))
        psO = es.enter_context(tc.tile_pool(name="psO", bufs=1, space="PSUM"))

        ws_sb = [pers.tile([128, 4096], f8, tag=f"ws{i}", name=f"ws{i}")
                 for i in range(6)]
        bq_sb = pers.tile([128, NPAIR], f32, tag="bq")
        bk_sb = pers.tile([128, NPAIR], f32, tag="bk")
        bv_row = pers.tile([1, FPC], f32, tag="bvr")
        bv_bc = pers.tile([128, FPC], f32, tag="bvb")
        wo_sb = pers.tile([128, NPAIR * D], f32r, tag="wo")
        vaug = pers.tile([128, HPC * HSTR], bf16, tag="vaug")
        # q/k in fp8 for DoubleRow scores: tile u holds pairs (2u, 2u+1);
        # partition = 64*(pr%2) + 32*head + hd%32, free = (hd//32, token)
        qT8 = [pers.tile([128, 2, T], f8, tag=f"qT8{u}", name=f"qT8{u}")
               for u in range(2)]
        kT8 = [pers.tile([128, 2, T], f8, tag=f"kT8{u}", name=f"kT8{u}")
               for u in range(2)]
        outT = [pers.tile([128, T], f32r, tag=f"oT{p}", name=f"oT{p}")
                for p in range(NPAIR)]

        vaug4 = vaug[:].rearrange("p (h k x) -> p h k x", h=HPC, k=KT)

        xtiles = {}

        def emit_xdma(ch):
            xh_sb = xsp.tile([128, NG * 1024], f8, tag="xh", name="xh_sb")
            xl_sb = xsp.tile([128, NG * 1024], f8, tag="xl", name="xl_sb")
            if ch == 0:
                half = NG * 512
                nc.sync.dma_start(xh_sb[:, 0:half], xh[ch][:, 0:half])
                nc.sync.dma_start(xl_sb[:, 0:half], xl[ch][:, 0:half])
                nc.sync.dma_start(xh_sb[:, half:], xh[ch][:, half:])
                nc.sync.dma_start(xl_sb[:, half:], xl[ch][:, half:])
            else:
                nc.sync.dma_start(xh_sb[:], xh[ch])
                nc.sync.dma_start(xl_sb[:], xl[ch])
            xtiles[ch] = [xx[:, g * 1024:(g + 1) * 1024].rearrange(
                "p (i t) -> p i t", i=2)
                for xx in (xh_sb, xl_sb) for g in range(NG)]

        def w_ap(sec, p, g):
            base = p * 1024 + g * 256
            return ws_sb[sec][:, base:base + 256].rearrange(
                "p (i f) -> p i f", i=2)

        def wv_ap(sec, g):
            base = g * 1024
            return ws_sb[sec][:, base:base + 1024].rearrange(
                "p (i f) -> p i f", i=2)

        # (x-part, w-section) term order: xl last so its DMA can trail
        # at startup. psum partitions are ordered (hd-half j, head, hd%32)
        # by the host weight packing; the evac writes scaled fp8 into a
        # staging tile whose two j-halves are then DMA'd into the
        # partition-sliced qT8/kT8 layout.
        def proj_qk(ch, p, sec, bsb, dst):
            xg = xtiles[ch]
            ps = psA.tile([128, 512], f32, tag="psA", name="psqk")
            n = 0
            for (xt, ws) in ((0, sec), (0, sec + 1), (1, sec)):
                for g in range(NG):
                    nc.tensor.matmul(
                        ps[:], w_ap(ws, p, g), xg[xt * NG + g],
                        start=(n == 0), stop=(n == 3 * NG - 1),
                        perf_mode=DR)
                    n += 1
            stg = stgp.tile([128, 512], f8, tag="stg", name="stg")
            nc.vector.tensor_scalar(
                stg[:], ps[:], QKS / WSC, bsb[:, p:p + 1],
                Alu.mult, Alu.add)
            u, e = p // 2, p % 2
            for j in range(2):
                nc.sync.dma_start(
                    dst[u][64 * e:64 * e + 64, j,
                           512 * ch:512 * (ch + 1)],
                    stg[64 * j:64 * j + 64, :])

        def proj_v(ch, tj):
            xg = xtiles[ch]
            lo = (tj - 4 * ch) * 128
            ps = psA.tile([128, FPC], f32, tag="psA", name="psv")
            n = 0
            for (xt, ws) in ((0, 4), (0, 5), (1, 4)):
                for g in range(NG):
                    nc.tensor.matmul(
                        ps[:], xg[xt * NG + g][:, :, lo:lo + 128],
                        wv_ap(ws, g),
                        start=(n == 0), stop=(n == 3 * NG - 1),
                        perf_mode=DR)
                    n += 1
            nc.vector.scalar_tensor_tensor(
                vaug4[:, :, tj, 0:64],
                ps[:].rearrange("p (h x) -> p h x", h=HPC),
                1.0 / WSC,
                bv_bc[:].rearrange("p (h x) -> p h x", h=HPC),
                Alu.mult, Alu.add)

        def proj_chunk_fill(ch):
            out = []
            for p in range(NPAIR):
                out.append(lambda p=p: proj_qk(ch, p, 0, bq_sb, qT8))
            for p in range(NPAIR):
                out.append(lambda p=p: proj_qk(ch, p, 2, bk_sb, kT8))
            for tj in range(4 * ch, 4 * (ch + 1)):
                out.append(lambda tj=tj: proj_v(ch, tj))
            return out

        Copy = mybir.ActivationFunctionType.Copy

        def outproj_tile(tj, act_evac=False):
            ob = obp.tile([128, D], f32, tag="ob", name="ob")
            for n in range(2):
                ps = psA.tile([128, 512], f32, tag="psA", name="pspr")
                for p in range(NPAIR):
                    nc.tensor.matmul(
                        ps[:],
                        outT[p][:, 128 * tj:128 * (tj + 1)],
                        wo_sb[:, p * D + 512 * n:p * D + 512 * (n + 1)],
                        start=(p == 0), stop=(p == NPAIR - 1))
                if act_evac:
                    # ACT evac keeps the tail normalize off the DVE queue
                    nc.scalar.activation(ob[:, 512 * n:512 * (n + 1)],
                                         ps[:], Copy)
                else:
                    nc.vector.tensor_copy(ob[:, 512 * n:512 * (n + 1)],
                                          ps[:])
            nc.sync.dma_start(y[128 * tj:128 * (tj + 1), :], ob[:])

        def emit_norm(p, ch, po, c0, c1):
            w = c1 - c0
            for h in range(2):
                og = nrm.tile([65, 512], f32, tag="og", name="og")
                nc.vector.tensor_copy(og[:, 0:w], po[h][:, c0:c1])
                rec = nrm.tile([1, 512], f32, tag="rec", name="rec")
                nc.vector.reciprocal(rec[:, 0:w], og[64:65, 0:w])
                bc = nrm.tile([64, 512], f32, tag="bc", name="bc")
                nc.gpsimd.partition_broadcast(bc[:, 0:w], rec[:, 0:w])
                nc.vector.tensor_mul(
                    outT[p][64 * h:64 * (h + 1),
                            512 * ch + c0:512 * ch + c1],
                    og[0:64, 0:w], bc[:, 0:w])

        def attn_column(p, ch, fill, tailjobs=None):
            """fill: iterator of emitters to interleave between k-tiles."""
            po = [psO.tile([65, 512], f32, tag=f"po{h}", name=f"po{h}")
                  for h in range(2)]
            nk = 4 * ch + 4

            def emit_pv(kt, pt):
                qo = max(kt - 4 * ch, 0) * 128
                for h in range(2):
                    nc.tensor.matmul(
                        po[h][:, qo:],
                        vaug4[:, 2 * p + h, kt, :],
                        pt[:, h, qo:],
                        start=(kt == 0), stop=(kt == nk - 1),
                        skip_group_check=True)

            pend = []
            for kt in range(nk):
                dg = kt - 4 * ch
                qo = max(dg, 0) * 128
                pt = ptp.tile([128, 2, 512], bf16, tag="pt", name="pt")
                ss = psS.tile([128, 2, 512], f32, tag="psS", name="ss")
                u, e = p // 2, p % 2
                for h in range(2):
                    b0 = 64 * e + 32 * h
                    nc.tensor.matmul(
                        ss[:, h, qo:],
                        kT8[u][b0:b0 + 32, :, 128 * kt:128 * (kt + 1)],
                        qT8[u][b0:b0 + 32, :,
                               512 * ch + qo:512 * (ch + 1)],
                        start=True, stop=True, perf_mode=DR,
                        tile_position=(b0, 0))
                nc.scalar.activation(pt[:, :, qo:], ss[:, :, qo:],
                                     Exp, scale=0.125 / (QKS * QKS))
                if dg >= 0:
                    # zero the upper (q < k) triangle of the diagonal block
                    blk = pt[:, :, qo:qo + 128]
                    nc.gpsimd.affine_select(
                        out=blk, in_=blk,
                        compare_op=mybir.AluOpType.is_ge,
                        fill=0.0, base=0,
                        pattern=[[0, 2], [1, 128]],
                        channel_multiplier=-1)
                # fill, then 2-tile-deep software-pipelined PV so the
                # PE never waits inline on an exp
                f = next(fill, None)
                if f:
                    f()
                pend.append((kt, pt))
                if len(pend) > 2:
                    emit_pv(*pend.pop(0))
            if tailjobs is None:
                for pv in pend:
                    f = next(fill, None)
                    if f:
                        f()
                    emit_pv(*pv)
                emit_norm(p, ch, po, 0, 512)
            else:
                # last column: po[:, 0:128*q] is complete as soon as the
                # q-th diagonal PV lands -- normalize and project each
                # quarter as it completes, with reserved chunk-2 output
                # tiles filling each normalize chain's latency
                emit_norm(p, ch, po, 0, 128)
                tailjobs[0]()
                tailjobs[4]()
                emit_norm(p, ch, po, 128, 256)
                tailjobs[1]()
                emit_pv(*pend.pop(0))
                tailjobs[5]()
                emit_pv(*pend.pop(0))
                emit_norm(p, ch, po, 256, 384)
                emit_norm(p, ch, po, 384, 512)
                tailjobs[2]()
                tailjobs[6]()
                tailjobs[3]()
                tailjobs[7]()

        # ---------------- schedule ----------------
        # startup DMA order: q weights and x first (in first-use order,
        # split so the first projection chain can start ~4us in)
        H2 = 2048
        nc.sync.dma_start(ws_sb[0][:, 0:H2], Wsec[0][:, 0:H2])
        emit_xdma(0)
        nc.sync.dma_start(ws_sb[1][:, 0:H2], Wsec[1][:, 0:H2])
        nc.sync.dma_start(bq_sb[:], BQ[:])
        nc.sync.dma_start(bv_row[:], BV[:])
        nc.sync.dma_start(ws_sb[0][:, H2:], Wsec[0][:, H2:])
        nc.sync.dma_start(ws_sb[1][:, H2:], Wsec[1][:, H2:])
        nc.sync.dma_start(ws_sb[2][:], Wsec[2][:])
        nc.sync.dma_start(ws_sb[3][:], Wsec[3][:])
        nc.sync.dma_start(bk_sb[:], BK[:])
        nc.gpsimd.partition_broadcast(bv_bc[:], bv_row[:])
        nc.vector.memset(vaug4[:, :, :, 64:65], 1.0)

        fill0 = proj_chunk_fill(0)
        # q p0-3, then V weights, k p0-3, then x prefetch for chunk 1
        for f in fill0[:4]:
            f()
        nc.sync.dma_start(ws_sb[4][:], Wsec[4][:])
        nc.sync.dma_start(ws_sb[5][:], Wsec[5][:])
        for f in fill0[4:8]:
            f()
        emit_xdma(1)
        for f in fill0[8:]:
            f()

        # ---- deadline-packed fill assignment over the k-tile slots ----
        slots = []           # (ch, p, kt) in emission order
        slot_of = {}
        for ch in range(NCH):
            for p in range(NPAIR):
                for kt in range(4 * ch + 4):
                    slot_of[(ch, p, kt)] = len(slots)
                    slots.append((ch, p, kt))
        nslots = len(slots)
        chunk_start = {ch: slot_of[(ch, 0, 0)] for ch in range(NCH)}
        chunk_end = {ch: slot_of[(ch, NPAIR - 1, 4 * ch + 3)]
                     for ch in range(NCH)}

        items = []  # (deadline, avail, fn)
        for ch in range(1, NCH):
            av = chunk_start[ch - 1]
            for p in range(NPAIR):
                dl = slot_of[(ch, p, 0)] - 6
                items.append((dl, av, lambda ch=ch, p=p:
                              proj_qk(ch, p, 0, bq_sb, qT8)))
                items.append((dl, av, lambda ch=ch, p=p:
                              proj_qk(ch, p, 2, bk_sb, kT8)))
            for tj in range(4 * ch, 4 * (ch + 1)):
                dl = slot_of[(ch, 0, tj)] - 2
                items.append((dl, av, lambda ch=ch, tj=tj: proj_v(ch, tj)))
        for tj in range(4 * (NCH - 2)):
            items.append((nslots - 1, chunk_end[tj // 4] + 1,
                          lambda tj=tj: outproj_tile(tj)))
        items.append((chunk_end[0] + 8, 0, lambda: nc.sync.dma_start(
            wo_sb[:].rearrange("p (t c) -> p t c", t=NPAIR),
            Wo[:].rearrange("(t p) c -> p t c", p=128))))

        assigned = {}
        for dl, av, fn in sorted(items, key=lambda it: -it[0]):
            s = min(dl, nslots - 1)
            while s >= av and s in assigned:
                s -= 1
            if s < av:
                s = av
                while s in assigned:
                    s += 1
            assigned[s] = fn

        fills = {}
        for s, fn in assigned.items():
            fills.setdefault(s, []).append(fn)

        def _slotfill(seq):
            for fl in seq:
                yield fl[0] if fl else None
            while True:
                yield None

        last = NCH - 1
        for ch in range(NCH):
            if 0 < ch and ch + 1 < NCH:
                emit_xdma(ch + 1)
            for p in range(NPAIR):
                nk = 4 * ch + 4
                seq = []
                for kt in range(nk):
                    fl = fills.get(slot_of[(ch, p, kt)], [])
                    seq.append(fl)
                it = _slotfill(seq)
                tailjobs = None
                if ch == last and p == NPAIR - 1:
                    tailjobs = [
                        lambda tj=tj: outproj_tile(tj, act_evac=True)
                        for tj in list(range(4 * (last - 1), 4 * last)) +
                        list(range(4 * last, 4 * last + 4))]
                attn_column(p, ch, it, tailjobs)
    nc.compile()
    return nc


class _Runner:
    def __init__(self, nc):
        import jax
        from jax.sharding import Mesh, PartitionSpec, NamedSharding
        from jax.experimental.shard_map import shard_map
        import concourse.mybir as mybir
        from concourse.bass2jax import (_bass_exec_p, partition_id_tensor,
                                        install_neuronx_cc_hook)
        install_neuronx_cc_hook()
        self.jax = jax
        part = nc.partition_id_tensor.name if nc.partition_id_tensor else None
        in_names, out_names, out_avals = [], [], []
        for alloc in nc.m.functions[0].allocations:
            if not isinstance(alloc, mybir.MemoryLocationSet):
                continue
            name = alloc.memorylocations[0].name
            if alloc.kind == "ExternalInput":
                if name != part:
                    in_names.append(name)
            elif alloc.kind == "ExternalOutput":
                out_names.append(name)
                out_avals.append(jax.core.ShapedArray(
                    tuple(alloc.tensor_shape), mybir.dt.np(alloc.dtype)))
        self.in_names, self.out_names, self.out_avals = in_names, out_names, out_avals
        all_in = list(in_names) + list(out_names) + ([part] if part else [])

        def _body(*args):
            ops = list(args)
            if part:
                ops.append(partition_id_tensor())
            return tuple(_bass_exec_p.bind(
                *ops, out_avals=tuple(out_avals), in_names=tuple(all_in),
                out_names=tuple(out_names), lowering_input_output_aliases=(),
                sim_require_finite=True, sim_require_nnan=True, nc=nc))

        devices = jax.devices()[:NCORES]
        mesh = Mesh(np.asarray(devices), ("core",))
        nin = len(in_names) + len(out_names)
        self.fn = jax.jit(
            shard_map(_body, mesh=mesh,
                      in_specs=(PartitionSpec("core"),) * nin,
                      out_specs=(PartitionSpec("core"),) * len(out_names),
                      check_rep=False),
            keep_unused=True)
        self.sharding = NamedSharding(mesh, PartitionSpec("core"))

    def put_inputs(self, in_maps):
        args = []
        for name in self.in_names:
            cat = np.concatenate([np.asarray(m[name]) for m in in_maps], axis=0)
            args.append(self.jax.device_put(cat, self.sharding))
        for av in self.out_avals:
            z = np.zeros((NCORES * av.shape[0], *av.shape[1:]), av.dtype)
            args.append(self.jax.device_put(z, self.sharding))
        return args

    def run_np(self, args):
        outs = self.fn(*args)
        return [
            {n: np.asarray(outs[i]).reshape(NCORES, *self.out_avals[i].shape)[c]
             for i, n in enumerate(self.out_names)}
            for c in range(NCORES)
        ]


def _get_runner():
    if "r" not in _CACHE:
        nc = _build()
        _CACHE["nc"] = nc
        _CACHE["r"] = _Runner(nc)
    return _CACHE["r"]


def _rne11(a):
    """Round fp32 to 11 mantissa bits, round-to-nearest-even (= hw fp32r)."""
    ai = np.ascontiguousarray(a, dtype=np.float32).view(np.uint32).astype(np.uint64)
    lsb = (ai >> 12) & 1
    out = (((ai + 2047 + lsb) >> 12) << 12).astype(np.uint32)
    return out.view(np.float32)


def _q8(a):
    import ml_dtypes
    a = np.clip(np.asarray(a, np.float32), -240.0, 240.0)
    return a.astype(ml_dtypes.float8_e4m3)


def _pack_bias(b):
    # [512] -> [128, NPAIR] with the (j, h, s) psum-partition order,
    # pre-scaled by the q/k fp8 storage scale
    br = (b * QKS).reshape(NPAIR, 2, 2, 32).transpose(2, 1, 3, 0)
    return np.ascontiguousarray(br.reshape(128, NPAIR).astype(np.float32))


def make_in_maps(x, Wqkv, bqkv, Wo, bo=None, mask=None):
    # x repack: [ch, part, g, i, tok], feature = 256g + 128i + part
    xhs, xls = [], []
    for b in range(B):
        xr = np.ascontiguousarray(
            x[b].reshape(NCH, 512, NG, 2, 128).transpose(0, 4, 2, 3, 1))
        h8 = _q8(xr)
        l8 = _q8(xr - h8.astype(np.float32))
        xhs.append(h8.reshape(NCH, 128, NG * 1024))
        xls.append(l8.reshape(NCH, 128, NG * 1024))

    in_maps = []
    for c in range(NCORES):
        b, g = c // 2, c % 2
        sl = slice(g * FPC, (g + 1) * FPC)

        def pack_qk(w):
            # [d, col] -> [part, p, g, i, f]: d = 256g+128i+part, col = 128p+f
            # and f (psum partition) reordered to (j, h, s):
            # feature-within-pair = 64h + 32j + s  ->  f = 64j + 32h + s
            wr = (w * WSC).reshape(NG, 2, 128, NPAIR, 2, 2, 32).transpose(
                2, 3, 0, 1, 5, 4, 6)
            hi = _q8(wr)
            lo = _q8(wr - hi.astype(np.float32))
            return (hi.reshape(128, 4096), lo.reshape(128, 4096))

        def pack_v(w):
            # [d, col] -> [part, g, i, f]: d = 256g+128i+part, col = f(512)
            wr = (w * WSC).reshape(NG, 2, 128, FPC).transpose(2, 0, 1, 3)
            hi = _q8(wr)
            lo = _q8(wr - hi.astype(np.float32))
            return (hi.reshape(128, 4096), lo.reshape(128, 4096))

        qh, ql = pack_qk(Wqkv[:, 0 * D:1 * D][:, sl])
        kh, kl = pack_qk(Wqkv[:, 1 * D:2 * D][:, sl])
        vh, vl = pack_v(Wqkv[:, 2 * D:3 * D][:, sl])

        in_maps.append({
            "xh": xhs[b], "xl": xls[b],
            "Wqh": qh, "Wql": ql, "Wkh": kh, "Wkl": kl, "Wvh": vh, "Wvl": vl,
            "BQ": _pack_bias(bqkv[0 * D:1 * D][sl]),
            "BK": _pack_bias(bqkv[1 * D:2 * D][sl]),
            "BV": np.ascontiguousarray(bqkv[2 * D:3 * D][sl]),
            "Wo": _rne11(Wo[sl, :]),
        })
    return in_maps


def kernel(x, Wqkv, bqkv, Wo, bo, mask=None, **_unused):
    x = np.asarray(x, dtype=np.float32)
    Wqkv = np.asarray(Wqkv, dtype=np.float32)
    bqkv = np.asarray(bqkv, dtype=np.float32)
    Wo = np.asarray(Wo, dtype=np.float32)
    bo = np.asarray(bo, dtype=np.float32)
    in_maps = make_in_maps(x, Wqkv, bqkv, Wo)
    last_err = None
    for _attempt in range(3):
        try:
            r = _get_runner()
            args = r.put_inputs(in_maps)
            res = r.run_np(args)
            break
        except Exception as e:  # transient device wedge: retry fresh
            last_err = e
            _CACHE.clear()
            import time
            time.sleep(5)
    else:
        raise last_err
    out = np.empty((B, T, D), dtype=np.float32)
    for b in range(B):
        out[b] = res[2 * b]["y"] + res[2 * b + 1]["y"] + bo
    return out


# revision 7
# speedup vs baseline: 1.2416x; 1.0150x over previous
"""Causal self-attention Trainium2 kernel, v2.

Sharding: 8 cores = 4 batches x 2 head-groups (8 heads each).

Per-core dataflow:
  - QKV projections as fp8e4 DoubleRow matmuls (256-feature contraction
    per instruction) with a hi/lo split of both x and W (host-prepared):
    q = xh@Wh + xl@Wh + xh@Wl  -- 3 DoubleRow passes = 6N cycles vs
    fp32r's 8N, with ~0.1% error.
  - q,k stored bf16 (rate-1 matmuls at any N, so causal diagonal blocks
    need no 256-col widening); scores per k-tile into PSUM.
  - exp on ACT writes P^T directly as bf16; causal triangle zeroed on
    GPSIMD (affine_select); PV matmuls in bf16 with a ones-column in the
    V tile accumulating softmax denominators.
  - out = PV / rowsum via DVE reciprocal + GPSIMD partition broadcast.
  - y = outT.T @ Wo in fp32r (partial; host sums the 2 head-groups).

Scheduling: projection chains for chunk ch+1 and output-projection tiles
for chunk ch-1 are interleaved between attention k-tiles of chunk ch so
the PE never idles while ACT paces the softmax.
"""
import numpy as np

B, T, D, H = 4, 2048, 1024, 16
HD = D // H            # 64
NCORES = 8
HPC = 8                # heads per core
FPC = HPC * HD         # 512 feature cols per core
NPAIR = HPC // 2       # 4 head pairs
NG = 4                 # fp8 DoubleRow contraction groups (256 feats each)
KT = T // 128          # 16 k-tiles
NCH = T // 512         # 4 q-chunks
WSC = 50.0             # host weight scaling before fp8 quantization
QKS = 16.0             # q/k fp8 storage scale
VSTR = 65              # per-k-tile stride in vaug free dim
HSTR = KT * VSTR       # per-head stride in vaug free dim

_CACHE = {}


def _build():
    import concourse.mybir as mybir
    import concourse.tile as tile
    from concourse import bacc
    from contextlib import ExitStack

    f32 = mybir.dt.float32
    f32r = mybir.dt.float32r
    bf16 = mybir.dt.bfloat16
    f8 = mybir.dt.float8e4
    DR = mybir.MatmulPerfMode.DoubleRow
    Exp = mybir.ActivationFunctionType.Exp
    Alu = mybir.AluOpType

    nc = bacc.Bacc("TRN2", target_bir_lowering=False, debug=False,
                   num_devices=NCORES)
    # x hi/lo fp8, repacked host-side as [ch, 128, g, i, tok]:
    # feature = 256*g + 128*i + partition
    xh = nc.dram_tensor("xh", [NCH, 128, NG * 1024], f8, kind="ExternalInput")
    xl = nc.dram_tensor("xl", [NCH, 128, NG * 1024], f8, kind="ExternalInput")
    # fp8 weights, 6 tensors: q/k: [p][g][i][f128]; v: [g][i][f512]
    Wsec = [nc.dram_tensor(nm, [128, 4096], f8, kind="ExternalInput")
            for nm in ("Wqh", "Wql", "Wkh", "Wkl", "Wvh", "Wvl")]
    BQ = nc.dram_tensor("BQ", [128, NPAIR], f32, kind="ExternalInput")
    BK = nc.dram_tensor("BK", [128, NPAIR], f32, kind="ExternalInput")
    BV = nc.dram_tensor("BV", [FPC], f32, kind="ExternalInput")
    Wo = nc.dram_tensor("Wo", [FPC, D], f32r, kind="ExternalInput")
    y = nc.dram_tensor("y", [T, D], f32, kind="ExternalOutput")

    with tile.TileContext(nc) as tc, ExitStack() as es:
        pers = es.enter_context(tc.tile_pool(name="pers", bufs=1))
        xsp = es.enter_context(tc.tile_pool(name="xsp", bufs=2))
        ptp = es.enter_context(tc.tile_pool(name="ptp", bufs=8))
        nrm = es.enter_context(tc.tile_pool(name="nrm", bufs=2))
        obp = es.enter_context(tc.tile_pool(name="obp", bufs=3))
        stgp = es.enter_context(tc.tile_pool(name="stgp", bufs=4))
        psA = es.enter_context(tc.tile_pool(name="psA", bufs=2, space="PSUM"))
        psS = es.enter_context(tc.tile_pool(name="psS", bufs=2, space="PSUM"))
        psO = es.enter_context(tc.tile_pool(name="psO", bufs=1, space="PSUM"))

        ws_sb = [pers.tile([128, 4096], f8, tag=f"ws{i}", name=f"ws{i}")
                 for i in range(6)]
        bq_sb = pers.tile([128, NPAIR], f32, tag="bq")
        bk_sb = pers.tile([128, NPAIR], f32, tag="bk")
        bv_row = pers.tile([1, FPC], f32, tag="bvr")
        bv_bc = pers.tile([128, FPC], f32, tag="bvb")
        wo_sb = pers.tile([128, NPAIR * D], f32r, tag="wo")
        vaug = pers.tile([128, HPC * HSTR], bf16, tag="vaug")
        # q/k in fp8 for DoubleRow scores: tile u holds pairs (2u, 2u+1);
        # partition = 64*(pr%2) + 32*head + hd%32, free = (hd//32, token)
        qT8 = [pers.tile([128, 2, T], f8, tag=f"qT8{u}", name=f"qT8{u}")
               for u in range(2)]
        kT8 = [pers.tile([128, 2, T], f8, tag=f"kT8{u}", name=f"kT8{u}")
               for u in range(2)]
        outT = [pers.tile([128, T], f32r, tag=f"oT{p}", name=f"oT{p}")
                for p in range(NPAIR)]

        vaug4 = vaug[:].rearrange("p (h k x) -> p h k x", h=HPC, k=KT)

        xtiles = {}

        def emit_xdma(ch):
            xh_sb = xsp.tile([128, NG * 1024], f8, tag="xh", name="xh_sb")
            xl_sb = xsp.tile([128, NG * 1024], f8, tag="xl", name="xl_sb")
            if ch == 0:
                half = NG * 512
                nc.sync.dma_start(xh_sb[:, 0:half], xh[ch][:, 0:half])
                nc.sync.dma_start(xl_sb[:, 0:half], xl[ch][:, 0:half])
                nc.sync.dma_start(xh_sb[:, half:], xh[ch][:, half:])
                nc.sync.dma_start(xl_sb[:, half:], xl[ch][:, half:])
            else:
                nc.sync.dma_start(xh_sb[:], xh[ch])
                nc.sync.dma_start(xl_sb[:], xl[ch])
            xtiles[ch] = [xx[:, g * 1024:(g + 1) * 1024].rearrange(
                "p (i t) -> p i t", i=2)
                for xx in (xh_sb, xl_sb) for g in range(NG)]

        def w_ap(sec, p, g):
            base = p * 1024 + g * 256
            return ws_sb[sec][:, base:base + 256].rearrange(
                "p (i f) -> p i f", i=2)

        def wv_ap(sec, g):
            base = g * 1024
            return ws_sb[sec][:, base:base + 1024].rearrange(
                "p (i f) -> p i f", i=2)

        # (x-part, w-section) term order: xl last so its DMA can trail
        # at startup. psum partitions are ordered (hd-half j, head, hd%32)
        # by the host weight packing; the evac writes scaled fp8 into a
        # staging tile whose two j-halves are then DMA'd into the
        # partition-sliced qT8/kT8 layout.
        def proj_qk(ch, p, sec, bsb, dst):
            xg = xtiles[ch]
            ps = psA.tile([128, 512], f32, tag="psA", name="psqk")
            n = 0
            for (xt, ws) in ((0, sec), (0, sec + 1), (1, sec)):
                for g in range(NG):
                    nc.tensor.matmul(
                        ps[:], w_ap(ws, p, g), xg[xt * NG + g],
                        start=(n == 0), stop=(n == 3 * NG - 1),
                        perf_mode=DR)
                    n += 1
            stg = stgp.tile([128, 512], f8, tag="stg", name="stg")
            nc.vector.tensor_scalar(
                stg[:], ps[:], QKS / WSC, bsb[:, p:p + 1],
                Alu.mult, Alu.add)
            u, e = p // 2, p % 2
            for j in range(2):
                nc.sync.dma_start(
                    dst[u][64 * e:64 * e + 64, j,
                           512 * ch:512 * (ch + 1)],
                    stg[64 * j:64 * j + 64, :])

        def proj_v(ch, tj):
            xg = xtiles[ch]
            lo = (tj - 4 * ch) * 128
            ps = psA.tile([128, FPC], f32, tag="psA", name="psv")
            n = 0
            for (xt, ws) in ((0, 4), (0, 5), (1, 4)):
                for g in range(NG):
                    nc.tensor.matmul(
                        ps[:], xg[xt * NG + g][:, :, lo:lo + 128],
                        wv_ap(ws, g),
                        start=(n == 0), stop=(n == 3 * NG - 1),
                        perf_mode=DR)
                    n += 1
            nc.vector.scalar_tensor_tensor(
                vaug4[:, :, tj, 0:64],
                ps[:].rearrange("p (h x) -> p h x", h=HPC),
                1.0 / WSC,
                bv_bc[:].rearrange("p (h x) -> p h x", h=HPC),
                Alu.mult, Alu.add)

        def proj_chunk_fill(ch):
            out = []
            for p in range(NPAIR):
                out.append(lambda p=p: proj_qk(ch, p, 0, bq_sb, qT8))
            for p in range(NPAIR):
                out.append(lambda p=p: proj_qk(ch, p, 2, bk_sb, kT8))
            for tj in range(4 * ch, 4 * (ch + 1)):
                out.append(lambda tj=tj: proj_v(ch, tj))
            return out

        Copy = mybir.ActivationFunctionType.Copy

        def outproj_tile(tj, act_evac=False):
            ob = obp.tile([128, D], f32, tag="ob", name="ob")
            for n in range(2):
                ps = psA.tile([128, 512], f32, tag="psA", name="pspr")
                for p in range(NPAIR):
                    nc.tensor.matmul(
                        ps[:],
                        outT[p][:, 128 * tj:128 * (tj + 1)],
                        wo_sb[:, p * D + 512 * n:p * D + 512 * (n + 1)],
                        start=(p == 0), stop=(p == NPAIR - 1))
                if act_evac:
                    # ACT evac + per-half writeback shortens the final
                    # drain (keeps the tail off the DVE queue too)
                    nc.scalar.activation(ob[:, 512 * n:512 * (n + 1)],
                                         ps[:], Copy)
                    nc.sync.dma_start(
                        y[128 * tj:128 * (tj + 1),
                          512 * n:512 * (n + 1)],
                        ob[:, 512 * n:512 * (n + 1)])
                else:
                    nc.vector.tensor_copy(ob[:, 512 * n:512 * (n + 1)],
                                          ps[:])
            if not act_evac:
                nc.sync.dma_start(y[128 * tj:128 * (tj + 1), :], ob[:])

        def emit_norm(p, ch, po, c0, c1):
            w = c1 - c0
            for h in range(2):
                og = nrm.tile([65, 512], f32, tag="og", name="og")
                nc.vector.tensor_copy(og[:, 0:w], po[h][:, c0:c1])
                rec = nrm.tile([1, 512], f32, tag="rec", name="rec")
                nc.vector.reciprocal(rec[:, 0:w], og[64:65, 0:w])
                bc = nrm.tile([64, 512], f32, tag="bc", name="bc")
                nc.gpsimd.partition_broadcast(bc[:, 0:w], rec[:, 0:w])
                nc.vector.tensor_mul(
                    outT[p][64 * h:64 * (h + 1),
                            512 * ch + c0:512 * ch + c1],
                    og[0:64, 0:w], bc[:, 0:w])

        def attn_column(p, ch, fill, tailjobs=None):
            """fill: iterator of emitters to interleave between k-tiles."""
            po = [psO.tile([65, 512], f32, tag=f"po{h}", name=f"po{h}")
                  for h in range(2)]
            nk = 4 * ch + 4

            def emit_pv(kt, pt):
                qo = max(kt - 4 * ch, 0) * 128
                for h in range(2):
                    nc.tensor.matmul(
                        po[h][:, qo:],
                        vaug4[:, 2 * p + h, kt, :],
                        pt[:, h, qo:],
                        start=(kt == 0), stop=(kt == nk - 1),
                        skip_group_check=True)

            pend = []
            for kt in range(nk):
                dg = kt - 4 * ch
                qo = max(dg, 0) * 128
                pt = ptp.tile([128, 2, 512], bf16, tag="pt", name="pt")
                ss = psS.tile([128, 2, 512], f32, tag="psS", name="ss")
                u, e = p // 2, p % 2
                for h in range(2):
                    b0 = 64 * e + 32 * h
                    nc.tensor.matmul(
                        ss[:, h, qo:],
                        kT8[u][b0:b0 + 32, :, 128 * kt:128 * (kt + 1)],
                        qT8[u][b0:b0 + 32, :,
                               512 * ch + qo:512 * (ch + 1)],
                        start=True, stop=True, perf_mode=DR,
                        tile_position=(b0, 0))
                nc.scalar.activation(pt[:, :, qo:], ss[:, :, qo:],
                                     Exp, scale=0.125 / (QKS * QKS))
                if dg >= 0:
                    # zero the upper (q < k) triangle of the diagonal block
                    blk = pt[:, :, qo:qo + 128]
                    nc.gpsimd.affine_select(
                        out=blk, in_=blk,
                        compare_op=mybir.AluOpType.is_ge,
                        fill=0.0, base=0,
                        pattern=[[0, 2], [1, 128]],
                        channel_multiplier=-1)
                # fill, then 2-tile-deep software-pipelined PV so the
                # PE never waits inline on an exp
                f = next(fill, None)
                if f:
                    f()
                pend.append((kt, pt))
                if len(pend) > 4:
                    emit_pv(*pend.pop(0))
            if tailjobs is None:
                for pv in pend:
                    f = next(fill, None)
                    if f:
                        f()
                    emit_pv(*pv)
                emit_norm(p, ch, po, 0, 512)
            else:
                # last column: po[:, 0:128*q] is complete as soon as the
                # q-th diagonal PV lands -- normalize and project each
                # quarter as it completes, with reserved chunk-2 output
                # tiles filling each normalize chain's latency
                while len(pend) > 2:
                    emit_pv(*pend.pop(0))
                emit_norm(p, ch, po, 0, 128)
                tailjobs[0]()
                tailjobs[4]()
                emit_norm(p, ch, po, 128, 256)
                tailjobs[1]()
                emit_pv(*pend.pop(0))
                tailjobs[5]()
                emit_pv(*pend.pop(0))
                emit_norm(p, ch, po, 256, 384)
                emit_norm(p, ch, po, 384, 512)
                tailjobs[2]()
                tailjobs[6]()
                tailjobs[3]()
                tailjobs[7]()

        # ---------------- schedule ----------------
        # startup DMA order: q weights and x first (in first-use order,
        # split so the first projection chain can start ~4us in)
        H2 = 2048
        nc.sync.dma_start(ws_sb[0][:, 0:H2], Wsec[0][:, 0:H2])
        emit_xdma(0)
        nc.sync.dma_start(ws_sb[1][:, 0:H2], Wsec[1][:, 0:H2])
        nc.sync.dma_start(bq_sb[:], BQ[:])
        nc.sync.dma_start(bv_row[:], BV[:])
        nc.sync.dma_start(ws_sb[0][:, H2:], Wsec[0][:, H2:])
        nc.sync.dma_start(ws_sb[1][:, H2:], Wsec[1][:, H2:])
        nc.sync.dma_start(ws_sb[2][:], Wsec[2][:])
        nc.sync.dma_start(ws_sb[3][:], Wsec[3][:])
        nc.sync.dma_start(bk_sb[:], BK[:])
        nc.gpsimd.partition_broadcast(bv_bc[:], bv_row[:])
        nc.vector.memset(vaug4[:, :, :, 64:65], 1.0)

        fill0 = proj_chunk_fill(0)
        # q p0-3, then V weights, k p0-3, then x prefetch for chunk 1
        for f in fill0[:4]:
            f()
        nc.sync.dma_start(ws_sb[4][:], Wsec[4][:])
        nc.sync.dma_start(ws_sb[5][:], Wsec[5][:])
        for f in fill0[4:8]:
            f()
        emit_xdma(1)
        for f in fill0[8:]:
            f()

        # ---- deadline-packed fill assignment over the k-tile slots ----
        slots = []           # (ch, p, kt) in emission order
        slot_of = {}
        for ch in range(NCH):
            for p in range(NPAIR):
                for kt in range(4 * ch + 4):
                    slot_of[(ch, p, kt)] = len(slots)
                    slots.append((ch, p, kt))
        nslots = len(slots)
        chunk_start = {ch: slot_of[(ch, 0, 0)] for ch in range(NCH)}
        chunk_end = {ch: slot_of[(ch, NPAIR - 1, 4 * ch + 3)]
                     for ch in range(NCH)}

        items = []  # (deadline, avail, fn)
        for ch in range(1, NCH):
            av = chunk_start[ch - 1]
            for p in range(NPAIR):
                dl = slot_of[(ch, p, 0)] - 8
                items.append((dl, av, lambda ch=ch, p=p:
                              proj_qk(ch, p, 0, bq_sb, qT8)))
                items.append((dl, av, lambda ch=ch, p=p:
                              proj_qk(ch, p, 2, bk_sb, kT8)))
            for tj in range(4 * ch, 4 * (ch + 1)):
                dl = slot_of[(ch, 0, tj)] - 2
                items.append((dl, av, lambda ch=ch, tj=tj: proj_v(ch, tj)))
        for tj in range(4 * (NCH - 2)):
            items.append((nslots - 1, chunk_end[tj // 4] + 1,
                          lambda tj=tj: outproj_tile(tj)))
        items.append((chunk_end[0] + 8, 0, lambda: nc.sync.dma_start(
            wo_sb[:].rearrange("p (t c) -> p t c", t=NPAIR),
            Wo[:].rearrange("(t p) c -> p t c", p=128))))

        assigned = {}
        for dl, av, fn in sorted(items, key=lambda it: -it[0]):
            s = min(dl, nslots - 1)
            while s >= av and s in assigned:
                s -= 1
            if s < av:
                s = av
                while s in assigned:
                    s += 1
            assigned[s] = fn

        fills = {}
        for s, fn in assigned.items():
            fills.setdefault(s, []).append(fn)

        def _slotfill(seq):
            for fl in seq:
                yield fl[0] if fl else None
            while True:
                yield None

        last = NCH - 1
        for ch in range(NCH):
            if 0 < ch and ch + 1 < NCH:
                emit_xdma(ch + 1)
            for p in range(NPAIR):
                nk = 4 * ch + 4
                seq = []
                for kt in range(nk):
                    fl = fills.get(slot_of[(ch, p, kt)], [])
                    seq.append(fl)
                it = _slotfill(seq)
                tailjobs = None
                if ch == last and p == NPAIR - 1:
                    tailjobs = [
                        lambda tj=tj: outproj_tile(tj, act_evac=True)
                        for tj in list(range(4 * (last - 1), 4 * last)) +
                        list(range(4 * last, 4 * last + 4))]
                attn_column(p, ch, it, tailjobs)
    nc.compile()
    return nc


class _Runner:
    def __init__(self, nc):
        import jax
        from jax.sharding import Mesh, PartitionSpec, NamedSharding
        from jax.experimental.shard_map import shard_map
        import concourse.mybir as mybir
        from concourse.bass2jax import (_bass_exec_p, partition_id_tensor,
                                        install_neuronx_cc_hook)
        install_neuronx_cc_hook()
        self.jax = jax
        part = nc.partition_id_tensor.name if nc.partition_id_tensor else None
        in_names, out_names, out_avals = [], [], []
        for alloc in nc.m.functions[0].allocations:
            if not isinstance(alloc, mybir.MemoryLocationSet):
                continue
            name = alloc.memorylocations[0].name
            if alloc.kind == "ExternalInput":
                if name != part:
                    in_names.append(name)
            elif alloc.kind == "ExternalOutput":
                out_names.append(name)
                out_avals.append(jax.core.ShapedArray(
                    tuple(alloc.tensor_shape), mybir.dt.np(alloc.dtype)))
        self.in_names, self.out_names, self.out_avals = in_names, out_names, out_avals
        all_in = list(in_names) + list(out_names) + ([part] if part else [])

        def _body(*args):
            ops = list(args)
            if part:
                ops.append(partition_id_tensor())
            return tuple(_bass_exec_p.bind(
                *ops, out_avals=tuple(out_avals), in_names=tuple(all_in),
                out_names=tuple(out_names), lowering_input_output_aliases=(),
                sim_require_finite=True, sim_require_nnan=True, nc=nc))

        devices = jax.devices()[:NCORES]
        mesh = Mesh(np.asarray(devices), ("core",))
        nin = len(in_names) + len(out_names)
        self.fn = jax.jit(
            shard_map(_body, mesh=mesh,
                      in_specs=(PartitionSpec("core"),) * nin,
                      out_specs=(PartitionSpec("core"),) * len(out_names),
                      check_rep=False),
            keep_unused=True)
        self.sharding = NamedSharding(mesh, PartitionSpec("core"))

    def put_inputs(self, in_maps):
        args = []
        for name in self.in_names:
            cat = np.concatenate([np.asarray(m[name]) for m in in_maps], axis=0)
            args.append(self.jax.device_put(cat, self.sharding))
        for av in self.out_avals:
            z = np.zeros((NCORES * av.shape[0], *av.shape[1:]), av.dtype)
            args.append(self.jax.device_put(z, self.sharding))
        return args

    def run_np(self, args):
        outs = self.fn(*args)
        return [
            {n: np.asarray(outs[i]).reshape(NCORES, *self.out_avals[i].shape)[c]
             for i, n in enumerate(self.out_names)}
            for c in range(NCORES)
        ]


def _get_runner():
    if "r" not in _CACHE:
        nc = _build()
        _CACHE["nc"] = nc
        _CACHE["r"] = _Runner(nc)
    return _CACHE["r"]


def _rne11(a):
    """Round fp32 to 11 mantissa bits, round-to-nearest-even (= hw fp32r)."""
    ai = np.ascontiguousarray(a, dtype=np.float32).view(np.uint32).astype(np.uint64)
    lsb = (ai >> 12) & 1
    out = (((ai + 2047 + lsb) >> 12) << 12).astype(np.uint32)
    return out.view(np.float32)


def _q8(a):
    import ml_dtypes
    a = np.clip(np.asarray(a, np.float32), -240.0, 240.0)
    return a.astype(ml_dtypes.float8_e4m3)


def _pack_bias(b):
    # [512] -> [128, NPAIR] with the (j, h, s) psum-partition order,
    # pre-scaled by the q/k fp8 storage scale
    br = (b * QKS).reshape(NPAIR, 2, 2, 32).transpose(2, 1, 3, 0)
    return np.ascontiguousarray(br.reshape(128, NPAIR).astype(np.float32))


def make_in_maps(x, Wqkv, bqkv, Wo, bo=None, mask=None):
    # x repack: [ch, part, g, i, tok], feature = 256g + 128i + part
    xhs, xls = [], []
    for b in range(B):
        xr = np.ascontiguousarray(
            x[b].reshape(NCH, 512, NG, 2, 128).transpose(0, 4, 2, 3, 1))
        h8 = _q8(xr)
        l8 = _q8(xr - h8.astype(np.float32))
        xhs.append(h8.reshape(NCH, 128, NG * 1024))
        xls.append(l8.reshape(NCH, 128, NG * 1024))

    in_maps = []
    for c in range(NCORES):
        b, g = c // 2, c % 2
        sl = slice(g * FPC, (g + 1) * FPC)

        def pack_qk(w):
            # [d, col] -> [part, p, g, i, f]: d = 256g+128i+part, col = 128p+f
            # and f (psum partition) reordered to (j, h, s):
            # feature-within-pair = 64h + 32j + s  ->  f = 64j + 32h + s
            wr = (w * WSC).reshape(NG, 2, 128, NPAIR, 2, 2, 32).transpose(
                2, 3, 0, 1, 5, 4, 6)
            hi = _q8(wr)
            lo = _q8(wr - hi.astype(np.float32))
            return (hi.reshape(128, 4096), lo.reshape(128, 4096))

        def pack_v(w):
            # [d, col] -> [part, g, i, f]: d = 256g+128i+part, col = f(512)
            wr = (w * WSC).reshape(NG, 2, 128, FPC).transpose(2, 0, 1, 3)
            hi = _q8(wr)
            lo = _q8(wr - hi.astype(np.float32))
            return (hi.reshape(128, 4096), lo.reshape(128, 4096))

        qh, ql = pack_qk(Wqkv[:, 0 * D:1 * D][:, sl])
        kh, kl = pack_qk(Wqkv[:, 1 * D:2 * D][:, sl])
        vh, vl = pack_v(Wqkv[:, 2 * D:3 * D][:, sl])

        in_maps.append({
            "xh": xhs[b], "xl": xls[b],
            "Wqh": qh, "Wql": ql, "Wkh": kh, "Wkl": kl, "Wvh": vh, "Wvl": vl,
            "BQ": _pack_bias(bqkv[0 * D:1 * D][sl]),
            "BK": _pack_bias(bqkv[1 * D:2 * D][sl]),
            "BV": np.ascontiguousarray(bqkv[2 * D:3 * D][sl]),
            "Wo": _rne11(Wo[sl, :]),
        })
    return in_maps


def kernel(x, Wqkv, bqkv, Wo, bo, mask=None, **_unused):
    x = np.asarray(x, dtype=np.float32)
    Wqkv = np.asarray(Wqkv, dtype=np.float32)
    bqkv = np.asarray(bqkv, dtype=np.float32)
    Wo = np.asarray(Wo, dtype=np.float32)
    bo = np.asarray(bo, dtype=np.float32)
    in_maps = make_in_maps(x, Wqkv, bqkv, Wo)
    last_err = None
    for _attempt in range(3):
        try:
            r = _get_runner()
            args = r.put_inputs(in_maps)
            res = r.run_np(args)
            break
        except Exception as e:  # transient device wedge: retry fresh
            last_err = e
            _CACHE.clear()
            import time
            time.sleep(5)
    else:
        raise last_err
    out = np.empty((B, T, D), dtype=np.float32)
    for b in range(B):
        out[b] = res[2 * b]["y"] + res[2 * b + 1]["y"] + bo
    return out


# revision 9
# speedup vs baseline: 1.2453x; 1.0029x over previous
"""Causal self-attention Trainium2 kernel, v2.

Sharding: 8 cores = 4 batches x 2 head-groups (8 heads each).

Per-core dataflow:
  - QKV projections as fp8e4 DoubleRow matmuls (256-feature contraction
    per instruction) with a hi/lo split of both x and W (host-prepared):
    q = xh@Wh + xl@Wh + xh@Wl  -- 3 DoubleRow passes = 6N cycles vs
    fp32r's 8N, with ~0.1% error.
  - q,k stored bf16 (rate-1 matmuls at any N, so causal diagonal blocks
    need no 256-col widening); scores per k-tile into PSUM.
  - exp on ACT writes P^T directly as bf16; causal triangle zeroed on
    GPSIMD (affine_select); PV matmuls in bf16 with a ones-column in the
    V tile accumulating softmax denominators.
  - out = PV / rowsum via DVE reciprocal + GPSIMD partition broadcast.
  - y = outT.T @ Wo in fp32r (partial; host sums the 2 head-groups).

Scheduling: projection chains for chunk ch+1 and output-projection tiles
for chunk ch-1 are interleaved between attention k-tiles of chunk ch so
the PE never idles while ACT paces the softmax.
"""
import numpy as np

B, T, D, H = 4, 2048, 1024, 16
HD = D // H            # 64
NCORES = 8
HPC = 8                # heads per core
FPC = HPC * HD         # 512 feature cols per core
NPAIR = HPC // 2       # 4 head pairs
NG = 4                 # fp8 DoubleRow contraction groups (256 feats each)
KT = T // 128          # 16 k-tiles
NCH = T // 512         # 4 q-chunks
WSC = 50.0             # host weight scaling before fp8 quantization
QKS = 16.0             # q/k fp8 storage scale
VSTR = 65              # per-k-tile stride in vaug free dim
HSTR = KT * VSTR       # per-head stride in vaug free dim

_CACHE = {}


def _build():
    import concourse.mybir as mybir
    import concourse.tile as tile
    from concourse import bacc
    from contextlib import ExitStack

    f32 = mybir.dt.float32
    f32r = mybir.dt.float32r
    bf16 = mybir.dt.bfloat16
    f8 = mybir.dt.float8e4
    DR = mybir.MatmulPerfMode.DoubleRow
    Exp = mybir.ActivationFunctionType.Exp
    Alu = mybir.AluOpType

    nc = bacc.Bacc("TRN2", target_bir_lowering=False, debug=False,
                   num_devices=NCORES)
    # x hi/lo fp8, repacked host-side as [ch, 128, g, i, tok]:
    # feature = 256*g + 128*i + partition
    xh = nc.dram_tensor("xh", [NCH, 128, NG * 1024], f8, kind="ExternalInput")
    xl = nc.dram_tensor("xl", [NCH, 128, NG * 1024], f8, kind="ExternalInput")
    # fp8 weights, 6 tensors: q/k: [p][g][i][f128]; v: [g][i][f512]
    Wsec = [nc.dram_tensor(nm, [128, 4096], f8, kind="ExternalInput")
            for nm in ("Wqh", "Wql", "Wkh", "Wkl", "Wvh", "Wvl")]
    BQ = nc.dram_tensor("BQ", [128, NPAIR], f32, kind="ExternalInput")
    BK = nc.dram_tensor("BK", [128, NPAIR], f32, kind="ExternalInput")
    BV = nc.dram_tensor("BV", [FPC], f32, kind="ExternalInput")
    Wo = nc.dram_tensor("Wo", [FPC, D], f32r, kind="ExternalInput")
    y = nc.dram_tensor("y", [T, D], f32, kind="ExternalOutput")

    with tile.TileContext(nc) as tc, ExitStack() as es:
        pers = es.enter_context(tc.tile_pool(name="pers", bufs=1))
        xsp = es.enter_context(tc.tile_pool(name="xsp", bufs=2))
        ptp = es.enter_context(tc.tile_pool(name="ptp", bufs=8))
        nrm = es.enter_context(tc.tile_pool(name="nrm", bufs=2))
        obp = es.enter_context(tc.tile_pool(name="obp", bufs=3))
        stgp = es.enter_context(tc.tile_pool(name="stgp", bufs=4))
        psA = es.enter_context(tc.tile_pool(name="psA", bufs=2, space="PSUM"))
        psS = es.enter_context(tc.tile_pool(name="psS", bufs=2, space="PSUM"))
        psO = es.enter_context(tc.tile_pool(name="psO", bufs=1, space="PSUM"))

        ws_sb = [pers.tile([128, 4096], f8, tag=f"ws{i}", name=f"ws{i}")
                 for i in range(6)]
        bq_sb = pers.tile([128, NPAIR], f32, tag="bq")
        bk_sb = pers.tile([128, NPAIR], f32, tag="bk")
        bv_row = pers.tile([1, FPC], f32, tag="bvr")
        bv_bc = pers.tile([128, FPC], f32, tag="bvb")
        wo_sb = pers.tile([128, NPAIR * D], f32r, tag="wo")
        vaug = pers.tile([128, HPC * HSTR], bf16, tag="vaug")
        # q/k in fp8 for DoubleRow scores: tile u holds pairs (2u, 2u+1);
        # partition = 64*(pr%2) + 32*head + hd%32, free = (hd-half j,
        # q-or-k, token) -- one tile so each remap DMA moves q and k
        qkT8 = [pers.tile([128, 2, 2, T], f8, tag=f"qkT8{u}",
                          name=f"qkT8{u}") for u in range(2)]
        outT = [pers.tile([128, T], f32r, tag=f"oT{p}", name=f"oT{p}")
                for p in range(NPAIR)]

        vaug4 = vaug[:].rearrange("p (h k x) -> p h k x", h=HPC, k=KT)

        xtiles = {}

        def emit_xdma(ch):
            xh_sb = xsp.tile([128, NG * 1024], f8, tag="xh", name="xh_sb")
            xl_sb = xsp.tile([128, NG * 1024], f8, tag="xl", name="xl_sb")
            if ch == 0:
                half = NG * 512
                nc.sync.dma_start(xh_sb[:, 0:half], xh[ch][:, 0:half])
                nc.sync.dma_start(xl_sb[:, 0:half], xl[ch][:, 0:half])
                nc.sync.dma_start(xh_sb[:, half:], xh[ch][:, half:])
                nc.sync.dma_start(xl_sb[:, half:], xl[ch][:, half:])
            else:
                nc.sync.dma_start(xh_sb[:], xh[ch])
                nc.sync.dma_start(xl_sb[:], xl[ch])
            xtiles[ch] = [xx[:, g * 1024:(g + 1) * 1024].rearrange(
                "p (i t) -> p i t", i=2)
                for xx in (xh_sb, xl_sb) for g in range(NG)]

        def w_ap(sec, p, g):
            base = p * 1024 + g * 256
            return ws_sb[sec][:, base:base + 256].rearrange(
                "p (i f) -> p i f", i=2)

        def wv_ap(sec, g):
            base = g * 1024
            return ws_sb[sec][:, base:base + 1024].rearrange(
                "p (i f) -> p i f", i=2)

        # (x-part, w-section) term order: xl last so its DMA can trail
        # at startup. psum partitions are ordered (hd-half j, head, hd%32)
        # by the host weight packing; both evacs write scaled fp8 into one
        # staging tile whose two j-halves are then DMA'd into the
        # partition-sliced qkT8 layout (one DMA moves q and k together).
        def proj_qk(ch, p):
            xg = xtiles[ch]
            stg = stgp.tile([128, 2, 512], f8, tag="stg", name="stg")
            for qk, (sec, bsb) in enumerate(((0, bq_sb), (2, bk_sb))):
                ps = psA.tile([128, 512], f32, tag="psA", name="psqk")
                n = 0
                for (xt, ws) in ((0, sec), (0, sec + 1), (1, sec)):
                    for g in range(NG):
                        nc.tensor.matmul(
                            ps[:], w_ap(ws, p, g), xg[xt * NG + g],
                            start=(n == 0), stop=(n == 3 * NG - 1),
                            perf_mode=DR)
                        n += 1
                nc.vector.tensor_scalar(
                    stg[:, qk, :], ps[:], QKS / WSC, bsb[:, p:p + 1],
                    Alu.mult, Alu.add)
            u, e = p // 2, p % 2
            for j in range(2):
                nc.sync.dma_start(
                    qkT8[u][64 * e:64 * e + 64, j, :,
                            512 * ch:512 * (ch + 1)],
                    stg[64 * j:64 * j + 64, :, :])

        def proj_v(ch, tj):
            xg = xtiles[ch]
            lo = (tj - 4 * ch) * 128
            ps = psA.tile([128, FPC], f32, tag="psA", name="psv")
            n = 0
            for (xt, ws) in ((0, 4), (0, 5), (1, 4)):
                for g in range(NG):
                    nc.tensor.matmul(
                        ps[:], xg[xt * NG + g][:, :, lo:lo + 128],
                        wv_ap(ws, g),
                        start=(n == 0), stop=(n == 3 * NG - 1),
                        perf_mode=DR)
                    n += 1
            nc.vector.scalar_tensor_tensor(
                vaug4[:, :, tj, 0:64],
                ps[:].rearrange("p (h x) -> p h x", h=HPC),
                1.0 / WSC,
                bv_bc[:].rearrange("p (h x) -> p h x", h=HPC),
                Alu.mult, Alu.add)

        def proj_chunk_fill(ch):
            out = []
            for p in range(NPAIR):
                out.append(lambda p=p: proj_qk(ch, p))
            for tj in range(4 * ch, 4 * (ch + 1)):
                out.append(lambda tj=tj: proj_v(ch, tj))
            return out

        Copy = mybir.ActivationFunctionType.Copy

        def outproj_tile(tj, act_evac=False):
            ob = obp.tile([128, D], f32, tag="ob", name="ob")
            for n in range(2):
                ps = psA.tile([128, 512], f32, tag="psA", name="pspr")
                for p in range(NPAIR):
                    nc.tensor.matmul(
                        ps[:],
                        outT[p][:, 128 * tj:128 * (tj + 1)],
                        wo_sb[:, p * D + 512 * n:p * D + 512 * (n + 1)],
                        start=(p == 0), stop=(p == NPAIR - 1))
                if act_evac:
                    # ACT evac + per-half writeback shortens the final
                    # drain (keeps the tail off the DVE queue too)
                    nc.scalar.activation(ob[:, 512 * n:512 * (n + 1)],
                                         ps[:], Copy)
                    nc.sync.dma_start(
                        y[128 * tj:128 * (tj + 1),
                          512 * n:512 * (n + 1)],
                        ob[:, 512 * n:512 * (n + 1)])
                else:
                    nc.vector.tensor_copy(ob[:, 512 * n:512 * (n + 1)],
                                          ps[:])
            if not act_evac:
                nc.sync.dma_start(y[128 * tj:128 * (tj + 1), :], ob[:])

        def emit_norm(p, ch, po, c0, c1):
            w = c1 - c0
            for h in range(2):
                og = nrm.tile([65, 512], f32, tag="og", name="og")
                nc.vector.tensor_copy(og[:, 0:w], po[h][:, c0:c1])
                rec = nrm.tile([1, 512], f32, tag="rec", name="rec")
                nc.vector.reciprocal(rec[:, 0:w], og[64:65, 0:w])
                bc = nrm.tile([64, 512], f32, tag="bc", name="bc")
                nc.gpsimd.partition_broadcast(bc[:, 0:w], rec[:, 0:w])
                nc.vector.tensor_mul(
                    outT[p][64 * h:64 * (h + 1),
                            512 * ch + c0:512 * ch + c1],
                    og[0:64, 0:w], bc[:, 0:w])

        def attn_column(p, ch, fill, tailjobs=None):
            """fill: iterator of emitters to interleave between k-tiles."""
            po = [psO.tile([65, 512], f32, tag=f"po{h}", name=f"po{h}")
                  for h in range(2)]
            nk = 4 * ch + 4

            def emit_pv(kt, pt):
                qo = max(kt - 4 * ch, 0) * 128
                for h in range(2):
                    nc.tensor.matmul(
                        po[h][:, qo:],
                        vaug4[:, 2 * p + h, kt, :],
                        pt[:, h, qo:],
                        start=(kt == 0), stop=(kt == nk - 1),
                        skip_group_check=True)

            pend = []
            for kt in range(nk):
                dg = kt - 4 * ch
                qo = max(dg, 0) * 128
                pt = ptp.tile([128, 2, 512], bf16, tag="pt", name="pt")
                ss = psS.tile([128, 2, 512], f32, tag="psS", name="ss")
                u, e = p // 2, p % 2
                for h in range(2):
                    b0 = 64 * e + 32 * h
                    nc.tensor.matmul(
                        ss[:, h, qo:],
                        qkT8[u][b0:b0 + 32, :, 1,
                                128 * kt:128 * (kt + 1)],
                        qkT8[u][b0:b0 + 32, :, 0,
                                512 * ch + qo:512 * (ch + 1)],
                        start=True, stop=True, perf_mode=DR,
                        tile_position=(b0, 0))
                nc.scalar.activation(pt[:, :, qo:], ss[:, :, qo:],
                                     Exp, scale=0.125 / (QKS * QKS))
                if dg >= 0:
                    # zero the upper (q < k) triangle of the diagonal block
                    blk = pt[:, :, qo:qo + 128]
                    nc.gpsimd.affine_select(
                        out=blk, in_=blk,
                        compare_op=mybir.AluOpType.is_ge,
                        fill=0.0, base=0,
                        pattern=[[0, 2], [1, 128]],
                        channel_multiplier=-1)
                # fill, then 2-tile-deep software-pipelined PV so the
                # PE never waits inline on an exp
                f = next(fill, None)
                if f:
                    f()
                pend.append((kt, pt))
                if len(pend) > 4:
                    emit_pv(*pend.pop(0))
            if tailjobs is None:
                for pv in pend:
                    f = next(fill, None)
                    if f:
                        f()
                    emit_pv(*pv)
                emit_norm(p, ch, po, 0, 512)
            else:
                # last column: po[:, 0:128*q] is complete as soon as the
                # q-th diagonal PV lands -- normalize and project each
                # quarter as it completes, with reserved chunk-2 output
                # tiles filling each normalize chain's latency
                while len(pend) > 2:
                    emit_pv(*pend.pop(0))
                emit_norm(p, ch, po, 0, 128)
                tailjobs[0]()
                tailjobs[4]()
                emit_norm(p, ch, po, 128, 256)
                tailjobs[1]()
                emit_pv(*pend.pop(0))
                tailjobs[5]()
                emit_pv(*pend.pop(0))
                emit_norm(p, ch, po, 256, 384)
                emit_norm(p, ch, po, 384, 512)
                tailjobs[2]()
                tailjobs[6]()
                tailjobs[3]()
                tailjobs[7]()

        # ---------------- schedule ----------------
        # startup DMA order: q weights and x first (in first-use order,
        # split so the first projection chain can start ~4us in)
        H2 = 2048
        nc.sync.dma_start(ws_sb[0][:, 0:H2], Wsec[0][:, 0:H2])
        emit_xdma(0)
        nc.sync.dma_start(ws_sb[1][:, 0:H2], Wsec[1][:, 0:H2])
        nc.sync.dma_start(bq_sb[:], BQ[:])
        nc.sync.dma_start(ws_sb[2][:, 0:H2], Wsec[2][:, 0:H2])
        nc.sync.dma_start(ws_sb[3][:, 0:H2], Wsec[3][:, 0:H2])
        nc.sync.dma_start(bk_sb[:], BK[:])
        nc.sync.dma_start(bv_row[:], BV[:])
        nc.sync.dma_start(ws_sb[0][:, H2:], Wsec[0][:, H2:])
        nc.sync.dma_start(ws_sb[1][:, H2:], Wsec[1][:, H2:])
        nc.sync.dma_start(ws_sb[2][:, H2:], Wsec[2][:, H2:])
        nc.sync.dma_start(ws_sb[3][:, H2:], Wsec[3][:, H2:])
        nc.gpsimd.partition_broadcast(bv_bc[:], bv_row[:])
        nc.vector.memset(vaug4[:, :, :, 64:65], 1.0)

        fill0 = proj_chunk_fill(0)
        # qk p0-1, V weights, qk p2-3, x prefetch for chunk 1, V chains
        for f in fill0[:2]:
            f()
        nc.sync.dma_start(ws_sb[4][:], Wsec[4][:])
        nc.sync.dma_start(ws_sb[5][:], Wsec[5][:])
        for f in fill0[2:4]:
            f()
        emit_xdma(1)
        for f in fill0[4:]:
            f()

        # ---- deadline-packed fill assignment over the k-tile slots ----
        slots = []           # (ch, p, kt) in emission order
        slot_of = {}
        for ch in range(NCH):
            for p in range(NPAIR):
                for kt in range(4 * ch + 4):
                    slot_of[(ch, p, kt)] = len(slots)
                    slots.append((ch, p, kt))
        nslots = len(slots)
        chunk_start = {ch: slot_of[(ch, 0, 0)] for ch in range(NCH)}
        chunk_end = {ch: slot_of[(ch, NPAIR - 1, 4 * ch + 3)]
                     for ch in range(NCH)}

        items = []  # (deadline, avail, fn)
        for ch in range(1, NCH):
            av = chunk_start[ch - 1]
            for p in range(NPAIR):
                dl = slot_of[(ch, p, 0)] - 4
                items.append((dl, av, lambda ch=ch, p=p: proj_qk(ch, p)))
            for tj in range(4 * ch, 4 * (ch + 1)):
                dl = slot_of[(ch, 0, tj)] - 2
                items.append((dl, av, lambda ch=ch, tj=tj: proj_v(ch, tj)))
        for tj in range(4 * (NCH - 2)):
            items.append((nslots - 1, chunk_end[tj // 4] + 1,
                          lambda tj=tj: outproj_tile(tj)))
        items.append((chunk_end[0] + 8, 0, lambda: nc.sync.dma_start(
            wo_sb[:].rearrange("p (t c) -> p t c", t=NPAIR),
            Wo[:].rearrange("(t p) c -> p t c", p=128))))

        assigned = {}
        for dl, av, fn in sorted(items, key=lambda it: -it[0]):
            s = min(dl, nslots - 1)
            while s >= av and s in assigned:
                s -= 1
            if s < av:
                s = av
                while s in assigned:
                    s += 1
            assigned[s] = fn

        fills = {}
        for s, fn in assigned.items():
            fills.setdefault(s, []).append(fn)

        def _slotfill(seq):
            for fl in seq:
                yield fl[0] if fl else None
            while True:
                yield None

        last = NCH - 1
        for ch in range(NCH):
            if 0 < ch and ch + 1 < NCH:
                emit_xdma(ch + 1)
            for p in range(NPAIR):
                nk = 4 * ch + 4
                seq = []
                for kt in range(nk):
                    fl = fills.get(slot_of[(ch, p, kt)], [])
                    seq.append(fl)
                it = _slotfill(seq)
                tailjobs = None
                if ch == last and p == NPAIR - 1:
                    tailjobs = [
                        lambda tj=tj: outproj_tile(tj, act_evac=True)
                        for tj in list(range(4 * (last - 1), 4 * last)) +
                        list(range(4 * last, 4 * last + 4))]
                attn_column(p, ch, it, tailjobs)
    nc.compile()
    return nc


class _Runner:
    def __init__(self, nc):
        import jax
        from jax.sharding import Mesh, PartitionSpec, NamedSharding
        from jax.experimental.shard_map import shard_map
        import concourse.mybir as mybir
        from concourse.bass2jax import (_bass_exec_p, partition_id_tensor,
                                        install_neuronx_cc_hook)
        install_neuronx_cc_hook()
        self.jax = jax
        part = nc.partition_id_tensor.name if nc.partition_id_tensor else None
        in_names, out_names, out_avals = [], [], []
        for alloc in nc.m.functions[0].allocations:
            if not isinstance(alloc, mybir.MemoryLocationSet):
                continue
            name = alloc.memorylocations[0].name
            if alloc.kind == "ExternalInput":
                if name != part:
                    in_names.append(name)
            elif alloc.kind == "ExternalOutput":
                out_names.append(name)
                out_avals.append(jax.core.ShapedArray(
                    tuple(alloc.tensor_shape), mybir.dt.np(alloc.dtype)))
        self.in_names, self.out_names, self.out_avals = in_names, out_names, out_avals
        all_in = list(in_names) + list(out_names) + ([part] if part else [])

        def _body(*args):
            ops = list(args)
            if part:
                ops.append(partition_id_tensor())
            return tuple(_bass_exec_p.bind(
                *ops, out_avals=tuple(out_avals), in_names=tuple(all_in),
                out_names=tuple(out_names), lowering_input_output_aliases=(),
                sim_require_finite=True, sim_require_nnan=True, nc=nc))

        devices = jax.devices()[:NCORES]
        mesh = Mesh(np.asarray(devices), ("core",))
        nin = len(in_names) + len(out_names)
        self.fn = jax.jit(
            shard_map(_body, mesh=mesh,
                      in_specs=(PartitionSpec("core"),) * nin,
                      out_specs=(PartitionSpec("core"),) * len(out_names),
                      check_rep=False),
            keep_unused=True)
        self.sharding = NamedSharding(mesh, PartitionSpec("core"))

    def put_inputs(self, in_maps):
        args = []
        for name in self.in_names:
            cat = np.concatenate([np.asarray(m[name]) for m in in_maps], axis=0)
            args.append(self.jax.device_put(cat, self.sharding))
        for av in self.out_avals:
            z = np.zeros((NCORES * av.shape[0], *av.shape[1:]), av.dtype)
            args.append(self.jax.device_put(z, self.sharding))
        return args

    def run_np(self, args):
        outs = self.fn(*args)
        return [
            {n: np.asarray(outs[i]).reshape(NCORES, *self.out_avals[i].shape)[c]
             for i, n in enumerate(self.out_names)}
            for c in range(NCORES)
        ]


def _get_runner():
    if "r" not in _CACHE:
        nc = _build()
        _CACHE["nc"] = nc
        _CACHE["r"] = _Runner(nc)
    return _CACHE["r"]


def _rne11(a):
    """Round fp32 to 11 mantissa bits, round-to-nearest-even (= hw fp32r)."""
    ai = np.ascontiguousarray(a, dtype=np.float32).view(np.uint32).astype(np.uint64)
    lsb = (ai >> 12) & 1
    out = (((ai + 2047 + lsb) >> 12) << 12).astype(np.uint32)
    return out.view(np.float32)


def _q8(a):
    import ml_dtypes
    a = np.clip(np.asarray(a, np.float32), -240.0, 240.0)
    return a.astype(ml_dtypes.float8_e4m3)


def _pack_bias(b):
    # [512] -> [128, NPAIR] with the (j, h, s) psum-partition order,
    # pre-scaled by the q/k fp8 storage scale
    br = (b * QKS).reshape(NPAIR, 2, 2, 32).transpose(2, 1, 3, 0)
    return np.ascontiguousarray(br.reshape(128, NPAIR).astype(np.float32))


def make_in_maps(x, Wqkv, bqkv, Wo, bo=None, mask=None):
    # x repack: [ch, part, g, i, tok], feature = 256g + 128i + part
    xhs, xls = [], []
    for b in range(B):
        xr = np.ascontiguousarray(
            x[b].reshape(NCH, 512, NG, 2, 128).transpose(0, 4, 2, 3, 1))
        h8 = _q8(xr)
        l8 = _q8(xr - h8.astype(np.float32))
        xhs.append(h8.reshape(NCH, 128, NG * 1024))
        xls.append(l8.reshape(NCH, 128, NG * 1024))

    in_maps = []
    for c in range(NCORES):
        b, g = c // 2, c % 2
        sl = slice(g * FPC, (g + 1) * FPC)

        def pack_qk(w):
            # [d, col] -> [part, p, g, i, f]: d = 256g+128i+part, col = 128p+f
            # and f (psum partition) reordered to (j, h, s):
            # feature-within-pair = 64h + 32j + s  ->  f = 64j + 32h + s
            wr = (w * WSC).reshape(NG, 2, 128, NPAIR, 2, 2, 32).transpose(
                2, 3, 0, 1, 5, 4, 6)
            hi = _q8(wr)
            lo = _q8(wr - hi.astype(np.float32))
            return (hi.reshape(128, 4096), lo.reshape(128, 4096))

        def pack_v(w):
            # [d, col] -> [part, g, i, f]: d = 256g+128i+part, col = f(512)
            wr = (w * WSC).reshape(NG, 2, 128, FPC).transpose(2, 0, 1, 3)
            hi = _q8(wr)
            lo = _q8(wr - hi.astype(np.float32))
            return (hi.reshape(128, 4096), lo.reshape(128, 4096))

        qh, ql = pack_qk(Wqkv[:, 0 * D:1 * D][:, sl])
        kh, kl = pack_qk(Wqkv[:, 1 * D:2 * D][:, sl])
        vh, vl = pack_v(Wqkv[:, 2 * D:3 * D][:, sl])

        in_maps.append({
            "xh": xhs[b], "xl": xls[b],
            "Wqh": qh, "Wql": ql, "Wkh": kh, "Wkl": kl, "Wvh": vh, "Wvl": vl,
            "BQ": _pack_bias(bqkv[0 * D:1 * D][sl]),
            "BK": _pack_bias(bqkv[1 * D:2 * D][sl]),
            "BV": np.ascontiguousarray(bqkv[2 * D:3 * D][sl]),
            "Wo": _rne11(Wo[sl, :]),
        })
    return in_maps


def kernel(x, Wqkv, bqkv, Wo, bo, mask=None, **_unused):
    x = np.asarray(x, dtype=np.float32)
    Wqkv = np.asarray(Wqkv, dtype=np.float32)
    bqkv = np.asarray(bqkv, dtype=np.float32)
    Wo = np.asarray(Wo, dtype=np.float32)
    bo = np.asarray(bo, dtype=np.float32)
    in_maps = make_in_maps(x, Wqkv, bqkv, Wo)
    last_err = None
    for _attempt in range(3):
        try:
            r = _get_runner()
            args = r.put_inputs(in_maps)
            res = r.run_np(args)
            break
        except Exception as e:  # transient device wedge: retry fresh
            last_err = e
            _CACHE.clear()
            import time
            time.sleep(5)
    else:
        raise last_err
    out = np.empty((B, T, D), dtype=np.float32)
    for b in range(B):
        out[b] = res[2 * b]["y"] + res[2 * b + 1]["y"] + bo
    return out


# revision 10
# speedup vs baseline: 1.2466x; 1.0010x over previous
"""Causal self-attention Trainium2 kernel, v2.

Sharding: 8 cores = 4 batches x 2 head-groups (8 heads each).

Per-core dataflow:
  - QKV projections as fp8e4 DoubleRow matmuls (256-feature contraction
    per instruction) with a hi/lo split of both x and W (host-prepared):
    q = xh@Wh + xl@Wh + xh@Wl  -- 3 DoubleRow passes = 6N cycles vs
    fp32r's 8N, with ~0.1% error.
  - q,k stored bf16 (rate-1 matmuls at any N, so causal diagonal blocks
    need no 256-col widening); scores per k-tile into PSUM.
  - exp on ACT writes P^T directly as bf16; causal triangle zeroed on
    GPSIMD (affine_select); PV matmuls in bf16 with a ones-column in the
    V tile accumulating softmax denominators.
  - out = PV / rowsum via DVE reciprocal + GPSIMD partition broadcast.
  - y = outT.T @ Wo in fp32r (partial; host sums the 2 head-groups).

Scheduling: projection chains for chunk ch+1 and output-projection tiles
for chunk ch-1 are interleaved between attention k-tiles of chunk ch so
the PE never idles while ACT paces the softmax.
"""
import numpy as np

B, T, D, H = 4, 2048, 1024, 16
HD = D // H            # 64
NCORES = 8
HPC = 8                # heads per core
FPC = HPC * HD         # 512 feature cols per core
NPAIR = HPC // 2       # 4 head pairs
NG = 4                 # fp8 DoubleRow contraction groups (256 feats each)
KT = T // 128          # 16 k-tiles
NCH = T // 512         # 4 q-chunks
WSC = 50.0             # host weight scaling before fp8 quantization
QKS = 16.0             # q/k fp8 storage scale
VSTR = 65              # per-k-tile stride in vaug free dim
HSTR = KT * VSTR       # per-head stride in vaug free dim

_CACHE = {}


def _build():
    import concourse.mybir as mybir
    import concourse.tile as tile
    from concourse import bacc
    from contextlib import ExitStack

    f32 = mybir.dt.float32
    f32r = mybir.dt.float32r
    bf16 = mybir.dt.bfloat16
    f8 = mybir.dt.float8e4
    DR = mybir.MatmulPerfMode.DoubleRow
    Exp = mybir.ActivationFunctionType.Exp
    Alu = mybir.AluOpType

    nc = bacc.Bacc("TRN2", target_bir_lowering=False, debug=False,
                   num_devices=NCORES)
    # x hi/lo fp8, repacked host-side as [ch, 128, g, i, tok]:
    # feature = 256*g + 128*i + partition
    xh = nc.dram_tensor("xh", [NCH, 128, NG * 1024], f8, kind="ExternalInput")
    xl = nc.dram_tensor("xl", [NCH, 128, NG * 1024], f8, kind="ExternalInput")
    # fp8 weights, 6 tensors: q/k: [p][g][i][f128]; v: [g][i][f512]
    Wsec = [nc.dram_tensor(nm, [128, 4096], f8, kind="ExternalInput")
            for nm in ("Wqh", "Wql", "Wkh", "Wkl", "Wvh", "Wvl")]
    BQ = nc.dram_tensor("BQ", [128, NPAIR], f32, kind="ExternalInput")
    BK = nc.dram_tensor("BK", [128, NPAIR], f32, kind="ExternalInput")
    BV = nc.dram_tensor("BV", [FPC], f32, kind="ExternalInput")
    Wo = nc.dram_tensor("Wo", [FPC, D], f32r, kind="ExternalInput")
    y = nc.dram_tensor("y", [T, D], f32, kind="ExternalOutput")

    with tile.TileContext(nc) as tc, ExitStack() as es:
        pers = es.enter_context(tc.tile_pool(name="pers", bufs=1))
        xsp = es.enter_context(tc.tile_pool(name="xsp", bufs=2))
        ptp = es.enter_context(tc.tile_pool(name="ptp", bufs=10))
        nrm = es.enter_context(tc.tile_pool(name="nrm", bufs=2))
        obp = es.enter_context(tc.tile_pool(name="obp", bufs=3))
        stgp = es.enter_context(tc.tile_pool(name="stgp", bufs=4))
        psA = es.enter_context(tc.tile_pool(name="psA", bufs=2, space="PSUM"))
        psS = es.enter_context(tc.tile_pool(name="psS", bufs=2, space="PSUM"))
        psO = es.enter_context(tc.tile_pool(name="psO", bufs=1, space="PSUM"))

        ws_sb = [pers.tile([128, 4096], f8, tag=f"ws{i}", name=f"ws{i}")
                 for i in range(6)]
        bq_sb = pers.tile([128, NPAIR], f32, tag="bq")
        bk_sb = pers.tile([128, NPAIR], f32, tag="bk")
        bv_row = pers.tile([1, FPC], f32, tag="bvr")
        bv_bc = pers.tile([128, FPC], f32, tag="bvb")
        wo_sb = pers.tile([128, NPAIR * D], f32r, tag="wo")
        vaug = pers.tile([128, HPC * HSTR], bf16, tag="vaug")
        # q/k in fp8 for DoubleRow scores: tile u holds pairs (2u, 2u+1);
        # partition = 64*(pr%2) + 32*head + hd%32, free = (hd-half j,
        # q-or-k, token) -- one tile so each remap DMA moves q and k
        qkT8 = [pers.tile([128, 2, 2, T], f8, tag=f"qkT8{u}",
                          name=f"qkT8{u}") for u in range(2)]
        outT = [pers.tile([128, T], f32r, tag=f"oT{p}", name=f"oT{p}")
                for p in range(NPAIR)]

        vaug4 = vaug[:].rearrange("p (h k x) -> p h k x", h=HPC, k=KT)

        xtiles = {}

        def emit_xdma(ch):
            xh_sb = xsp.tile([128, NG * 1024], f8, tag="xh", name="xh_sb")
            xl_sb = xsp.tile([128, NG * 1024], f8, tag="xl", name="xl_sb")
            if ch == 0:
                half = NG * 512
                nc.sync.dma_start(xh_sb[:, 0:half], xh[ch][:, 0:half])
                nc.sync.dma_start(xl_sb[:, 0:half], xl[ch][:, 0:half])
                nc.sync.dma_start(xh_sb[:, half:], xh[ch][:, half:])
                nc.sync.dma_start(xl_sb[:, half:], xl[ch][:, half:])
            else:
                nc.sync.dma_start(xh_sb[:], xh[ch])
                nc.sync.dma_start(xl_sb[:], xl[ch])
            xtiles[ch] = [xx[:, g * 1024:(g + 1) * 1024].rearrange(
                "p (i t) -> p i t", i=2)
                for xx in (xh_sb, xl_sb) for g in range(NG)]

        def w_ap(sec, p, g):
            base = p * 1024 + g * 256
            return ws_sb[sec][:, base:base + 256].rearrange(
                "p (i f) -> p i f", i=2)

        def wv_ap(sec, g):
            base = g * 1024
            return ws_sb[sec][:, base:base + 1024].rearrange(
                "p (i f) -> p i f", i=2)

        # (x-part, w-section) term order: xl last so its DMA can trail
        # at startup. psum partitions are ordered (hd-half j, head, hd%32)
        # by the host weight packing; both evacs write scaled fp8 into one
        # staging tile whose two j-halves are then DMA'd into the
        # partition-sliced qkT8 layout (one DMA moves q and k together).
        def proj_qk(ch, p):
            xg = xtiles[ch]
            stg = stgp.tile([128, 2, 512], f8, tag="stg", name="stg")
            for qk, (sec, bsb) in enumerate(((0, bq_sb), (2, bk_sb))):
                ps = psA.tile([128, 512], f32, tag="psA", name="psqk")
                n = 0
                for (xt, ws) in ((0, sec), (0, sec + 1), (1, sec)):
                    for g in range(NG):
                        nc.tensor.matmul(
                            ps[:], w_ap(ws, p, g), xg[xt * NG + g],
                            start=(n == 0), stop=(n == 3 * NG - 1),
                            perf_mode=DR)
                        n += 1
                nc.vector.tensor_scalar(
                    stg[:, qk, :], ps[:], QKS / WSC, bsb[:, p:p + 1],
                    Alu.mult, Alu.add)
            u, e = p // 2, p % 2
            for j in range(2):
                nc.sync.dma_start(
                    qkT8[u][64 * e:64 * e + 64, j, :,
                            512 * ch:512 * (ch + 1)],
                    stg[64 * j:64 * j + 64, :, :])

        def proj_v(ch, tj):
            xg = xtiles[ch]
            lo = (tj - 4 * ch) * 128
            ps = psA.tile([128, FPC], f32, tag="psA", name="psv")
            n = 0
            for (xt, ws) in ((0, 4), (0, 5), (1, 4)):
                for g in range(NG):
                    nc.tensor.matmul(
                        ps[:], xg[xt * NG + g][:, :, lo:lo + 128],
                        wv_ap(ws, g),
                        start=(n == 0), stop=(n == 3 * NG - 1),
                        perf_mode=DR)
                    n += 1
            nc.vector.scalar_tensor_tensor(
                vaug4[:, :, tj, 0:64],
                ps[:].rearrange("p (h x) -> p h x", h=HPC),
                1.0 / WSC,
                bv_bc[:].rearrange("p (h x) -> p h x", h=HPC),
                Alu.mult, Alu.add)

        def proj_chunk_fill(ch):
            out = []
            for p in range(NPAIR):
                out.append(lambda p=p: proj_qk(ch, p))
            for tj in range(4 * ch, 4 * (ch + 1)):
                out.append(lambda tj=tj: proj_v(ch, tj))
            return out

        Copy = mybir.ActivationFunctionType.Copy

        def outproj_tile(tj, act_evac=False):
            ob = obp.tile([128, D], f32, tag="ob", name="ob")
            for n in range(2):
                ps = psA.tile([128, 512], f32, tag="psA", name="pspr")
                for p in range(NPAIR):
                    nc.tensor.matmul(
                        ps[:],
                        outT[p][:, 128 * tj:128 * (tj + 1)],
                        wo_sb[:, p * D + 512 * n:p * D + 512 * (n + 1)],
                        start=(p == 0), stop=(p == NPAIR - 1))
                if act_evac:
                    # ACT evac + per-half writeback shortens the final
                    # drain (keeps the tail off the DVE queue too)
                    nc.scalar.activation(ob[:, 512 * n:512 * (n + 1)],
                                         ps[:], Copy)
                    nc.sync.dma_start(
                        y[128 * tj:128 * (tj + 1),
                          512 * n:512 * (n + 1)],
                        ob[:, 512 * n:512 * (n + 1)])
                else:
                    nc.vector.tensor_copy(ob[:, 512 * n:512 * (n + 1)],
                                          ps[:])
            if not act_evac:
                nc.sync.dma_start(y[128 * tj:128 * (tj + 1), :], ob[:])

        def emit_norm(p, ch, po, c0, c1):
            w = c1 - c0
            for h in range(2):
                og = nrm.tile([65, 512], f32, tag="og", name="og")
                nc.vector.tensor_copy(og[:, 0:w], po[h][:, c0:c1])
                rec = nrm.tile([1, 512], f32, tag="rec", name="rec")
                nc.vector.reciprocal(rec[:, 0:w], og[64:65, 0:w])
                bc = nrm.tile([64, 512], f32, tag="bc", name="bc")
                nc.gpsimd.partition_broadcast(bc[:, 0:w], rec[:, 0:w])
                nc.vector.tensor_mul(
                    outT[p][64 * h:64 * (h + 1),
                            512 * ch + c0:512 * ch + c1],
                    og[0:64, 0:w], bc[:, 0:w])

        def attn_column(p, ch, fill, tailjobs=None):
            """fill: iterator of emitters to interleave between k-tiles."""
            po = [psO.tile([65, 512], f32, tag=f"po{h}", name=f"po{h}")
                  for h in range(2)]
            nk = 4 * ch + 4

            def emit_pv(kt, pt):
                qo = max(kt - 4 * ch, 0) * 128
                for h in range(2):
                    nc.tensor.matmul(
                        po[h][:, qo:],
                        vaug4[:, 2 * p + h, kt, :],
                        pt[:, h, qo:],
                        start=(kt == 0), stop=(kt == nk - 1),
                        skip_group_check=True)

            pend = []
            for kt in range(nk):
                dg = kt - 4 * ch
                qo = max(dg, 0) * 128
                pt = ptp.tile([128, 2, 512], bf16, tag="pt", name="pt")
                ss = psS.tile([128, 2, 512], f32, tag="psS", name="ss")
                u, e = p // 2, p % 2
                for h in range(2):
                    b0 = 64 * e + 32 * h
                    nc.tensor.matmul(
                        ss[:, h, qo:],
                        qkT8[u][b0:b0 + 32, :, 1,
                                128 * kt:128 * (kt + 1)],
                        qkT8[u][b0:b0 + 32, :, 0,
                                512 * ch + qo:512 * (ch + 1)],
                        start=True, stop=True, perf_mode=DR,
                        tile_position=(b0, 0))
                nc.scalar.activation(pt[:, :, qo:], ss[:, :, qo:],
                                     Exp, scale=0.125 / (QKS * QKS))
                if dg >= 0:
                    # zero the upper (q < k) triangle of the diagonal block
                    blk = pt[:, :, qo:qo + 128]
                    nc.gpsimd.affine_select(
                        out=blk, in_=blk,
                        compare_op=mybir.AluOpType.is_ge,
                        fill=0.0, base=0,
                        pattern=[[0, 2], [1, 128]],
                        channel_multiplier=-1)
                # fill, then 2-tile-deep software-pipelined PV so the
                # PE never waits inline on an exp
                f = next(fill, None)
                if f:
                    f()
                pend.append((kt, pt))
                if len(pend) > 4:
                    emit_pv(*pend.pop(0))
            if tailjobs is None:
                for pv in pend:
                    f = next(fill, None)
                    if f:
                        f()
                    emit_pv(*pv)
                emit_norm(p, ch, po, 0, 512)
            else:
                # last column: po[:, 0:128*q] is complete as soon as the
                # q-th diagonal PV lands -- normalize and project each
                # quarter as it completes, with reserved chunk-2 output
                # tiles filling each normalize chain's latency
                while len(pend) > 2:
                    emit_pv(*pend.pop(0))
                emit_norm(p, ch, po, 0, 128)
                tailjobs[0]()
                tailjobs[4]()
                emit_norm(p, ch, po, 128, 256)
                tailjobs[1]()
                emit_pv(*pend.pop(0))
                tailjobs[5]()
                emit_pv(*pend.pop(0))
                emit_norm(p, ch, po, 256, 384)
                emit_norm(p, ch, po, 384, 512)
                tailjobs[2]()
                tailjobs[6]()
                tailjobs[3]()
                tailjobs[7]()

        # ---------------- schedule ----------------
        # startup DMA order: q weights and x first (in first-use order,
        # split so the first projection chain can start ~4us in)
        H2 = 2048
        nc.sync.dma_start(ws_sb[0][:, 0:H2], Wsec[0][:, 0:H2])
        emit_xdma(0)
        nc.sync.dma_start(ws_sb[1][:, 0:H2], Wsec[1][:, 0:H2])
        nc.sync.dma_start(bq_sb[:], BQ[:])
        nc.sync.dma_start(ws_sb[2][:, 0:H2], Wsec[2][:, 0:H2])
        nc.sync.dma_start(ws_sb[3][:, 0:H2], Wsec[3][:, 0:H2])
        nc.sync.dma_start(bk_sb[:], BK[:])
        nc.sync.dma_start(bv_row[:], BV[:])
        nc.sync.dma_start(ws_sb[0][:, H2:], Wsec[0][:, H2:])
        nc.sync.dma_start(ws_sb[1][:, H2:], Wsec[1][:, H2:])
        nc.sync.dma_start(ws_sb[2][:, H2:], Wsec[2][:, H2:])
        nc.sync.dma_start(ws_sb[3][:, H2:], Wsec[3][:, H2:])
        nc.gpsimd.partition_broadcast(bv_bc[:], bv_row[:])
        nc.vector.memset(vaug4[:, :, :, 64:65], 1.0)

        fill0 = proj_chunk_fill(0)
        # qk p0-1, V weights, qk p2-3, x prefetch for chunk 1, V chains
        for f in fill0[:2]:
            f()
        nc.sync.dma_start(ws_sb[4][:], Wsec[4][:])
        nc.sync.dma_start(ws_sb[5][:], Wsec[5][:])
        for f in fill0[2:4]:
            f()
        emit_xdma(1)
        for f in fill0[4:]:
            f()

        # ---- deadline-packed fill assignment over the k-tile slots ----
        slots = []           # (ch, p, kt) in emission order
        slot_of = {}
        for ch in range(NCH):
            for p in range(NPAIR):
                for kt in range(4 * ch + 4):
                    slot_of[(ch, p, kt)] = len(slots)
                    slots.append((ch, p, kt))
        nslots = len(slots)
        chunk_start = {ch: slot_of[(ch, 0, 0)] for ch in range(NCH)}
        chunk_end = {ch: slot_of[(ch, NPAIR - 1, 4 * ch + 3)]
                     for ch in range(NCH)}

        items = []  # (deadline, avail, fn)
        for ch in range(1, NCH):
            av = chunk_start[ch - 1]
            for p in range(NPAIR):
                dl = slot_of[(ch, p, 0)] - 4
                items.append((dl, av, lambda ch=ch, p=p: proj_qk(ch, p)))
            for tj in range(4 * ch, 4 * (ch + 1)):
                dl = slot_of[(ch, 0, tj)] - 2
                items.append((dl, av, lambda ch=ch, tj=tj: proj_v(ch, tj)))
        for tj in range(4 * (NCH - 2)):
            items.append((nslots - 1, chunk_end[tj // 4] + 1,
                          lambda tj=tj: outproj_tile(tj)))
        items.append((chunk_end[0] + 8, 0, lambda: nc.sync.dma_start(
            wo_sb[:].rearrange("p (t c) -> p t c", t=NPAIR),
            Wo[:].rearrange("(t p) c -> p t c", p=128))))

        assigned = {}
        for dl, av, fn in sorted(items, key=lambda it: -it[0]):
            s = min(dl, nslots - 1)
            while s >= av and s in assigned:
                s -= 1
            if s < av:
                s = av
                while s in assigned:
                    s += 1
            assigned[s] = fn

        fills = {}
        for s, fn in assigned.items():
            fills.setdefault(s, []).append(fn)

        def _slotfill(seq):
            for fl in seq:
                yield fl[0] if fl else None
            while True:
                yield None

        last = NCH - 1
        for ch in range(NCH):
            if 0 < ch and ch + 1 < NCH:
                emit_xdma(ch + 1)
            for p in range(NPAIR):
                nk = 4 * ch + 4
                seq = []
                for kt in range(nk):
                    fl = fills.get(slot_of[(ch, p, kt)], [])
                    seq.append(fl)
                it = _slotfill(seq)
                tailjobs = None
                if ch == last and p == NPAIR - 1:
                    tailjobs = [
                        lambda tj=tj: outproj_tile(tj, act_evac=True)
                        for tj in list(range(4 * (last - 1), 4 * last)) +
                        list(range(4 * last, 4 * last + 4))]
                attn_column(p, ch, it, tailjobs)
    nc.compile()
    return nc


class _Runner:
    def __init__(self, nc):
        import jax
        from jax.sharding import Mesh, PartitionSpec, NamedSharding
        from jax.experimental.shard_map import shard_map
        import concourse.mybir as mybir
        from concourse.bass2jax import (_bass_exec_p, partition_id_tensor,
                                        install_neuronx_cc_hook)
        install_neuronx_cc_hook()
        self.jax = jax
        part = nc.partition_id_tensor.name if nc.partition_id_tensor else None
        in_names, out_names, out_avals = [], [], []
        for alloc in nc.m.functions[0].allocations:
            if not isinstance(alloc, mybir.MemoryLocationSet):
                continue
            name = alloc.memorylocations[0].name
            if alloc.kind == "ExternalInput":
                if name != part:
                    in_names.append(name)
            elif alloc.kind == "ExternalOutput":
                out_names.append(name)
                out_avals.append(jax.core.ShapedArray(
                    tuple(alloc.tensor_shape), mybir.dt.np(alloc.dtype)))
        self.in_names, self.out_names, self.out_avals = in_names, out_names, out_avals
        all_in = list(in_names) + list(out_names) + ([part] if part else [])

        def _body(*args):
            ops = list(args)
            if part:
                ops.append(partition_id_tensor())
            return tuple(_bass_exec_p.bind(
                *ops, out_avals=tuple(out_avals), in_names=tuple(all_in),
                out_names=tuple(out_names), lowering_input_output_aliases=(),
                sim_require_finite=True, sim_require_nnan=True, nc=nc))

        devices = jax.devices()[:NCORES]
        mesh = Mesh(np.asarray(devices), ("core",))
        nin = len(in_names) + len(out_names)
        self.fn = jax.jit(
            shard_map(_body, mesh=mesh,
                      in_specs=(PartitionSpec("core"),) * nin,
                      out_specs=(PartitionSpec("core"),) * len(out_names),
                      check_rep=False),
            keep_unused=True)
        self.sharding = NamedSharding(mesh, PartitionSpec("core"))

    def put_inputs(self, in_maps):
        args = []
        for name in self.in_names:
            cat = np.concatenate([np.asarray(m[name]) for m in in_maps], axis=0)
            args.append(self.jax.device_put(cat, self.sharding))
        for av in self.out_avals:
            z = np.zeros((NCORES * av.shape[0], *av.shape[1:]), av.dtype)
            args.append(self.jax.device_put(z, self.sharding))
        return args

    def run_np(self, args):
        outs = self.fn(*args)
        return [
            {n: np.asarray(outs[i]).reshape(NCORES, *self.out_avals[i].shape)[c]
             for i, n in enumerate(self.out_names)}
            for c in range(NCORES)
        ]


def _get_runner():
    if "r" not in _CACHE:
        nc = _build()
        _CACHE["nc"] = nc
        _CACHE["r"] = _Runner(nc)
    return _CACHE["r"]


def _rne11(a):
    """Round fp32 to 11 mantissa bits, round-to-nearest-even (= hw fp32r)."""
    ai = np.ascontiguousarray(a, dtype=np.float32).view(np.uint32).astype(np.uint64)
    lsb = (ai >> 12) & 1
    out = (((ai + 2047 + lsb) >> 12) << 12).astype(np.uint32)
    return out.view(np.float32)


def _q8(a):
    import ml_dtypes
    a = np.clip(np.asarray(a, np.float32), -240.0, 240.0)
    return a.astype(ml_dtypes.float8_e4m3)


def _pack_bias(b):
    # [512] -> [128, NPAIR] with the (j, h, s) psum-partition order,
    # pre-scaled by the q/k fp8 storage scale
    br = (b * QKS).reshape(NPAIR, 2, 2, 32).transpose(2, 1, 3, 0)
    return np.ascontiguousarray(br.reshape(128, NPAIR).astype(np.float32))


def make_in_maps(x, Wqkv, bqkv, Wo, bo=None, mask=None):
    # x repack: [ch, part, g, i, tok], feature = 256g + 128i + part
    xhs, xls = [], []
    for b in range(B):
        xr = np.ascontiguousarray(
            x[b].reshape(NCH, 512, NG, 2, 128).transpose(0, 4, 2, 3, 1))
        h8 = _q8(xr)
        l8 = _q8(xr - h8.astype(np.float32))
        xhs.append(h8.reshape(NCH, 128, NG * 1024))
        xls.append(l8.reshape(NCH, 128, NG * 1024))

    in_maps = []
    for c in range(NCORES):
        b, g = c // 2, c % 2
        sl = slice(g * FPC, (g + 1) * FPC)

        def pack_qk(w):
            # [d, col] -> [part, p, g, i, f]: d = 256g+128i+part, col = 128p+f
            # and f (psum partition) reordered to (j, h, s):
            # feature-within-pair = 64h + 32j + s  ->  f = 64j + 32h + s
            wr = (w * WSC).reshape(NG, 2, 128, NPAIR, 2, 2, 32).transpose(
                2, 3, 0, 1, 5, 4, 6)
            hi = _q8(wr)
            lo = _q8(wr - hi.astype(np.float32))
            return (hi.reshape(128, 4096), lo.reshape(128, 4096))

        def pack_v(w):
            # [d, col] -> [part, g, i, f]: d = 256g+128i+part, col = f(512)
            wr = (w * WSC).reshape(NG, 2, 128, FPC).transpose(2, 0, 1, 3)
            hi = _q8(wr)
            lo = _q8(wr - hi.astype(np.float32))
            return (hi.reshape(128, 4096), lo.reshape(128, 4096))

        qh, ql = pack_qk(Wqkv[:, 0 * D:1 * D][:, sl])
        kh, kl = pack_qk(Wqkv[:, 1 * D:2 * D][:, sl])
        vh, vl = pack_v(Wqkv[:, 2 * D:3 * D][:, sl])

        in_maps.append({
            "xh": xhs[b], "xl": xls[b],
            "Wqh": qh, "Wql": ql, "Wkh": kh, "Wkl": kl, "Wvh": vh, "Wvl": vl,
            "BQ": _pack_bias(bqkv[0 * D:1 * D][sl]),
            "BK": _pack_bias(bqkv[1 * D:2 * D][sl]),
            "BV": np.ascontiguousarray(bqkv[2 * D:3 * D][sl]),
            "Wo": _rne11(Wo[sl, :]),
        })
    return in_maps


def kernel(x, Wqkv, bqkv, Wo, bo, mask=None, **_unused):
    x = np.asarray(x, dtype=np.float32)
    Wqkv = np.asarray(Wqkv, dtype=np.float32)
    bqkv = np.asarray(bqkv, dtype=np.float32)
    Wo = np.asarray(Wo, dtype=np.float32)
    bo = np.asarray(bo, dtype=np.float32)
    in_maps = make_in_maps(x, Wqkv, bqkv, Wo)
    last_err = None
    for _attempt in range(3):
        try:
            r = _get_runner()
            args = r.put_inputs(in_maps)
            res = r.run_np(args)
            break
        except Exception as e:  # transient device wedge: retry fresh
            last_err = e
            _CACHE.clear()
            import time
            time.sleep(5)
    else:
        raise last_err
    out = np.empty((B, T, D), dtype=np.float32)
    for b in range(B):
        out[b] = res[2 * b]["y"] + res[2 * b + 1]["y"] + bo
    return out


# revision 11
# speedup vs baseline: 1.2491x; 1.0021x over previous
"""Causal self-attention Trainium2 kernel, v2.

Sharding: 8 cores = 4 batches x 2 head-groups (8 heads each).

Per-core dataflow:
  - QKV projections as fp8e4 DoubleRow matmuls (256-feature contraction
    per instruction) with a hi/lo split of both x and W (host-prepared):
    q = xh@Wh + xl@Wh + xh@Wl  -- 3 DoubleRow passes = 6N cycles vs
    fp32r's 8N, with ~0.1% error.
  - q,k stored bf16 (rate-1 matmuls at any N, so causal diagonal blocks
    need no 256-col widening); scores per k-tile into PSUM.
  - exp on ACT writes P^T directly as bf16; causal triangle zeroed on
    GPSIMD (affine_select); PV matmuls in bf16 with a ones-column in the
    V tile accumulating softmax denominators.
  - out = PV / rowsum via DVE reciprocal + GPSIMD partition broadcast.
  - y = outT.T @ Wo in fp32r (partial; host sums the 2 head-groups).

Scheduling: projection chains for chunk ch+1 and output-projection tiles
for chunk ch-1 are interleaved between attention k-tiles of chunk ch so
the PE never idles while ACT paces the softmax.
"""
import numpy as np

B, T, D, H = 4, 2048, 1024, 16
HD = D // H            # 64
NCORES = 8
HPC = 8                # heads per core
FPC = HPC * HD         # 512 feature cols per core
NPAIR = HPC // 2       # 4 head pairs
NG = 4                 # fp8 DoubleRow contraction groups (256 feats each)
KT = T // 128          # 16 k-tiles
NCH = T // 512         # 4 q-chunks
WSC = 50.0             # host weight scaling before fp8 quantization
QKS = 16.0             # q/k fp8 storage scale
VSTR = 65              # per-k-tile stride in vaug free dim
HSTR = KT * VSTR       # per-head stride in vaug free dim

_CACHE = {}


def _build():
    import concourse.mybir as mybir
    import concourse.tile as tile
    from concourse import bacc
    from contextlib import ExitStack

    f32 = mybir.dt.float32
    f32r = mybir.dt.float32r
    bf16 = mybir.dt.bfloat16
    f8 = mybir.dt.float8e4
    DR = mybir.MatmulPerfMode.DoubleRow
    Exp = mybir.ActivationFunctionType.Exp
    Alu = mybir.AluOpType

    nc = bacc.Bacc("TRN2", target_bir_lowering=False, debug=False,
                   num_devices=NCORES)
    # x hi/lo fp8, repacked host-side as [ch, 128, g, i, tok]:
    # feature = 256*g + 128*i + partition
    xh = nc.dram_tensor("xh", [NCH, 128, NG * 1024], f8, kind="ExternalInput")
    xl = nc.dram_tensor("xl", [NCH, 128, NG * 1024], f8, kind="ExternalInput")
    # fp8 weights, 6 tensors: q/k: [p][g][i][f128]; v: [g][i][f512]
    Wsec = [nc.dram_tensor(nm, [128, 4096], f8, kind="ExternalInput")
            for nm in ("Wqh", "Wql", "Wkh", "Wkl", "Wvh", "Wvl")]
    BQ = nc.dram_tensor("BQ", [128, NPAIR], f32, kind="ExternalInput")
    BK = nc.dram_tensor("BK", [128, NPAIR], f32, kind="ExternalInput")
    BV = nc.dram_tensor("BV", [FPC], f32, kind="ExternalInput")
    Wo = nc.dram_tensor("Wo", [FPC, D], f32r, kind="ExternalInput")
    y = nc.dram_tensor("y", [T, D], f32, kind="ExternalOutput")

    with tile.TileContext(nc) as tc, ExitStack() as es:
        pers = es.enter_context(tc.tile_pool(name="pers", bufs=1))
        xsp = es.enter_context(tc.tile_pool(name="xsp", bufs=3))
        ptp = es.enter_context(tc.tile_pool(name="ptp", bufs=10))
        nrm = es.enter_context(tc.tile_pool(name="nrm", bufs=2))
        obp = es.enter_context(tc.tile_pool(name="obp", bufs=3))
        stgp = es.enter_context(tc.tile_pool(name="stgp", bufs=4))
        psA = es.enter_context(tc.tile_pool(name="psA", bufs=2, space="PSUM"))
        psS = es.enter_context(tc.tile_pool(name="psS", bufs=2, space="PSUM"))
        psO = es.enter_context(tc.tile_pool(name="psO", bufs=1, space="PSUM"))

        ws_sb = [pers.tile([128, 4096], f8, tag=f"ws{i}", name=f"ws{i}")
                 for i in range(6)]
        bq_sb = pers.tile([128, NPAIR], f32, tag="bq")
        bk_sb = pers.tile([128, NPAIR], f32, tag="bk")
        bv_row = pers.tile([1, FPC], f32, tag="bvr")
        bv_bc = pers.tile([128, FPC], f32, tag="bvb")
        wo_sb = pers.tile([128, NPAIR * D], f32r, tag="wo")
        vaug = pers.tile([128, HPC * HSTR], bf16, tag="vaug")
        # q/k in fp8 for DoubleRow scores: tile u holds pairs (2u, 2u+1);
        # partition = 64*(pr%2) + 32*head + hd%32, free = (hd-half j,
        # q-or-k, token) -- one tile so each remap DMA moves q and k
        qkT8 = [pers.tile([128, 2, 2, T], f8, tag=f"qkT8{u}",
                          name=f"qkT8{u}") for u in range(2)]
        outT = [pers.tile([128, T], f32r, tag=f"oT{p}", name=f"oT{p}")
                for p in range(NPAIR)]

        vaug4 = vaug[:].rearrange("p (h k x) -> p h k x", h=HPC, k=KT)

        xtiles = {}

        def emit_xdma(ch):
            xh_sb = xsp.tile([128, NG * 1024], f8, tag="xh", name="xh_sb")
            xl_sb = xsp.tile([128, NG * 1024], f8, tag="xl", name="xl_sb")
            if ch == 0:
                half = NG * 512
                nc.sync.dma_start(xh_sb[:, 0:half], xh[ch][:, 0:half])
                nc.sync.dma_start(xl_sb[:, 0:half], xl[ch][:, 0:half])
                nc.sync.dma_start(xh_sb[:, half:], xh[ch][:, half:])
                nc.sync.dma_start(xl_sb[:, half:], xl[ch][:, half:])
            else:
                nc.sync.dma_start(xh_sb[:], xh[ch])
                nc.sync.dma_start(xl_sb[:], xl[ch])
            xtiles[ch] = [xx[:, g * 1024:(g + 1) * 1024].rearrange(
                "p (i t) -> p i t", i=2)
                for xx in (xh_sb, xl_sb) for g in range(NG)]

        def w_ap(sec, p, g):
            base = p * 1024 + g * 256
            return ws_sb[sec][:, base:base + 256].rearrange(
                "p (i f) -> p i f", i=2)

        def wv_ap(sec, g):
            base = g * 1024
            return ws_sb[sec][:, base:base + 1024].rearrange(
                "p (i f) -> p i f", i=2)

        # (x-part, w-section) term order: xl last so its DMA can trail
        # at startup. psum partitions are ordered (hd-half j, head, hd%32)
        # by the host weight packing; both evacs write scaled fp8 into one
        # staging tile whose two j-halves are then DMA'd into the
        # partition-sliced qkT8 layout (one DMA moves q and k together).
        def proj_qk(ch, p):
            xg = xtiles[ch]
            stg = stgp.tile([128, 2, 512], f8, tag="stg", name="stg")
            for qk, (sec, bsb) in enumerate(((0, bq_sb), (2, bk_sb))):
                ps = psA.tile([128, 512], f32, tag="psA", name="psqk")
                n = 0
                for (xt, ws) in ((0, sec), (0, sec + 1), (1, sec)):
                    for g in range(NG):
                        nc.tensor.matmul(
                            ps[:], w_ap(ws, p, g), xg[xt * NG + g],
                            start=(n == 0), stop=(n == 3 * NG - 1),
                            perf_mode=DR)
                        n += 1
                nc.vector.tensor_scalar(
                    stg[:, qk, :], ps[:], QKS / WSC, bsb[:, p:p + 1],
                    Alu.mult, Alu.add)
            u, e = p // 2, p % 2
            for j in range(2):
                nc.sync.dma_start(
                    qkT8[u][64 * e:64 * e + 64, j, :,
                            512 * ch:512 * (ch + 1)],
                    stg[64 * j:64 * j + 64, :, :])

        def proj_v(ch, tj):
            xg = xtiles[ch]
            lo = (tj - 4 * ch) * 128
            ps = psA.tile([128, FPC], f32, tag="psA", name="psv")
            n = 0
            for (xt, ws) in ((0, 4), (0, 5), (1, 4)):
                for g in range(NG):
                    nc.tensor.matmul(
                        ps[:], xg[xt * NG + g][:, :, lo:lo + 128],
                        wv_ap(ws, g),
                        start=(n == 0), stop=(n == 3 * NG - 1),
                        perf_mode=DR)
                    n += 1
            nc.vector.scalar_tensor_tensor(
                vaug4[:, :, tj, 0:64],
                ps[:].rearrange("p (h x) -> p h x", h=HPC),
                1.0 / WSC,
                bv_bc[:].rearrange("p (h x) -> p h x", h=HPC),
                Alu.mult, Alu.add)

        def proj_chunk_fill(ch):
            out = []
            for p in range(NPAIR):
                out.append(lambda p=p: proj_qk(ch, p))
            for tj in range(4 * ch, 4 * (ch + 1)):
                out.append(lambda tj=tj: proj_v(ch, tj))
            return out

        Copy = mybir.ActivationFunctionType.Copy

        def outproj_tile(tj, act_evac=False):
            ob = obp.tile([128, D], f32, tag="ob", name="ob")
            for n in range(2):
                ps = psA.tile([128, 512], f32, tag="psA", name="pspr")
                for p in range(NPAIR):
                    nc.tensor.matmul(
                        ps[:],
                        outT[p][:, 128 * tj:128 * (tj + 1)],
                        wo_sb[:, p * D + 512 * n:p * D + 512 * (n + 1)],
                        start=(p == 0), stop=(p == NPAIR - 1))
                if act_evac:
                    # ACT evac + per-half writeback shortens the final
                    # drain (keeps the tail off the DVE queue too)
                    nc.scalar.activation(ob[:, 512 * n:512 * (n + 1)],
                                         ps[:], Copy)
                    nc.sync.dma_start(
                        y[128 * tj:128 * (tj + 1),
                          512 * n:512 * (n + 1)],
                        ob[:, 512 * n:512 * (n + 1)])
                else:
                    nc.vector.tensor_copy(ob[:, 512 * n:512 * (n + 1)],
                                          ps[:])
            if not act_evac:
                nc.sync.dma_start(y[128 * tj:128 * (tj + 1), :], ob[:])

        def emit_norm(p, ch, po, c0, c1):
            w = c1 - c0
            for h in range(2):
                og = nrm.tile([65, 512], f32, tag="og", name="og")
                nc.vector.tensor_copy(og[:, 0:w], po[h][:, c0:c1])
                rec = nrm.tile([1, 512], f32, tag="rec", name="rec")
                nc.vector.reciprocal(rec[:, 0:w], og[64:65, 0:w])
                bc = nrm.tile([64, 512], f32, tag="bc", name="bc")
                nc.gpsimd.partition_broadcast(bc[:, 0:w], rec[:, 0:w])
                nc.vector.tensor_mul(
                    outT[p][64 * h:64 * (h + 1),
                            512 * ch + c0:512 * ch + c1],
                    og[0:64, 0:w], bc[:, 0:w])

        def attn_column(p, ch, fill, tailjobs=None):
            """fill: iterator of emitters to interleave between k-tiles."""
            po = [psO.tile([65, 512], f32, tag=f"po{h}", name=f"po{h}")
                  for h in range(2)]
            nk = 4 * ch + 4

            def emit_pv(kt, pt):
                qo = max(kt - 4 * ch, 0) * 128
                for h in range(2):
                    nc.tensor.matmul(
                        po[h][:, qo:],
                        vaug4[:, 2 * p + h, kt, :],
                        pt[:, h, qo:],
                        start=(kt == 0), stop=(kt == nk - 1),
                        skip_group_check=True)

            pend = []
            for kt in range(nk):
                dg = kt - 4 * ch
                qo = max(dg, 0) * 128
                pt = ptp.tile([128, 2, 512], bf16, tag="pt", name="pt")
                ss = psS.tile([128, 2, 512], f32, tag="psS", name="ss")
                u, e = p // 2, p % 2
                for h in range(2):
                    b0 = 64 * e + 32 * h
                    nc.tensor.matmul(
                        ss[:, h, qo:],
                        qkT8[u][b0:b0 + 32, :, 1,
                                128 * kt:128 * (kt + 1)],
                        qkT8[u][b0:b0 + 32, :, 0,
                                512 * ch + qo:512 * (ch + 1)],
                        start=True, stop=True, perf_mode=DR,
                        tile_position=(b0, 0))
                nc.scalar.activation(pt[:, :, qo:], ss[:, :, qo:],
                                     Exp, scale=0.125 / (QKS * QKS))
                if dg >= 0:
                    # zero the upper (q < k) triangle of the diagonal block
                    blk = pt[:, :, qo:qo + 128]
                    nc.gpsimd.affine_select(
                        out=blk, in_=blk,
                        compare_op=mybir.AluOpType.is_ge,
                        fill=0.0, base=0,
                        pattern=[[0, 2], [1, 128]],
                        channel_multiplier=-1)
                # fill, then 2-tile-deep software-pipelined PV so the
                # PE never waits inline on an exp
                f = next(fill, None)
                if f:
                    f()
                pend.append((kt, pt))
                if len(pend) > 4:
                    emit_pv(*pend.pop(0))
            if tailjobs is None:
                for pv in pend:
                    f = next(fill, None)
                    if f:
                        f()
                    emit_pv(*pv)
                emit_norm(p, ch, po, 0, 512)
            else:
                # last column: po[:, 0:128*q] is complete as soon as the
                # q-th diagonal PV lands -- normalize and project each
                # quarter as it completes, with reserved chunk-2 output
                # tiles filling each normalize chain's latency
                while len(pend) > 2:
                    emit_pv(*pend.pop(0))
                emit_norm(p, ch, po, 0, 128)
                tailjobs[0]()
                tailjobs[4]()
                emit_norm(p, ch, po, 128, 256)
                tailjobs[1]()
                emit_pv(*pend.pop(0))
                tailjobs[5]()
                emit_pv(*pend.pop(0))
                emit_norm(p, ch, po, 256, 384)
                emit_norm(p, ch, po, 384, 512)
                tailjobs[2]()
                tailjobs[6]()
                tailjobs[3]()
                tailjobs[7]()

        # ---------------- schedule ----------------
        # startup DMA order: q weights and x first (in first-use order,
        # split so the first projection chain can start ~4us in)
        H2 = 2048
        nc.sync.dma_start(ws_sb[0][:, 0:H2], Wsec[0][:, 0:H2])
        emit_xdma(0)
        nc.sync.dma_start(ws_sb[1][:, 0:H2], Wsec[1][:, 0:H2])
        nc.sync.dma_start(bq_sb[:], BQ[:])
        nc.sync.dma_start(ws_sb[2][:, 0:H2], Wsec[2][:, 0:H2])
        nc.sync.dma_start(ws_sb[3][:, 0:H2], Wsec[3][:, 0:H2])
        nc.sync.dma_start(bk_sb[:], BK[:])
        nc.sync.dma_start(bv_row[:], BV[:])
        nc.sync.dma_start(ws_sb[0][:, H2:], Wsec[0][:, H2:])
        nc.sync.dma_start(ws_sb[1][:, H2:], Wsec[1][:, H2:])
        nc.sync.dma_start(ws_sb[2][:, H2:], Wsec[2][:, H2:])
        nc.sync.dma_start(ws_sb[3][:, H2:], Wsec[3][:, H2:])
        nc.gpsimd.partition_broadcast(bv_bc[:], bv_row[:])
        nc.vector.memset(vaug4[:, :, :, 64:65], 1.0)

        fill0 = proj_chunk_fill(0)
        # qk p0-1, V weights, qk p2-3, x prefetch for chunk 1, V chains
        for f in fill0[:2]:
            f()
        nc.sync.dma_start(ws_sb[4][:], Wsec[4][:])
        nc.sync.dma_start(ws_sb[5][:], Wsec[5][:])
        for f in fill0[2:4]:
            f()
        emit_xdma(1)
        for f in fill0[4:]:
            f()

        # ---- deadline-packed fill assignment over the k-tile slots ----
        slots = []           # (ch, p, kt) in emission order
        slot_of = {}
        for ch in range(NCH):
            for p in range(NPAIR):
                for kt in range(4 * ch + 4):
                    slot_of[(ch, p, kt)] = len(slots)
                    slots.append((ch, p, kt))
        nslots = len(slots)
        chunk_start = {ch: slot_of[(ch, 0, 0)] for ch in range(NCH)}
        chunk_end = {ch: slot_of[(ch, NPAIR - 1, 4 * ch + 3)]
                     for ch in range(NCH)}

        items = []  # (deadline, avail, fn)
        for ch in range(1, NCH):
            av = chunk_start[ch - 1]
            for p in range(NPAIR):
                dl = slot_of[(ch, p, 0)] - 4
                items.append((dl, av, lambda ch=ch, p=p: proj_qk(ch, p)))
            for tj in range(4 * ch, 4 * (ch + 1)):
                dl = slot_of[(ch, 0, tj)] - 2
                items.append((dl, av, lambda ch=ch, tj=tj: proj_v(ch, tj)))
        for tj in range(4 * (NCH - 2)):
            items.append((nslots - 1, chunk_end[tj // 4] + 1,
                          lambda tj=tj: outproj_tile(tj)))
        items.append((chunk_end[0] + 8, 0, lambda: nc.sync.dma_start(
            wo_sb[:].rearrange("p (t c) -> p t c", t=NPAIR),
            Wo[:].rearrange("(t p) c -> p t c", p=128))))

        assigned = {}
        for dl, av, fn in sorted(items, key=lambda it: -it[0]):
            s = min(dl, nslots - 1)
            while s >= av and s in assigned:
                s -= 1
            if s < av:
                s = av
                while s in assigned:
                    s += 1
            assigned[s] = fn

        fills = {}
        for s, fn in assigned.items():
            fills.setdefault(s, []).append(fn)

        def _slotfill(seq):
            for fl in seq:
                yield fl[0] if fl else None
            while True:
                yield None

        last = NCH - 1
        for ch in range(NCH):
            if 0 < ch and ch + 1 < NCH:
                emit_xdma(ch + 1)
            for p in range(NPAIR):
                nk = 4 * ch + 4
                seq = []
                for kt in range(nk):
                    fl = fills.get(slot_of[(ch, p, kt)], [])
                    seq.append(fl)
                it = _slotfill(seq)
                tailjobs = None
                if ch == last and p == NPAIR - 1:
                    tailjobs = [
                        lambda tj=tj: outproj_tile(tj, act_evac=True)
                        for tj in list(range(4 * (last - 1), 4 * last)) +
                        list(range(4 * last, 4 * last + 4))]
                attn_column(p, ch, it, tailjobs)
    nc.compile()
    return nc


class _Runner:
    def __init__(self, nc):
        import jax
        from jax.sharding import Mesh, PartitionSpec, NamedSharding
        from jax.experimental.shard_map import shard_map
        import concourse.mybir as mybir
        from concourse.bass2jax import (_bass_exec_p, partition_id_tensor,
                                        install_neuronx_cc_hook)
        install_neuronx_cc_hook()
        self.jax = jax
        part = nc.partition_id_tensor.name if nc.partition_id_tensor else None
        in_names, out_names, out_avals = [], [], []
        for alloc in nc.m.functions[0].allocations:
            if not isinstance(alloc, mybir.MemoryLocationSet):
                continue
            name = alloc.memorylocations[0].name
            if alloc.kind == "ExternalInput":
                if name != part:
                    in_names.append(name)
            elif alloc.kind == "ExternalOutput":
                out_names.append(name)
                out_avals.append(jax.core.ShapedArray(
                    tuple(alloc.tensor_shape), mybir.dt.np(alloc.dtype)))
        self.in_names, self.out_names, self.out_avals = in_names, out_names, out_avals
        all_in = list(in_names) + list(out_names) + ([part] if part else [])

        def _body(*args):
            ops = list(args)
            if part:
                ops.append(partition_id_tensor())
            return tuple(_bass_exec_p.bind(
                *ops, out_avals=tuple(out_avals), in_names=tuple(all_in),
                out_names=tuple(out_names), lowering_input_output_aliases=(),
                sim_require_finite=True, sim_require_nnan=True, nc=nc))

        devices = jax.devices()[:NCORES]
        mesh = Mesh(np.asarray(devices), ("core",))
        nin = len(in_names) + len(out_names)
        self.fn = jax.jit(
            shard_map(_body, mesh=mesh,
                      in_specs=(PartitionSpec("core"),) * nin,
                      out_specs=(PartitionSpec("core"),) * len(out_names),
                      check_rep=False),
            keep_unused=True)
        self.sharding = NamedSharding(mesh, PartitionSpec("core"))

    def put_inputs(self, in_maps):
        args = []
        for name in self.in_names:
            cat = np.concatenate([np.asarray(m[name]) for m in in_maps], axis=0)
            args.append(self.jax.device_put(cat, self.sharding))
        for av in self.out_avals:
            z = np.zeros((NCORES * av.shape[0], *av.shape[1:]), av.dtype)
            args.append(self.jax.device_put(z, self.sharding))
        return args

    def run_np(self, args):
        outs = self.fn(*args)
        return [
            {n: np.asarray(outs[i]).reshape(NCORES, *self.out_avals[i].shape)[c]
             for i, n in enumerate(self.out_names)}
            for c in range(NCORES)
        ]


def _get_runner():
    if "r" not in _CACHE:
        nc = _build()
        _CACHE["nc"] = nc
        _CACHE["r"] = _Runner(nc)
    return _CACHE["r"]


def _rne11(a):
    """Round fp32 to 11 mantissa bits, round-to-nearest-even (= hw fp32r)."""
    ai = np.ascontiguousarray(a, dtype=np.float32).view(np.uint32).astype(np.uint64)
    lsb = (ai >> 12) & 1
    out = (((ai + 2047 + lsb) >> 12) << 12).astype(np.uint32)
    return out.view(np.float32)


def _q8(a):
    import ml_dtypes
    a = np.clip(np.asarray(a, np.float32), -240.0, 240.0)
    return a.astype(ml_dtypes.float8_e4m3)


def _pack_bias(b):
    # [512] -> [128, NPAIR] with the (j, h, s) psum-partition order,
    # pre-scaled by the q/k fp8 storage scale
    br = (b * QKS).reshape(NPAIR, 2, 2, 32).transpose(2, 1, 3, 0)
    return np.ascontiguousarray(br.reshape(128, NPAIR).astype(np.float32))


def make_in_maps(x, Wqkv, bqkv, Wo, bo=None, mask=None):
    # x repack: [ch, part, g, i, tok], feature = 256g + 128i + part
    xhs, xls = [], []
    for b in range(B):
        xr = np.ascontiguousarray(
            x[b].reshape(NCH, 512, NG, 2, 128).transpose(0, 4, 2, 3, 1))
        h8 = _q8(xr)
        l8 = _q8(xr - h8.astype(np.float32))
        xhs.append(h8.reshape(NCH, 128, NG * 1024))
        xls.append(l8.reshape(NCH, 128, NG * 1024))

    in_maps = []
    for c in range(NCORES):
        b, g = c // 2, c % 2
        sl = slice(g * FPC, (g + 1) * FPC)

        def pack_qk(w):
            # [d, col] -> [part, p, g, i, f]: d = 256g+128i+part, col = 128p+f
            # and f (psum partition) reordered to (j, h, s):
            # feature-within-pair = 64h + 32j + s  ->  f = 64j + 32h + s
            wr = (w * WSC).reshape(NG, 2, 128, NPAIR, 2, 2, 32).transpose(
                2, 3, 0, 1, 5, 4, 6)
            hi = _q8(wr)
            lo = _q8(wr - hi.astype(np.float32))
            return (hi.reshape(128, 4096), lo.reshape(128, 4096))

        def pack_v(w):
            # [d, col] -> [part, g, i, f]: d = 256g+128i+part, col = f(512)
            wr = (w * WSC).reshape(NG, 2, 128, FPC).transpose(2, 0, 1, 3)
            hi = _q8(wr)
            lo = _q8(wr - hi.astype(np.float32))
            return (hi.reshape(128, 4096), lo.reshape(128, 4096))

        qh, ql = pack_qk(Wqkv[:, 0 * D:1 * D][:, sl])
        kh, kl = pack_qk(Wqkv[:, 1 * D:2 * D][:, sl])
        vh, vl = pack_v(Wqkv[:, 2 * D:3 * D][:, sl])

        in_maps.append({
            "xh": xhs[b], "xl": xls[b],
            "Wqh": qh, "Wql": ql, "Wkh": kh, "Wkl": kl, "Wvh": vh, "Wvl": vl,
            "BQ": _pack_bias(bqkv[0 * D:1 * D][sl]),
            "BK": _pack_bias(bqkv[1 * D:2 * D][sl]),
            "BV": np.ascontiguousarray(bqkv[2 * D:3 * D][sl]),
            "Wo": _rne11(Wo[sl, :]),
        })
    return in_maps


def kernel(x, Wqkv, bqkv, Wo, bo, mask=None, **_unused):
    x = np.asarray(x, dtype=np.float32)
    Wqkv = np.asarray(Wqkv, dtype=np.float32)
    bqkv = np.asarray(bqkv, dtype=np.float32)
    Wo = np.asarray(Wo, dtype=np.float32)
    bo = np.asarray(bo, dtype=np.float32)
    in_maps = make_in_maps(x, Wqkv, bqkv, Wo)
    last_err = None
    for _attempt in range(3):
        try:
            r = _get_runner()
            args = r.put_inputs(in_maps)
            res = r.run_np(args)
            break
        except Exception as e:  # transient device wedge: retry fresh
            last_err = e
            _CACHE.clear()
            import time
            time.sleep(5)
    else:
        raise last_err
    out = np.empty((B, T, D), dtype=np.float32)
    for b in range(B):
        out[b] = res[2 * b]["y"] + res[2 * b + 1]["y"] + bo
    return out


# revision 12
# speedup vs baseline: 1.2492x; 1.0001x over previous
"""Causal self-attention Trainium2 kernel, v2.

Sharding: 8 cores = 4 batches x 2 head-groups (8 heads each).

Per-core dataflow:
  - QKV projections as fp8e4 DoubleRow matmuls (256-feature contraction
    per instruction) with a hi/lo split of both x and W (host-prepared):
    q = xh@Wh + xl@Wh + xh@Wl  -- 3 DoubleRow passes = 6N cycles vs
    fp32r's 8N, with ~0.1% error.
  - q,k stored bf16 (rate-1 matmuls at any N, so causal diagonal blocks
    need no 256-col widening); scores per k-tile into PSUM.
  - exp on ACT writes P^T directly as bf16; causal triangle zeroed on
    GPSIMD (affine_select); PV matmuls in bf16 with a ones-column in the
    V tile accumulating softmax denominators.
  - out = PV / rowsum via DVE reciprocal + GPSIMD partition broadcast.
  - y = outT.T @ Wo in fp32r (partial; host sums the 2 head-groups).

Scheduling: projection chains for chunk ch+1 and output-projection tiles
for chunk ch-1 are interleaved between attention k-tiles of chunk ch so
the PE never idles while ACT paces the softmax.
"""
import numpy as np

B, T, D, H = 4, 2048, 1024, 16
HD = D // H            # 64
NCORES = 8
HPC = 8                # heads per core
FPC = HPC * HD         # 512 feature cols per core
NPAIR = HPC // 2       # 4 head pairs
NG = 4                 # fp8 DoubleRow contraction groups (256 feats each)
KT = T // 128          # 16 k-tiles
NCH = T // 512         # 4 q-chunks
WSC = 50.0             # host weight scaling before fp8 quantization
QKS = 16.0             # q/k fp8 storage scale
VSTR = 65              # per-k-tile stride in vaug free dim
HSTR = KT * VSTR       # per-head stride in vaug free dim

_CACHE = {}


def _build():
    import concourse.mybir as mybir
    import concourse.tile as tile
    from concourse import bacc
    from contextlib import ExitStack

    f32 = mybir.dt.float32
    f32r = mybir.dt.float32r
    bf16 = mybir.dt.bfloat16
    f8 = mybir.dt.float8e4
    DR = mybir.MatmulPerfMode.DoubleRow
    Exp = mybir.ActivationFunctionType.Exp
    Alu = mybir.AluOpType

    nc = bacc.Bacc("TRN2", target_bir_lowering=False, debug=False,
                   num_devices=NCORES)
    # x hi/lo fp8, repacked host-side as [ch, 128, g, i, tok]:
    # feature = 256*g + 128*i + partition
    xh = nc.dram_tensor("xh", [NCH, 128, NG * 1024], f8, kind="ExternalInput")
    xl = nc.dram_tensor("xl", [NCH, 128, NG * 1024], f8, kind="ExternalInput")
    # fp8 weights, 6 tensors: q/k: [p][g][i][f128]; v: [g][i][f512]
    Wsec = [nc.dram_tensor(nm, [128, 4096], f8, kind="ExternalInput")
            for nm in ("Wqh", "Wql", "Wkh", "Wkl", "Wvh", "Wvl")]
    BQ = nc.dram_tensor("BQ", [128, NPAIR], f32, kind="ExternalInput")
    BK = nc.dram_tensor("BK", [128, NPAIR], f32, kind="ExternalInput")
    BV = nc.dram_tensor("BV", [FPC], f32, kind="ExternalInput")
    Wo = nc.dram_tensor("Wo", [FPC, D], f32r, kind="ExternalInput")
    y = nc.dram_tensor("y", [T, D], f32, kind="ExternalOutput")

    with tile.TileContext(nc) as tc, ExitStack() as es:
        pers = es.enter_context(tc.tile_pool(name="pers", bufs=1))
        xsp = es.enter_context(tc.tile_pool(name="xsp", bufs=3))
        ptp = es.enter_context(tc.tile_pool(name="ptp", bufs=10))
        nrm = es.enter_context(tc.tile_pool(name="nrm", bufs=2))
        obp = es.enter_context(tc.tile_pool(name="obp", bufs=4))
        stgp = es.enter_context(tc.tile_pool(name="stgp", bufs=4))
        psA = es.enter_context(tc.tile_pool(name="psA", bufs=2, space="PSUM"))
        psS = es.enter_context(tc.tile_pool(name="psS", bufs=2, space="PSUM"))
        psO = es.enter_context(tc.tile_pool(name="psO", bufs=1, space="PSUM"))

        ws_sb = [pers.tile([128, 4096], f8, tag=f"ws{i}", name=f"ws{i}")
                 for i in range(6)]
        bq_sb = pers.tile([128, NPAIR], f32, tag="bq")
        bk_sb = pers.tile([128, NPAIR], f32, tag="bk")
        bv_row = pers.tile([1, FPC], f32, tag="bvr")
        bv_bc = pers.tile([128, FPC], f32, tag="bvb")
        wo_sb = pers.tile([128, NPAIR * D], f32r, tag="wo")
        vaug = pers.tile([128, HPC * HSTR], bf16, tag="vaug")
        # q/k in fp8 for DoubleRow scores: tile u holds pairs (2u, 2u+1);
        # partition = 64*(pr%2) + 32*head + hd%32, free = (hd-half j,
        # q-or-k, token) -- one tile so each remap DMA moves q and k
        qkT8 = [pers.tile([128, 2, 2, T], f8, tag=f"qkT8{u}",
                          name=f"qkT8{u}") for u in range(2)]
        outT = [pers.tile([128, T], f32r, tag=f"oT{p}", name=f"oT{p}")
                for p in range(NPAIR)]

        vaug4 = vaug[:].rearrange("p (h k x) -> p h k x", h=HPC, k=KT)

        xtiles = {}

        def emit_xdma(ch):
            xh_sb = xsp.tile([128, NG * 1024], f8, tag="xh", name="xh_sb")
            xl_sb = xsp.tile([128, NG * 1024], f8, tag="xl", name="xl_sb")
            if ch == 0:
                half = NG * 512
                nc.sync.dma_start(xh_sb[:, 0:half], xh[ch][:, 0:half])
                nc.sync.dma_start(xl_sb[:, 0:half], xl[ch][:, 0:half])
                nc.sync.dma_start(xh_sb[:, half:], xh[ch][:, half:])
                nc.sync.dma_start(xl_sb[:, half:], xl[ch][:, half:])
            else:
                nc.sync.dma_start(xh_sb[:], xh[ch])
                nc.sync.dma_start(xl_sb[:], xl[ch])
            xtiles[ch] = [xx[:, g * 1024:(g + 1) * 1024].rearrange(
                "p (i t) -> p i t", i=2)
                for xx in (xh_sb, xl_sb) for g in range(NG)]

        def w_ap(sec, p, g):
            base = p * 1024 + g * 256
            return ws_sb[sec][:, base:base + 256].rearrange(
                "p (i f) -> p i f", i=2)

        def wv_ap(sec, g):
            base = g * 1024
            return ws_sb[sec][:, base:base + 1024].rearrange(
                "p (i f) -> p i f", i=2)

        # (x-part, w-section) term order: xl last so its DMA can trail
        # at startup. psum partitions are ordered (hd-half j, head, hd%32)
        # by the host weight packing; both evacs write scaled fp8 into one
        # staging tile whose two j-halves are then DMA'd into the
        # partition-sliced qkT8 layout (one DMA moves q and k together).
        def proj_qk(ch, p):
            xg = xtiles[ch]
            stg = stgp.tile([128, 2, 512], f8, tag="stg", name="stg")
            for qk, (sec, bsb) in enumerate(((0, bq_sb), (2, bk_sb))):
                ps = psA.tile([128, 512], f32, tag="psA", name="psqk")
                n = 0
                for (xt, ws) in ((0, sec), (0, sec + 1), (1, sec)):
                    for g in range(NG):
                        nc.tensor.matmul(
                            ps[:], w_ap(ws, p, g), xg[xt * NG + g],
                            start=(n == 0), stop=(n == 3 * NG - 1),
                            perf_mode=DR)
                        n += 1
                nc.vector.tensor_scalar(
                    stg[:, qk, :], ps[:], QKS / WSC, bsb[:, p:p + 1],
                    Alu.mult, Alu.add)
            u, e = p // 2, p % 2
            for j in range(2):
                nc.sync.dma_start(
                    qkT8[u][64 * e:64 * e + 64, j, :,
                            512 * ch:512 * (ch + 1)],
                    stg[64 * j:64 * j + 64, :, :])

        def proj_v(ch, tj):
            xg = xtiles[ch]
            lo = (tj - 4 * ch) * 128
            ps = psA.tile([128, FPC], f32, tag="psA", name="psv")
            n = 0
            for (xt, ws) in ((0, 4), (0, 5), (1, 4)):
                for g in range(NG):
                    nc.tensor.matmul(
                        ps[:], xg[xt * NG + g][:, :, lo:lo + 128],
                        wv_ap(ws, g),
                        start=(n == 0), stop=(n == 3 * NG - 1),
                        perf_mode=DR)
                    n += 1
            nc.vector.scalar_tensor_tensor(
                vaug4[:, :, tj, 0:64],
                ps[:].rearrange("p (h x) -> p h x", h=HPC),
                1.0 / WSC,
                bv_bc[:].rearrange("p (h x) -> p h x", h=HPC),
                Alu.mult, Alu.add)

        def proj_chunk_fill(ch):
            out = []
            for p in range(NPAIR):
                out.append(lambda p=p: proj_qk(ch, p))
            for tj in range(4 * ch, 4 * (ch + 1)):
                out.append(lambda tj=tj: proj_v(ch, tj))
            return out

        Copy = mybir.ActivationFunctionType.Copy

        def outproj_tile(tj, act_evac=False):
            ob = obp.tile([128, D], f32, tag="ob", name="ob")
            for n in range(2):
                ps = psA.tile([128, 512], f32, tag="psA", name="pspr")
                for p in range(NPAIR):
                    nc.tensor.matmul(
                        ps[:],
                        outT[p][:, 128 * tj:128 * (tj + 1)],
                        wo_sb[:, p * D + 512 * n:p * D + 512 * (n + 1)],
                        start=(p == 0), stop=(p == NPAIR - 1))
                if act_evac:
                    # ACT evac + per-half writeback shortens the final
                    # drain (keeps the tail off the DVE queue too)
                    nc.scalar.activation(ob[:, 512 * n:512 * (n + 1)],
                                         ps[:], Copy)
                    nc.sync.dma_start(
                        y[128 * tj:128 * (tj + 1),
                          512 * n:512 * (n + 1)],
                        ob[:, 512 * n:512 * (n + 1)])
                else:
                    nc.vector.tensor_copy(ob[:, 512 * n:512 * (n + 1)],
                                          ps[:])
            if not act_evac:
                nc.sync.dma_start(y[128 * tj:128 * (tj + 1), :], ob[:])

        def emit_norm(p, ch, po, c0, c1):
            w = c1 - c0
            for h in range(2):
                og = nrm.tile([65, 512], f32, tag="og", name="og")
                nc.vector.tensor_copy(og[:, 0:w], po[h][:, c0:c1])
                rec = nrm.tile([1, 512], f32, tag="rec", name="rec")
                nc.vector.reciprocal(rec[:, 0:w], og[64:65, 0:w])
                bc = nrm.tile([64, 512], f32, tag="bc", name="bc")
                nc.gpsimd.partition_broadcast(bc[:, 0:w], rec[:, 0:w])
                nc.vector.tensor_mul(
                    outT[p][64 * h:64 * (h + 1),
                            512 * ch + c0:512 * ch + c1],
                    og[0:64, 0:w], bc[:, 0:w])

        def attn_column(p, ch, fill, tailjobs=None):
            """fill: iterator of emitters to interleave between k-tiles."""
            po = [psO.tile([65, 512], f32, tag=f"po{h}", name=f"po{h}")
                  for h in range(2)]
            nk = 4 * ch + 4

            def emit_pv(kt, pt):
                qo = max(kt - 4 * ch, 0) * 128
                for h in range(2):
                    nc.tensor.matmul(
                        po[h][:, qo:],
                        vaug4[:, 2 * p + h, kt, :],
                        pt[:, h, qo:],
                        start=(kt == 0), stop=(kt == nk - 1),
                        skip_group_check=True)

            pend = []
            for kt in range(nk):
                dg = kt - 4 * ch
                qo = max(dg, 0) * 128
                pt = ptp.tile([128, 2, 512], bf16, tag="pt", name="pt")
                ss = psS.tile([128, 2, 512], f32, tag="psS", name="ss")
                u, e = p // 2, p % 2
                for h in range(2):
                    b0 = 64 * e + 32 * h
                    nc.tensor.matmul(
                        ss[:, h, qo:],
                        qkT8[u][b0:b0 + 32, :, 1,
                                128 * kt:128 * (kt + 1)],
                        qkT8[u][b0:b0 + 32, :, 0,
                                512 * ch + qo:512 * (ch + 1)],
                        start=True, stop=True, perf_mode=DR,
                        tile_position=(b0, 0))
                nc.scalar.activation(pt[:, :, qo:], ss[:, :, qo:],
                                     Exp, scale=0.125 / (QKS * QKS))
                if dg >= 0:
                    # zero the upper (q < k) triangle of the diagonal block
                    blk = pt[:, :, qo:qo + 128]
                    nc.gpsimd.affine_select(
                        out=blk, in_=blk,
                        compare_op=mybir.AluOpType.is_ge,
                        fill=0.0, base=0,
                        pattern=[[0, 2], [1, 128]],
                        channel_multiplier=-1)
                # fill, then 2-tile-deep software-pipelined PV so the
                # PE never waits inline on an exp
                f = next(fill, None)
                if f:
                    f()
                pend.append((kt, pt))
                if len(pend) > 4:
                    emit_pv(*pend.pop(0))
            if tailjobs is None:
                for pv in pend:
                    f = next(fill, None)
                    if f:
                        f()
                    emit_pv(*pv)
                emit_norm(p, ch, po, 0, 512)
            else:
                # last column: po[:, 0:128*q] is complete as soon as the
                # q-th diagonal PV lands -- normalize and project each
                # quarter as it completes, with reserved chunk-2 output
                # tiles filling each normalize chain's latency
                while len(pend) > 2:
                    emit_pv(*pend.pop(0))
                emit_norm(p, ch, po, 0, 128)
                tailjobs[0]()
                tailjobs[4]()
                emit_norm(p, ch, po, 128, 256)
                tailjobs[1]()
                emit_pv(*pend.pop(0))
                tailjobs[5]()
                emit_pv(*pend.pop(0))
                emit_norm(p, ch, po, 256, 384)
                emit_norm(p, ch, po, 384, 512)
                tailjobs[2]()
                tailjobs[6]()
                tailjobs[3]()
                tailjobs[7]()

        # ---------------- schedule ----------------
        # startup DMA order: q weights and x first (in first-use order,
        # split so the first projection chain can start ~4us in)
        H2 = 2048
        nc.sync.dma_start(ws_sb[0][:, 0:H2], Wsec[0][:, 0:H2])
        emit_xdma(0)
        nc.sync.dma_start(ws_sb[1][:, 0:H2], Wsec[1][:, 0:H2])
        nc.sync.dma_start(bq_sb[:], BQ[:])
        nc.sync.dma_start(ws_sb[2][:, 0:H2], Wsec[2][:, 0:H2])
        nc.sync.dma_start(ws_sb[3][:, 0:H2], Wsec[3][:, 0:H2])
        nc.sync.dma_start(bk_sb[:], BK[:])
        nc.sync.dma_start(bv_row[:], BV[:])
        nc.sync.dma_start(ws_sb[0][:, H2:], Wsec[0][:, H2:])
        nc.sync.dma_start(ws_sb[1][:, H2:], Wsec[1][:, H2:])
        nc.sync.dma_start(ws_sb[2][:, H2:], Wsec[2][:, H2:])
        nc.sync.dma_start(ws_sb[3][:, H2:], Wsec[3][:, H2:])
        nc.gpsimd.partition_broadcast(bv_bc[:], bv_row[:])
        nc.vector.memset(vaug4[:, :, :, 64:65], 1.0)

        fill0 = proj_chunk_fill(0)
        # qk p0-1, V weights, qk p2-3, x prefetch for chunk 1, V chains
        for f in fill0[:2]:
            f()
        nc.sync.dma_start(ws_sb[4][:], Wsec[4][:])
        nc.sync.dma_start(ws_sb[5][:], Wsec[5][:])
        for f in fill0[2:4]:
            f()
        emit_xdma(1)
        for f in fill0[4:]:
            f()

        # ---- deadline-packed fill assignment over the k-tile slots ----
        slots = []           # (ch, p, kt) in emission order
        slot_of = {}
        for ch in range(NCH):
            for p in range(NPAIR):
                for kt in range(4 * ch + 4):
                    slot_of[(ch, p, kt)] = len(slots)
                    slots.append((ch, p, kt))
        nslots = len(slots)
        chunk_start = {ch: slot_of[(ch, 0, 0)] for ch in range(NCH)}
        chunk_end = {ch: slot_of[(ch, NPAIR - 1, 4 * ch + 3)]
                     for ch in range(NCH)}

        items = []  # (deadline, avail, fn)
        for ch in range(1, NCH):
            av = chunk_start[ch - 1]
            for p in range(NPAIR):
                dl = slot_of[(ch, p, 0)] - 4
                items.append((dl, av, lambda ch=ch, p=p: proj_qk(ch, p)))
            for tj in range(4 * ch, 4 * (ch + 1)):
                dl = slot_of[(ch, 0, tj)] - 2
                items.append((dl, av, lambda ch=ch, tj=tj: proj_v(ch, tj)))
        for tj in range(4 * (NCH - 2)):
            items.append((nslots - 1, chunk_end[tj // 4] + 1,
                          lambda tj=tj: outproj_tile(tj)))
        items.append((chunk_end[0] + 8, 0, lambda: nc.sync.dma_start(
            wo_sb[:].rearrange("p (t c) -> p t c", t=NPAIR),
            Wo[:].rearrange("(t p) c -> p t c", p=128))))

        assigned = {}
        for dl, av, fn in sorted(items, key=lambda it: -it[0]):
            s = min(dl, nslots - 1)
            while s >= av and s in assigned:
                s -= 1
            if s < av:
                s = av
                while s in assigned:
                    s += 1
            assigned[s] = fn

        fills = {}
        for s, fn in assigned.items():
            fills.setdefault(s, []).append(fn)

        def _slotfill(seq):
            for fl in seq:
                yield fl[0] if fl else None
            while True:
                yield None

        last = NCH - 1
        for ch in range(NCH):
            if 0 < ch and ch + 1 < NCH:
                emit_xdma(ch + 1)
            for p in range(NPAIR):
                nk = 4 * ch + 4
                seq = []
                for kt in range(nk):
                    fl = fills.get(slot_of[(ch, p, kt)], [])
                    seq.append(fl)
                it = _slotfill(seq)
                tailjobs = None
                if ch == last and p == NPAIR - 1:
                    tailjobs = [
                        lambda tj=tj: outproj_tile(tj, act_evac=True)
                        for tj in list(range(4 * (last - 1), 4 * last)) +
                        list(range(4 * last, 4 * last + 4))]
                attn_column(p, ch, it, tailjobs)
    nc.compile()
    return nc


class _Runner:
    def __init__(self, nc):
        import jax
        from jax.sharding import Mesh, PartitionSpec, NamedSharding
        from jax.experimental.shard_map import shard_map
        import concourse.mybir as mybir
        from concourse.bass2jax import (_bass_exec_p, partition_id_tensor,
                                        install_neuronx_cc_hook)
        install_neuronx_cc_hook()
        self.jax = jax
        part = nc.partition_id_tensor.name if nc.partition_id_tensor else None
        in_names, out_names, out_avals = [], [], []
        for alloc in nc.m.functions[0].allocations:
            if not isinstance(alloc, mybir.MemoryLocationSet):
                continue
            name = alloc.memorylocations[0].name
            if alloc.kind == "ExternalInput":
                if name != part:
                    in_names.append(name)
            elif alloc.kind == "ExternalOutput":
                out_names.append(name)
                out_avals.append(jax.core.ShapedArray(
                    tuple(alloc.tensor_shape), mybir.dt.np(alloc.dtype)))
        self.in_names, self.out_names, self.out_avals = in_names, out_names, out_avals
        all_in = list(in_names) + list(out_names) + ([part] if part else [])

        def _body(*args):
            ops = list(args)
            if part:
                ops.append(partition_id_tensor())
            return tuple(_bass_exec_p.bind(
                *ops, out_avals=tuple(out_avals), in_names=tuple(all_in),
                out_names=tuple(out_names), lowering_input_output_aliases=(),
                sim_require_finite=True, sim_require_nnan=True, nc=nc))

        devices = jax.devices()[:NCORES]
        mesh = Mesh(np.asarray(devices), ("core",))
        nin = len(in_names) + len(out_names)
        self.fn = jax.jit(
            shard_map(_body, mesh=mesh,
                      in_specs=(PartitionSpec("core"),) * nin,
                      out_specs=(PartitionSpec("core"),) * len(out_names),
                      check_rep=False),
            keep_unused=True)
        self.sharding = NamedSharding(mesh, PartitionSpec("core"))

    def put_inputs(self, in_maps):
        args = []
        for name in self.in_names:
            cat = np.concatenate([np.asarray(m[name]) for m in in_maps], axis=0)
            args.append(self.jax.device_put(cat, self.sharding))
        for av in self.out_avals:
            z = np.zeros((NCORES * av.shape[0], *av.shape[1:]), av.dtype)
            args.append(self.jax.device_put(z, self.sharding))
        return args

    def run_np(self, args):
        outs = self.fn(*args)
        return [
            {n: np.asarray(outs[i]).reshape(NCORES, *self.out_avals[i].shape)[c]
             for i, n in enumerate(self.out_names)}
            for c in range(NCORES)
        ]


def _get_runner():
    if "r" not in _CACHE:
        nc = _build()
        _CACHE["nc"] = nc
        _CACHE["r"] = _Runner(nc)
    return _CACHE["r"]


def _rne11(a):
    """Round fp32 to 11 mantissa bits, round-to-nearest-even (= hw fp32r)."""
    ai = np.ascontiguousarray(a, dtype=np.float32).view(np.uint32).astype(np.uint64)
    lsb = (ai >> 12) & 1
    out = (((ai + 2047 + lsb) >> 12) << 12).astype(np.uint32)
    return out.view(np.float32)


def _q8(a):
    import ml_dtypes
    a = np.clip(np.asarray(a, np.float32), -240.0, 240.0)
    return a.astype(ml_dtypes.float8_e4m3)


def _pack_bias(b):
    # [512] -> [128, NPAIR] with the (j, h, s) psum-partition order,
    # pre-scaled by the q/k fp8 storage scale
    br = (b * QKS).reshape(NPAIR, 2, 2, 32).transpose(2, 1, 3, 0)
    return np.ascontiguousarray(br.reshape(128, NPAIR).astype(np.float32))


def make_in_maps(x, Wqkv, bqkv, Wo, bo=None, mask=None):
    # x repack: [ch, part, g, i, tok], feature = 256g + 128i + part
    xhs, xls = [], []
    for b in range(B):
        xr = np.ascontiguousarray(
            x[b].reshape(NCH, 512, NG, 2, 128).transpose(0, 4, 2, 3, 1))
        h8 = _q8(xr)
        l8 = _q8(xr - h8.astype(np.float32))
        xhs.append(h8.reshape(NCH, 128, NG * 1024))
        xls.append(l8.reshape(NCH, 128, NG * 1024))

    in_maps = []
    for c in range(NCORES):
        b, g = c // 2, c % 2
        sl = slice(g * FPC, (g + 1) * FPC)

        def pack_qk(w):
            # [d, col] -> [part, p, g, i, f]: d = 256g+128i+part, col = 128p+f
            # and f (psum partition) reordered to (j, h, s):
            # feature-within-pair = 64h + 32j + s  ->  f = 64j + 32h + s
            wr = (w * WSC).reshape(NG, 2, 128, NPAIR, 2, 2, 32).transpose(
                2, 3, 0, 1, 5, 4, 6)
            hi = _q8(wr)
            lo = _q8(wr - hi.astype(np.float32))
            return (hi.reshape(128, 4096), lo.reshape(128, 4096))

        def pack_v(w):
            # [d, col] -> [part, g, i, f]: d = 256g+128i+part, col = f(512)
            wr = (w * WSC).reshape(NG, 2, 128, FPC).transpose(2, 0, 1, 3)
            hi = _q8(wr)
            lo = _q8(wr - hi.astype(np.float32))
            return (hi.reshape(128, 4096), lo.reshape(128, 4096))

        qh, ql = pack_qk(Wqkv[:, 0 * D:1 * D][:, sl])
        kh, kl = pack_qk(Wqkv[:, 1 * D:2 * D][:, sl])
        vh, vl = pack_v(Wqkv[:, 2 * D:3 * D][:, sl])

        in_maps.append({
            "xh": xhs[b], "xl": xls[b],
            "Wqh": qh, "Wql": ql, "Wkh": kh, "Wkl": kl, "Wvh": vh, "Wvl": vl,
            "BQ": _pack_bias(bqkv[0 * D:1 * D][sl]),
            "BK": _pack_bias(bqkv[1 * D:2 * D][sl]),
            "BV": np.ascontiguousarray(bqkv[2 * D:3 * D][sl]),
            "Wo": _rne11(Wo[sl, :]),
        })
    return in_maps


def kernel(x, Wqkv, bqkv, Wo, bo, mask=None, **_unused):
    x = np.asarray(x, dtype=np.float32)
    Wqkv = np.asarray(Wqkv, dtype=np.float32)
    bqkv = np.asarray(bqkv, dtype=np.float32)
    Wo = np.asarray(Wo, dtype=np.float32)
    bo = np.asarray(bo, dtype=np.float32)
    in_maps = make_in_maps(x, Wqkv, bqkv, Wo)
    last_err = None
    for _attempt in range(3):
        try:
            r = _get_runner()
            args = r.put_inputs(in_maps)
            res = r.run_np(args)
            break
        except Exception as e:  # transient device wedge: retry fresh
            last_err = e
            _CACHE.clear()
            import time
            time.sleep(5)
    else:
        raise last_err
    out = np.empty((B, T, D), dtype=np.float32)
    for b in range(B):
        out[b] = res[2 * b]["y"] + res[2 * b + 1]["y"] + bo
    return out


# revision 13
# speedup vs baseline: 1.2497x; 1.0004x over previous
"""Causal self-attention Trainium2 kernel, v2.

Sharding: 8 cores = 4 batches x 2 head-groups (8 heads each).

Per-core dataflow:
  - QKV projections as fp8e4 DoubleRow matmuls (256-feature contraction
    per instruction) with a hi/lo split of both x and W (host-prepared):
    q = xh@Wh + xl@Wh + xh@Wl  -- 3 DoubleRow passes = 6N cycles vs
    fp32r's 8N, with ~0.1% error.
  - q,k stored bf16 (rate-1 matmuls at any N, so causal diagonal blocks
    need no 256-col widening); scores per k-tile into PSUM.
  - exp on ACT writes P^T directly as bf16; causal triangle zeroed on
    GPSIMD (affine_select); PV matmuls in bf16 with a ones-column in the
    V tile accumulating softmax denominators.
  - out = PV / rowsum via DVE reciprocal + GPSIMD partition broadcast.
  - y = outT.T @ Wo in fp32r (partial; host sums the 2 head-groups).

Scheduling: projection chains for chunk ch+1 and output-projection tiles
for chunk ch-1 are interleaved between attention k-tiles of chunk ch so
the PE never idles while ACT paces the softmax.
"""
import numpy as np

B, T, D, H = 4, 2048, 1024, 16
HD = D // H            # 64
NCORES = 8
HPC = 8                # heads per core
FPC = HPC * HD         # 512 feature cols per core
NPAIR = HPC // 2       # 4 head pairs
NG = 4                 # fp8 DoubleRow contraction groups (256 feats each)
KT = T // 128          # 16 k-tiles
NCH = T // 512         # 4 q-chunks
WSC = 50.0             # host weight scaling before fp8 quantization
QKS = 16.0             # q/k fp8 storage scale
VSTR = 65              # per-k-tile stride in vaug free dim
HSTR = KT * VSTR       # per-head stride in vaug free dim

_CACHE = {}


def _build():
    import concourse.mybir as mybir
    import concourse.tile as tile
    from concourse import bacc
    from contextlib import ExitStack

    f32 = mybir.dt.float32
    f32r = mybir.dt.float32r
    bf16 = mybir.dt.bfloat16
    f8 = mybir.dt.float8e4
    DR = mybir.MatmulPerfMode.DoubleRow
    Exp = mybir.ActivationFunctionType.Exp
    Alu = mybir.AluOpType

    nc = bacc.Bacc("TRN2", target_bir_lowering=False, debug=False,
                   num_devices=NCORES)
    # x hi/lo fp8, repacked host-side as [ch, 128, g, i, tok]:
    # feature = 256*g + 128*i + partition
    xh = nc.dram_tensor("xh", [NCH, 128, NG * 1024], f8, kind="ExternalInput")
    xl = nc.dram_tensor("xl", [NCH, 128, NG * 1024], f8, kind="ExternalInput")
    # fp8 weights, 6 tensors: q/k: [p][g][i][f128]; v: [g][i][f512]
    Wsec = [nc.dram_tensor(nm, [128, 4096], f8, kind="ExternalInput")
            for nm in ("Wqh", "Wql", "Wkh", "Wkl", "Wvh", "Wvl")]
    BQ = nc.dram_tensor("BQ", [128, NPAIR], f32, kind="ExternalInput")
    BK = nc.dram_tensor("BK", [128, NPAIR], f32, kind="ExternalInput")
    BV = nc.dram_tensor("BV", [FPC], f32, kind="ExternalInput")
    Wo = nc.dram_tensor("Wo", [FPC, D], f32r, kind="ExternalInput")
    y = nc.dram_tensor("y", [T, D], f32, kind="ExternalOutput")

    with tile.TileContext(nc) as tc, ExitStack() as es:
        pers = es.enter_context(tc.tile_pool(name="pers", bufs=1))
        xsp = es.enter_context(tc.tile_pool(name="xsp", bufs=3))
        ptp = es.enter_context(tc.tile_pool(name="ptp", bufs=10))
        nrm = es.enter_context(tc.tile_pool(name="nrm", bufs=2))
        obp = es.enter_context(tc.tile_pool(name="obp", bufs=5))
        stgp = es.enter_context(tc.tile_pool(name="stgp", bufs=4))
        psA = es.enter_context(tc.tile_pool(name="psA", bufs=2, space="PSUM"))
        psS = es.enter_context(tc.tile_pool(name="psS", bufs=2, space="PSUM"))
        psO = es.enter_context(tc.tile_pool(name="psO", bufs=1, space="PSUM"))

        ws_sb = [pers.tile([128, 4096], f8, tag=f"ws{i}", name=f"ws{i}")
                 for i in range(6)]
        bq_sb = pers.tile([128, NPAIR], f32, tag="bq")
        bk_sb = pers.tile([128, NPAIR], f32, tag="bk")
        bv_row = pers.tile([1, FPC], f32, tag="bvr")
        bv_bc = pers.tile([128, FPC], f32, tag="bvb")
        wo_sb = pers.tile([128, NPAIR * D], f32r, tag="wo")
        vaug = pers.tile([128, HPC * HSTR], bf16, tag="vaug")
        # q/k in fp8 for DoubleRow scores: tile u holds pairs (2u, 2u+1);
        # partition = 64*(pr%2) + 32*head + hd%32, free = (hd-half j,
        # q-or-k, token) -- one tile so each remap DMA moves q and k
        qkT8 = [pers.tile([128, 2, 2, T], f8, tag=f"qkT8{u}",
                          name=f"qkT8{u}") for u in range(2)]
        outT = [pers.tile([128, T], f32r, tag=f"oT{p}", name=f"oT{p}")
                for p in range(NPAIR)]

        vaug4 = vaug[:].rearrange("p (h k x) -> p h k x", h=HPC, k=KT)

        xtiles = {}

        def emit_xdma(ch):
            xh_sb = xsp.tile([128, NG * 1024], f8, tag="xh", name="xh_sb")
            xl_sb = xsp.tile([128, NG * 1024], f8, tag="xl", name="xl_sb")
            if ch == 0:
                half = NG * 512
                nc.sync.dma_start(xh_sb[:, 0:half], xh[ch][:, 0:half])
                nc.sync.dma_start(xl_sb[:, 0:half], xl[ch][:, 0:half])
                nc.sync.dma_start(xh_sb[:, half:], xh[ch][:, half:])
                nc.sync.dma_start(xl_sb[:, half:], xl[ch][:, half:])
            else:
                nc.sync.dma_start(xh_sb[:], xh[ch])
                nc.sync.dma_start(xl_sb[:], xl[ch])
            xtiles[ch] = [xx[:, g * 1024:(g + 1) * 1024].rearrange(
                "p (i t) -> p i t", i=2)
                for xx in (xh_sb, xl_sb) for g in range(NG)]

        def w_ap(sec, p, g):
            base = p * 1024 + g * 256
            return ws_sb[sec][:, base:base + 256].rearrange(
                "p (i f) -> p i f", i=2)

        def wv_ap(sec, g):
            base = g * 1024
            return ws_sb[sec][:, base:base + 1024].rearrange(
                "p (i f) -> p i f", i=2)

        # (x-part, w-section) term order: xl last so its DMA can trail
        # at startup. psum partitions are ordered (hd-half j, head, hd%32)
        # by the host weight packing; both evacs write scaled fp8 into one
        # staging tile whose two j-halves are then DMA'd into the
        # partition-sliced qkT8 layout (one DMA moves q and k together).
        def proj_qk(ch, p):
            xg = xtiles[ch]
            stg = stgp.tile([128, 2, 512], f8, tag="stg", name="stg")
            for qk, (sec, bsb) in enumerate(((0, bq_sb), (2, bk_sb))):
                ps = psA.tile([128, 512], f32, tag="psA", name="psqk")
                n = 0
                for (xt, ws) in ((0, sec), (0, sec + 1), (1, sec)):
                    for g in range(NG):
                        nc.tensor.matmul(
                            ps[:], w_ap(ws, p, g), xg[xt * NG + g],
                            start=(n == 0), stop=(n == 3 * NG - 1),
                            perf_mode=DR)
                        n += 1
                nc.vector.tensor_scalar(
                    stg[:, qk, :], ps[:], QKS / WSC, bsb[:, p:p + 1],
                    Alu.mult, Alu.add)
            u, e = p // 2, p % 2
            for j in range(2):
                nc.sync.dma_start(
                    qkT8[u][64 * e:64 * e + 64, j, :,
                            512 * ch:512 * (ch + 1)],
                    stg[64 * j:64 * j + 64, :, :])

        def proj_v(ch, tj):
            xg = xtiles[ch]
            lo = (tj - 4 * ch) * 128
            ps = psA.tile([128, FPC], f32, tag="psA", name="psv")
            n = 0
            for (xt, ws) in ((0, 4), (0, 5), (1, 4)):
                for g in range(NG):
                    nc.tensor.matmul(
                        ps[:], xg[xt * NG + g][:, :, lo:lo + 128],
                        wv_ap(ws, g),
                        start=(n == 0), stop=(n == 3 * NG - 1),
                        perf_mode=DR)
                    n += 1
            nc.vector.scalar_tensor_tensor(
                vaug4[:, :, tj, 0:64],
                ps[:].rearrange("p (h x) -> p h x", h=HPC),
                1.0 / WSC,
                bv_bc[:].rearrange("p (h x) -> p h x", h=HPC),
                Alu.mult, Alu.add)

        def proj_chunk_fill(ch):
            out = []
            for p in range(NPAIR):
                out.append(lambda p=p: proj_qk(ch, p))
            for tj in range(4 * ch, 4 * (ch + 1)):
                out.append(lambda tj=tj: proj_v(ch, tj))
            return out

        Copy = mybir.ActivationFunctionType.Copy

        def outproj_tile(tj, act_evac=False):
            ob = obp.tile([128, D], f32, tag="ob", name="ob")
            for n in range(2):
                ps = psA.tile([128, 512], f32, tag="psA", name="pspr")
                for p in range(NPAIR):
                    nc.tensor.matmul(
                        ps[:],
                        outT[p][:, 128 * tj:128 * (tj + 1)],
                        wo_sb[:, p * D + 512 * n:p * D + 512 * (n + 1)],
                        start=(p == 0), stop=(p == NPAIR - 1))
                if act_evac:
                    # ACT evac + per-half writeback shortens the final
                    # drain (keeps the tail off the DVE queue too)
                    nc.scalar.activation(ob[:, 512 * n:512 * (n + 1)],
                                         ps[:], Copy)
                    nc.sync.dma_start(
                        y[128 * tj:128 * (tj + 1),
                          512 * n:512 * (n + 1)],
                        ob[:, 512 * n:512 * (n + 1)])
                else:
                    nc.vector.tensor_copy(ob[:, 512 * n:512 * (n + 1)],
                                          ps[:])
            if not act_evac:
                nc.sync.dma_start(y[128 * tj:128 * (tj + 1), :], ob[:])

        def emit_norm(p, ch, po, c0, c1):
            w = c1 - c0
            for h in range(2):
                og = nrm.tile([65, 512], f32, tag="og", name="og")
                nc.vector.tensor_copy(og[:, 0:w], po[h][:, c0:c1])
                rec = nrm.tile([1, 512], f32, tag="rec", name="rec")
                nc.vector.reciprocal(rec[:, 0:w], og[64:65, 0:w])
                bc = nrm.tile([64, 512], f32, tag="bc", name="bc")
                nc.gpsimd.partition_broadcast(bc[:, 0:w], rec[:, 0:w])
                nc.vector.tensor_mul(
                    outT[p][64 * h:64 * (h + 1),
                            512 * ch + c0:512 * ch + c1],
                    og[0:64, 0:w], bc[:, 0:w])

        def attn_column(p, ch, fill, tailjobs=None):
            """fill: iterator of emitters to interleave between k-tiles."""
            po = [psO.tile([65, 512], f32, tag=f"po{h}", name=f"po{h}")
                  for h in range(2)]
            nk = 4 * ch + 4

            def emit_pv(kt, pt):
                qo = max(kt - 4 * ch, 0) * 128
                for h in range(2):
                    nc.tensor.matmul(
                        po[h][:, qo:],
                        vaug4[:, 2 * p + h, kt, :],
                        pt[:, h, qo:],
                        start=(kt == 0), stop=(kt == nk - 1),
                        skip_group_check=True)

            pend = []
            for kt in range(nk):
                dg = kt - 4 * ch
                qo = max(dg, 0) * 128
                pt = ptp.tile([128, 2, 512], bf16, tag="pt", name="pt")
                ss = psS.tile([128, 2, 512], f32, tag="psS", name="ss")
                u, e = p // 2, p % 2
                for h in range(2):
                    b0 = 64 * e + 32 * h
                    nc.tensor.matmul(
                        ss[:, h, qo:],
                        qkT8[u][b0:b0 + 32, :, 1,
                                128 * kt:128 * (kt + 1)],
                        qkT8[u][b0:b0 + 32, :, 0,
                                512 * ch + qo:512 * (ch + 1)],
                        start=True, stop=True, perf_mode=DR,
                        tile_position=(b0, 0))
                nc.scalar.activation(pt[:, :, qo:], ss[:, :, qo:],
                                     Exp, scale=0.125 / (QKS * QKS))
                if dg >= 0:
                    # zero the upper (q < k) triangle of the diagonal block
                    blk = pt[:, :, qo:qo + 128]
                    nc.gpsimd.affine_select(
                        out=blk, in_=blk,
                        compare_op=mybir.AluOpType.is_ge,
                        fill=0.0, base=0,
                        pattern=[[0, 2], [1, 128]],
                        channel_multiplier=-1)
                # fill, then 2-tile-deep software-pipelined PV so the
                # PE never waits inline on an exp
                f = next(fill, None)
                if f:
                    f()
                pend.append((kt, pt))
                if len(pend) > 4:
                    emit_pv(*pend.pop(0))
            if tailjobs is None:
                for pv in pend:
                    f = next(fill, None)
                    if f:
                        f()
                    emit_pv(*pv)
                emit_norm(p, ch, po, 0, 512)
            else:
                # last column: po[:, 0:128*q] is complete as soon as the
                # q-th diagonal PV lands -- normalize and project each
                # quarter as it completes, with reserved chunk-2 output
                # tiles filling each normalize chain's latency
                while len(pend) > 2:
                    emit_pv(*pend.pop(0))
                emit_norm(p, ch, po, 0, 128)
                tailjobs[0]()
                tailjobs[4]()
                emit_norm(p, ch, po, 128, 256)
                tailjobs[1]()
                emit_pv(*pend.pop(0))
                tailjobs[5]()
                emit_pv(*pend.pop(0))
                emit_norm(p, ch, po, 256, 384)
                emit_norm(p, ch, po, 384, 512)
                tailjobs[2]()
                tailjobs[6]()
                tailjobs[3]()
                tailjobs[7]()

        # ---------------- schedule ----------------
        # startup DMA order: q weights and x first (in first-use order,
        # split so the first projection chain can start ~4us in)
        H2 = 2048
        nc.sync.dma_start(ws_sb[0][:, 0:H2], Wsec[0][:, 0:H2])
        emit_xdma(0)
        nc.sync.dma_start(ws_sb[1][:, 0:H2], Wsec[1][:, 0:H2])
        nc.sync.dma_start(bq_sb[:], BQ[:])
        nc.sync.dma_start(ws_sb[2][:, 0:H2], Wsec[2][:, 0:H2])
        nc.sync.dma_start(ws_sb[3][:, 0:H2], Wsec[3][:, 0:H2])
        nc.sync.dma_start(bk_sb[:], BK[:])
        nc.sync.dma_start(bv_row[:], BV[:])
        nc.sync.dma_start(ws_sb[0][:, H2:], Wsec[0][:, H2:])
        nc.sync.dma_start(ws_sb[1][:, H2:], Wsec[1][:, H2:])
        nc.sync.dma_start(ws_sb[2][:, H2:], Wsec[2][:, H2:])
        nc.sync.dma_start(ws_sb[3][:, H2:], Wsec[3][:, H2:])
        nc.gpsimd.partition_broadcast(bv_bc[:], bv_row[:])
        nc.vector.memset(vaug4[:, :, :, 64:65], 1.0)

        fill0 = proj_chunk_fill(0)
        # qk p0-1, V weights, qk p2-3, x prefetch for chunk 1, V chains
        for f in fill0[:2]:
            f()
        nc.sync.dma_start(ws_sb[4][:], Wsec[4][:])
        nc.sync.dma_start(ws_sb[5][:], Wsec[5][:])
        for f in fill0[2:4]:
            f()
        emit_xdma(1)
        for f in fill0[4:]:
            f()

        # ---- deadline-packed fill assignment over the k-tile slots ----
        slots = []           # (ch, p, kt) in emission order
        slot_of = {}
        for ch in range(NCH):
            for p in range(NPAIR):
                for kt in range(4 * ch + 4):
                    slot_of[(ch, p, kt)] = len(slots)
                    slots.append((ch, p, kt))
        nslots = len(slots)
        chunk_start = {ch: slot_of[(ch, 0, 0)] for ch in range(NCH)}
        chunk_end = {ch: slot_of[(ch, NPAIR - 1, 4 * ch + 3)]
                     for ch in range(NCH)}

        items = []  # (deadline, avail, fn)
        for ch in range(1, NCH):
            av = chunk_start[ch - 1]
            for p in range(NPAIR):
                dl = slot_of[(ch, p, 0)] - 4
                items.append((dl, av, lambda ch=ch, p=p: proj_qk(ch, p)))
            for tj in range(4 * ch, 4 * (ch + 1)):
                dl = slot_of[(ch, 0, tj)] - 2
                items.append((dl, av, lambda ch=ch, tj=tj: proj_v(ch, tj)))
        for tj in range(4 * (NCH - 2)):
            items.append((nslots - 1, chunk_end[tj // 4] + 1,
                          lambda tj=tj: outproj_tile(tj)))
        items.append((chunk_end[0] + 8, 0, lambda: nc.sync.dma_start(
            wo_sb[:].rearrange("p (t c) -> p t c", t=NPAIR),
            Wo[:].rearrange("(t p) c -> p t c", p=128))))

        assigned = {}
        for dl, av, fn in sorted(items, key=lambda it: -it[0]):
            s = min(dl, nslots - 1)
            while s >= av and s in assigned:
                s -= 1
            if s < av:
                s = av
                while s in assigned:
                    s += 1
            assigned[s] = fn

        fills = {}
        for s, fn in assigned.items():
            fills.setdefault(s, []).append(fn)

        def _slotfill(seq):
            for fl in seq:
                yield fl[0] if fl else None
            while True:
                yield None

        last = NCH - 1
        for ch in range(NCH):
            if 0 < ch and ch + 1 < NCH:
                emit_xdma(ch + 1)
            for p in range(NPAIR):
                nk = 4 * ch + 4
                seq = []
                for kt in range(nk):
                    fl = fills.get(slot_of[(ch, p, kt)], [])
                    seq.append(fl)
                it = _slotfill(seq)
                tailjobs = None
                if ch == last and p == NPAIR - 1:
                    tailjobs = [
                        lambda tj=tj: outproj_tile(tj, act_evac=True)
                        for tj in list(range(4 * (last - 1), 4 * last)) +
                        list(range(4 * last, 4 * last + 4))]
                attn_column(p, ch, it, tailjobs)
    nc.compile()
    return nc


class _Runner:
    def __init__(self, nc):
        import jax
        from jax.sharding import Mesh, PartitionSpec, NamedSharding
        from jax.experimental.shard_map import shard_map
        import concourse.mybir as mybir
        from concourse.bass2jax import (_bass_exec_p, partition_id_tensor,
                                        install_neuronx_cc_hook)
        install_neuronx_cc_hook()
        self.jax = jax
        part = nc.partition_id_tensor.name if nc.partition_id_tensor else None
        in_names, out_names, out_avals = [], [], []
        for alloc in nc.m.functions[0].allocations:
            if not isinstance(alloc, mybir.MemoryLocationSet):
                continue
            name = alloc.memorylocations[0].name
            if alloc.kind == "ExternalInput":
                if name != part:
                    in_names.append(name)
            elif alloc.kind == "ExternalOutput":
                out_names.append(name)
                out_avals.append(jax.core.ShapedArray(
                    tuple(alloc.tensor_shape), mybir.dt.np(alloc.dtype)))
        self.in_names, self.out_names, self.out_avals = in_names, out_names, out_avals
        all_in = list(in_names) + list(out_names) + ([part] if part else [])

        def _body(*args):
            ops = list(args)
            if part:
                ops.append(partition_id_tensor())
            return tuple(_bass_exec_p.bind(
                *ops, out_avals=tuple(out_avals), in_names=tuple(all_in),
                out_names=tuple(out_names), lowering_input_output_aliases=(),
                sim_require_finite=True, sim_require_nnan=True, nc=nc))

        devices = jax.devices()[:NCORES]
        mesh = Mesh(np.asarray(devices), ("core",))
        nin = len(in_names) + len(out_names)
        self.fn = jax.jit(
            shard_map(_body, mesh=mesh,
                      in_specs=(PartitionSpec("core"),) * nin,
                      out_specs=(PartitionSpec("core"),) * len(out_names),
                      check_rep=False),
            keep_unused=True)
        self.sharding = NamedSharding(mesh, PartitionSpec("core"))

    def put_inputs(self, in_maps):
        args = []
        for name in self.in_names:
            cat = np.concatenate([np.asarray(m[name]) for m in in_maps], axis=0)
            args.append(self.jax.device_put(cat, self.sharding))
        for av in self.out_avals:
            z = np.zeros((NCORES * av.shape[0], *av.shape[1:]), av.dtype)
            args.append(self.jax.device_put(z, self.sharding))
        return args

    def run_np(self, args):
        outs = self.fn(*args)
        return [
            {n: np.asarray(outs[i]).reshape(NCORES, *self.out_avals[i].shape)[c]
             for i, n in enumerate(self.out_names)}
            for c in range(NCORES)
        ]


def _get_runner():
    if "r" not in _CACHE:
        nc = _build()
        _CACHE["nc"] = nc
        _CACHE["r"] = _Runner(nc)
    return _CACHE["r"]


def _rne11(a):
    """Round fp32 to 11 mantissa bits, round-to-nearest-even (= hw fp32r)."""
    ai = np.ascontiguousarray(a, dtype=np.float32).view(np.uint32).astype(np.uint64)
    lsb = (ai >> 12) & 1
    out = (((ai + 2047 + lsb) >> 12) << 12).astype(np.uint32)
    return out.view(np.float32)


def _q8(a):
    import ml_dtypes
    a = np.clip(np.asarray(a, np.float32), -240.0, 240.0)
    return a.astype(ml_dtypes.float8_e4m3)


def _pack_bias(b):
    # [512] -> [128, NPAIR] with the (j, h, s) psum-partition order,
    # pre-scaled by the q/k fp8 storage scale
    br = (b * QKS).reshape(NPAIR, 2, 2, 32).transpose(2, 1, 3, 0)
    return np.ascontiguousarray(br.reshape(128, NPAIR).astype(np.float32))


def make_in_maps(x, Wqkv, bqkv, Wo, bo=None, mask=None):
    # x repack: [ch, part, g, i, tok], feature = 256g + 128i + part
    xhs, xls = [], []
    for b in range(B):
        xr = np.ascontiguousarray(
            x[b].reshape(NCH, 512, NG, 2, 128).transpose(0, 4, 2, 3, 1))
        h8 = _q8(xr)
        l8 = _q8(xr - h8.astype(np.float32))
        xhs.append(h8.reshape(NCH, 128, NG * 1024))
        xls.append(l8.reshape(NCH, 128, NG * 1024))

    in_maps = []
    for c in range(NCORES):
        b, g = c // 2, c % 2
        sl = slice(g * FPC, (g + 1) * FPC)

        def pack_qk(w):
            # [d, col] -> [part, p, g, i, f]: d = 256g+128i+part, col = 128p+f
            # and f (psum partition) reordered to (j, h, s):
            # feature-within-pair = 64h + 32j + s  ->  f = 64j + 32h + s
            wr = (w * WSC).reshape(NG, 2, 128, NPAIR, 2, 2, 32).transpose(
                2, 3, 0, 1, 5, 4, 6)
            hi = _q8(wr)
            lo = _q8(wr - hi.astype(np.float32))
            return (hi.reshape(128, 4096), lo.reshape(128, 4096))

        def pack_v(w):
            # [d, col] -> [part, g, i, f]: d = 256g+128i+part, col = f(512)
            wr = (w * WSC).reshape(NG, 2, 128, FPC).transpose(2, 0, 1, 3)
            hi = _q8(wr)
            lo = _q8(wr - hi.astype(np.float32))
            return (hi.reshape(128, 4096), lo.reshape(128, 4096))

        qh, ql = pack_qk(Wqkv[:, 0 * D:1 * D][:, sl])
        kh, kl = pack_qk(Wqkv[:, 1 * D:2 * D][:, sl])
        vh, vl = pack_v(Wqkv[:, 2 * D:3 * D][:, sl])

        in_maps.append({
            "xh": xhs[b], "xl": xls[b],
            "Wqh": qh, "Wql": ql, "Wkh": kh, "Wkl": kl, "Wvh": vh, "Wvl": vl,
            "BQ": _pack_bias(bqkv[0 * D:1 * D][sl]),
            "BK": _pack_bias(bqkv[1 * D:2 * D][sl]),
            "BV": np.ascontiguousarray(bqkv[2 * D:3 * D][sl]),
            "Wo": _rne11(Wo[sl, :]),
        })
    return in_maps


def kernel(x, Wqkv, bqkv, Wo, bo, mask=None, **_unused):
    x = np.asarray(x, dtype=np.float32)
    Wqkv = np.asarray(Wqkv, dtype=np.float32)
    bqkv = np.asarray(bqkv, dtype=np.float32)
    Wo = np.asarray(Wo, dtype=np.float32)
    bo = np.asarray(bo, dtype=np.float32)
    in_maps = make_in_maps(x, Wqkv, bqkv, Wo)
    last_err = None
    for _attempt in range(3):
        try:
            r = _get_runner()
            args = r.put_inputs(in_maps)
            res = r.run_np(args)
            break
        except Exception as e:  # transient device wedge: retry fresh
            last_err = e
            _CACHE.clear()
            import time
            time.sleep(5)
    else:
        raise last_err
    out = np.empty((B, T, D), dtype=np.float32)
    for b in range(B):
        out[b] = res[2 * b]["y"] + res[2 * b + 1]["y"] + bo
    return out
